# revision 1
# baseline (speedup 1.0000x reference)
"""Causal multi-head attention block (B=2, S=2048, D=1024, H=16) on 8 TRN2 cores.

Sharding: core i handles batch b = i//4 and head group hg = i%4 (4 heads =
256 model dims). Each core computes its heads' attention and a partial
output projection; the host sums the 4 partials per batch and adds b_out.

Per-core device pipeline (bf16 matmuls, fp32 PSUM accumulation):
  1. QKV. Q^T,K^T land as [head_cols, tokens] (lhsT = W, rhs = x^T);
     V lands as [tokens, head_cols] (lhsT = x^T tiles, rhs = W_v), stored
     augmented with a ones column so the z-matmul also produces softmax
     row sums.
  2. Attention per head, flash-style in the S^T = K.Q^T orientation over
     the causal lower triangle only: S^T[k_tile, q_span] -> exp on ScalarE
     (scale=1/8, no max subtraction; logits ~N(0,1)) -> P^T bf16 ->
     multiplicative 0/1 mask on diagonal blocks -> z[q_tile, 65]
     += P^T_chunk^T @ V_aug accumulated over k tiles in PSUM. The [q, d+1]
     z orientation makes each z matmul only 65 PE columns (vs a full
     q-span) and puts the softmax row sum in PSUM column 64 of the same
     partition as its query, so normalization is a per-partition
     tensor_scalar multiply fused into the PSUM->SBUF copy.
  3. z[q,d] tiles are transposed back to z^T[d,q] via PE transpose
     (identity matmul, 128 cols per 2-head tile) for the out-projection.
  4. Out-proj: y_partial[t, n] accumulated over the 256 local dims.

Program order is a fine-grained software pipeline: the attention loop is
a flat sequence over (q-quarter, head, k-group) with the z matmuls
lagging one group behind the S matmuls, and a filler queue (next token
group's QKV chains, previous quarters' out-proj chunks) drained between
S and z so the PE never waits on ScalarE exp. Host pre-packs all inputs
into SBUF layouts (bf16); the V bias is folded into the output bias on
the host (b_v @ w_out).
"""

import numpy as np
import ml_dtypes

import concourse.mybir as mybir
import concourse.tile as tile
from concourse import bacc
from concourse.bass_utils import run_bass_kernel_spmd

B = 2
S = 2048
DM = 1024
HD = 64
HLOC = 4                 # heads per core
CLOC = HLOC * HD         # local model dims (256)
MO = DM // 128           # 8 k-subtiles of the model dim
NKT = S // 128           # 16 key tiles

f32 = mybir.dt.float32
bf16 = mybir.dt.bfloat16
EXP = mybir.ActivationFunctionType.Exp

ACT_NS = 0.8333333333333334
PE_NS = 0.4166666666666667

_CACHE = {}


def _groups(qg, cap=1024):
    """Pack the causal k-tile spans of query quarter qg into exp groups of
    <= cap columns. Returns list of groups; each group is a list of
    (kt, offset_in_group, width)."""
    g0 = qg * 512
    last_kt = 4 * qg + 3
    groups, cur, cum = [], [], 0
    for kt in range(last_kt + 1):
        w = g0 + 512 - max(kt * 128, g0)
        if cum + w > cap:
            groups.append(cur)
            cur, cum = [], 0
        cur.append((kt, cum, w))
        cum += w
    groups.append(cur)
    return groups


def build(pt_bufs=20, zq_bufs=4, y_bufs=4, sreg_w=1024, fill_scale=1.0,
          fill_pad=0.0, dma_splits=(2, 2, 2, 2), tri_engine="dve",
          z_delay=1, defer_v=True, copy_split=False, rr_pop=False,
          pair_s=False, op3_split=False, schr_groups=(0, 0, 0, 0),
          fill_cfg=((1.025, 53.0), (0.995, 56.0), (1.014, 5.0), (0.974, -158.0))):
    nc = bacc.Bacc("TRN2", target_bir_lowering=False, debug=False)

    xT_d = nc.dram_tensor("xT", [128, MO, S], bf16, kind="ExternalInput")
    # wqk grouped per QKV chain (ctj = ct*2+j) so each chain's weights are
    # one contiguous 2KB/partition DMA
    wqk_d = nc.dram_tensor("wqk", [128, 4, MO, 128], bf16, kind="ExternalInput")
    wv_d = nc.dram_tensor("wv", [128, MO, CLOC], bf16, kind="ExternalInput")
    wo_d = nc.dram_tensor("wo", [128, 2, DM], bf16, kind="ExternalInput")
    # consts packed as raw bf16 columns: tri[0:128], identity[128:256],
    # bq[256:260], bk[260:264] (f32 values bit-split across bf16 pairs)
    cst_d = nc.dram_tensor("cst", [128, 264], bf16, kind="ExternalInput")
    y_d = nc.dram_tensor("y", [S, DM], bf16, kind="ExternalOutput")

    with tile.TileContext(nc) as tc:
        with (
            tc.tile_pool(name="consts", bufs=1) as consts,
            tc.tile_pool(name="acts", bufs=1) as apool,
            tc.tile_pool(name="pt", bufs=pt_bufs) as ppool,
            tc.tile_pool(name="zq", bufs=zq_bufs) as zqpool,
            tc.tile_pool(name="norm", bufs=4) as spool,
            tc.tile_pool(name="ycopy", bufs=y_bufs) as ypool,
            # 8 PSUM banks: ps_s 2x[128,1024]=4 (QK logits), ps_z
            # 2x[128,4,65]=2 (z accumulators), ps_b 2x[128,512]=2
            # (QKV / V / out-proj chains and z transposes)
            tc.tile_pool(name="ps_s", bufs=2, space="PSUM") as ps_s,
            tc.tile_pool(name="ps_z", bufs=2, space="PSUM") as ps_z,
            tc.tile_pool(name="ps_b", bufs=2, space="PSUM") as ps_b,
        ):
            csb = consts.tile([128, 264], bf16)
            wqk = consts.tile([128, 4, MO, 128], bf16)
            wv = consts.tile([128, MO, CLOC], bf16)
            wo = consts.tile([128, 2, DM], bf16)
            xT = apool.tile([128, MO, S], bf16)

            # DMA order = consumption order (transfers serialize on the DMA
            # engines): first QKV chain's weights, then xT mo-pairs at the
            # pace the first chain consumes them, then the rest in bulk.
            nc.sync.dma_start(wqk[:, 0, :, :], wqk_d[:, 0, :, :])
            nc.scalar.dma_start(xT[:, 0:2, 0:512], xT_d[:, 0:2, 0:512])
            nc.sync.dma_start(csb[:], cst_d[:])
            nc.scalar.dma_start(xT[:, 2:4, 0:512], xT_d[:, 2:4, 0:512])
            nc.sync.dma_start(wqk[:, 1, :, :], wqk_d[:, 1, :, :])
            nc.scalar.dma_start(xT[:, 4:6, 0:512], xT_d[:, 4:6, 0:512])
            nc.sync.dma_start(wqk[:, 2:4, :, :], wqk_d[:, 2:4, :, :])
            nc.scalar.dma_start(xT[:, 6:8, 0:512], xT_d[:, 6:8, 0:512])
            nc.sync.dma_start(wv[:], wv_d[:])
            nc.scalar.dma_start(xT[:, :, 512:1024], xT_d[:, :, 512:1024])
            nc.sync.dma_start(xT[:, :, 1024:1536], xT_d[:, :, 1024:1536])
            nc.scalar.dma_start(xT[:, :, 1536:2048], xT_d[:, :, 1536:2048])
            nc.sync.dma_start(wo[:], wo_d[:])

            tri = csb[:, 0:128]
            ident = csb[:, 128:256]
            bq_sb = csb[:, 256:260].bitcast(f32)
            bk_sb = csb[:, 260:264].bitcast(f32)

            QT = apool.tile([128, 2, S], bf16)
            KT = apool.tile([128, 2, S], bf16)
            # V augmented: [t-part, kt, h, 0:64] = v dims, col 64 = ones
            VA = apool.tile([128, NKT, HLOC, 72], bf16)
            nc.vector.memset(VA[:, :, :, 64:65], 1.0)
            zT = apool.tile([128, 2, S], bf16)

            # ---- filler queue: PE work units drained while ScalarE exps ----
            fillers = []          # list of (key, pe_ns, thunk)
            fill_debt = [0.0]
            fill_count = [0]      # fill() invocations, for z-pop delay
            z_pushed_at = [0]
            rr_state = [0]

            def _pop_next():
                """z thunks once ScalarE has had time to produce their exp
                inputs (z_delay fill periods after push), then QKV (needed
                by the next quarter anyway), out-proj last (the only filler
                class available during the final ACT-bound quarter)."""
                z_ready = fill_count[0] - z_pushed_at[0] >= z_delay
                if rr_pop:
                    # rotate class preference so short op chunks interleave
                    # with z/v/qkv work and their copies hide
                    base = ("z", "v", "qkv", "op") if z_ready else \
                        ("v", "qkv", "op", "z")
                    r = rr_state[0] % 4
                    order = base[r:] + base[:r] if z_ready else base
                    if z_ready:
                        rr_state[0] += 1
                    for cls in order:
                        if cls == "z" and not z_ready:
                            continue
                        for i, (k, ns, thunk) in enumerate(fillers):
                            if k[0] == cls:
                                return fillers.pop(i)
                    return fillers.pop(0)
                order = ("z", "v", "qkv") if z_ready else ("v", "qkv")
                for cls in order:
                    for i, (k, ns, thunk) in enumerate(fillers):
                        if k[0] == cls:
                            return fillers.pop(i)
                for i, (k, ns, thunk) in enumerate(fillers):
                    if k[0] != "z":
                        return fillers.pop(i)
                return fillers.pop(0)

            def fill(budget_ns):
                fill_count[0] += 1
                budget = budget_ns + fill_debt[0]
                spent = 0.0
                while fillers and spent < budget:
                    _, ns, thunk = _pop_next()
                    thunk()
                    spent += ns
                fill_debt[0] = budget - spent if fillers else 0.0

            def drain(key):
                """Force-emit queued fillers matching key (dependency
                barrier: attention on quarter qg needs all of QKV(tg=qg))."""
                rest = []
                for k, ns, thunk in fillers:
                    if k == key:
                        thunk()
                    else:
                        rest.append((k, ns, thunk))
                fillers[:] = rest

            def drain_class(cls):
                rest = []
                for k, ns, thunk in fillers:
                    if k[0] == cls:
                        thunk()
                    else:
                        rest.append((k, ns, thunk))
                fillers[:] = rest

            def emit_qk_chain(tg, ct, j):
                tsl = slice(tg * 512, (tg + 1) * 512)
                dst, b_sb = ((QT, bq_sb), (KT, bk_sb))[j]
                ps = ps_b.tile([128, 512], f32, tag="b",
                               name=f"qk_{tg}_{ct}_{j}")
                for mo in range(MO):
                    nc.tensor.matmul(
                        ps[:],
                        wqk[:, ct * 2 + j, mo, :],
                        xT[:, mo, tsl],
                        start=(mo == 0),
                        stop=(mo == MO - 1),
                    )
                nc.vector.tensor_scalar_add(
                    dst[:, ct, tsl], ps[:], b_sb[:, ct : ct + 1]
                )

            def emit_v_chain(tg, ti):
                tt = tg * 4 + ti
                ps = ps_b.tile([128, 512], f32, tag="b", name=f"v_{tg}_{ti}")
                for mo in range(MO):
                    nc.tensor.matmul(
                        ps[:, 0:CLOC],
                        xT[:, mo, tt * 128 : (tt + 1) * 128],
                        wv[:, mo, :],
                        start=(mo == 0),
                        stop=(mo == MO - 1),
                    )
                nc.vector.tensor_copy(
                    VA[:, tt, :, 0:64],
                    ps[:, 0:CLOC].rearrange("p (h d) -> p h d", d=64),
                )

            def emit_qkv(tg):
                for ct in range(2):
                    for j in range(2):
                        emit_qk_chain(tg, ct, j)
                for ti in range(4):
                    emit_v_chain(tg, ti)

            def push_qkv_fillers(tg):
                for ct in range(2):
                    for j in range(2):
                        fillers.append(
                            (("qkv", tg), 4096 * PE_NS,
                             lambda tg=tg, ct=ct, j=j: emit_qk_chain(tg, ct, j))
                        )
                vcls = "v" if defer_v else "qkv"
                for ti in range(4):
                    fillers.append(
                        ((vcls, tg), 2048 * PE_NS,
                         lambda tg=tg, ti=ti: emit_v_chain(tg, ti))
                    )

            # ---- attention ----
            def emit_sgrp(h, qg, gi, grp):
                """S^T matmuls for one exp group + the exp + diag masks."""
                hp = (h % 2) * 64
                ct = h // 2
                g0 = qg * 512
                cum = grp[-1][1] + grp[-1][2]
                sreg = ps_s.tile([128, sreg_w], f32, tag="s",
                                 name=f"s_{h}_{qg}_{gi}")
                for kt, off, w in grp:
                    q0 = g0 + 512 - w
                    c0 = off
                    while c0 < off + w:
                        cw = min(off + w - c0, 512 - c0 % 512)
                        nc.tensor.matmul(
                            sreg[:, c0 : c0 + cw],
                            KT[hp : hp + 64, ct, kt * 128 : (kt + 1) * 128],
                            QT[hp : hp + 64, ct,
                               q0 + c0 - off : q0 + c0 - off + cw],
                        )
                        c0 += cw
                pT = ppool.tile([128, sreg_w], bf16, tag="pT",
                                name=f"pT_{h}_{qg}_{gi}")
                no_diag = all(kt * 128 < g0 for kt, _, _ in grp)
                if gi < schr_groups[qg] and no_diag:
                    # Schraudolph exp on DVE: bf16 bits of exp(x*0.125) ~=
                    # int16(x*(0.125*128/ln2) + (127*128 - 5.6)); offloads
                    # the ACT engine (the attention-band pacer) at ~2% rms
                    # error on this group's P entries
                    nc.vector.tensor_scalar(
                        pT[:, :cum].bitcast(mybir.dt.int16), sreg[:, :cum],
                        scalar1=23.0831253, op0=mybir.AluOpType.mult,
                        scalar2=16250.4, op1=mybir.AluOpType.add)
                else:
                    nc.scalar.activation(pT[:, :cum], sreg[:, :cum], EXP,
                                         scale=0.125)
                teng = nc.gpsimd if tri_engine == "gpsimd" else nc.vector
                for kt, off, w in grp:
                    if kt * 128 >= g0:  # diagonal block leads the span
                        teng.tensor_mul(
                            pT[:, off : off + 128],
                            pT[:, off : off + 128],
                            tri[:],
                        )
                return pT

            def emit_zchain(h, qg, qt, pts, kt2g, zp):
                """z[q,65] = sum_kt pT_chunk^T @ V_aug: one sequential PSUM
                accumulation chain per q-tile (a PSUM bank supports only one
                open accumulation group at a time)."""
                g0 = qg * 512
                qa = 4 * qg + qt
                for kt in range(qa + 1):
                    gi, off, w = kt2g[kt]
                    q0 = g0 + 512 - w
                    c0 = off + (g0 + qt * 128) - q0
                    nc.tensor.matmul(
                        zp[:, qt, 0:65],
                        pts[gi][:, c0 : c0 + 128],
                        VA[:, kt, h, 0:65],
                        start=(kt == 0),
                        stop=(kt == qa),
                    )

            def emit_norm(h, qg, zp, zq):
                """1/rowsum fused into the PSUM->SBUF copy of z."""
                hp = (h % 2) * 64
                rec = spool.tile([128, 4, 1], f32, tag="rec",
                                 name=f"rec_{h}_{qg}")
                nc.vector.reciprocal(rec[:], zp[:, :, 64:65])
                with nc.allow_low_precision(reason="attn out to bf16"):
                    for qt in range(4):
                        nc.vector.tensor_scalar_mul(
                            zq[:, qt, hp : hp + 64],
                            zp[:, qt, 0:64],
                            rec[:, qt, :],
                        )

            def emit_transpose(qg, pair, zq):
                """zq [q,128d] -> zT [128d, q] via PE transpose of 4 tiles."""
                quad = ps_b.tile([128, 4, 128], bf16, tag="b",
                                 name=f"tq_{qg}_{pair}")
                for qt in range(4):
                    nc.tensor.transpose(quad[:, qt, :], zq[:, qt, :], ident)
                with nc.allow_low_precision(reason="zT copy"):
                    nc.vector.tensor_copy(
                        zT[:, pair, qg * 512 : (qg + 1) * 512],
                        quad[:].rearrange("p a b -> p (a b)"),
                    )

            def emit_op_half(qg, nh, ti, yA):
                tt = qg * 4 + ti
                ps = ps_b.tile([128, 512], f32, tag="b",
                               name=f"opA_{qg}_{nh}_{ti}")
                nc.tensor.matmul(
                    ps[:],
                    zT[:, 0, tt * 128 : (tt + 1) * 128],
                    wo[:, 0, nh * 512 : (nh + 1) * 512],
                )
                nc.vector.tensor_copy(yA[:, ti, :], ps[:])

            def emit_op_chunk(qg, nh, ti, ysb, dma_split, yA=None):
                tt = qg * 4 + ti
                ps = ps_b.tile([128, 512], f32, tag="b",
                               name=f"op_{qg}_{nh}_{ti}")
                cos = (1,) if yA is not None else (0, 1)
                for co in cos:
                    nc.tensor.matmul(
                        ps[:],
                        zT[:, co, tt * 128 : (tt + 1) * 128],
                        wo[:, co, nh * 512 : (nh + 1) * 512],
                        start=(co == cos[0]),
                        stop=(co == 1),
                    )
                with nc.allow_low_precision(reason="y partial to bf16"):
                    if yA is not None:
                        nc.vector.tensor_add(ysb[:, ti, :], ps[:],
                                             yA[:, ti, :])
                    elif qg == 3:
                        # tail: alternate engines so the copy stream (612ns
                        # each) does not pace the final 427ns-chunk unroll
                        (nc.scalar.copy if ti % 2 == 0
                         else nc.vector.tensor_copy)(ysb[:, ti, :], ps[:])
                    else:
                        nc.vector.tensor_copy(ysb[:, ti, :], ps[:])
                nper = 4 // dma_split
                if ti % nper == nper - 1:
                    t0 = tt - nper + 1
                    deng = nc.sync if (ti // nper + nh) % 2 == 0 else nc.scalar
                    deng.dma_start(
                        y_d[t0 * 128 : (tt + 1) * 128,
                            nh * 512 : (nh + 1) * 512].rearrange(
                            "(ti p) n -> p ti n", p=128
                        ),
                        ysb[:, ti - nper + 1 : ti + 1, :],
                    )

            yA_box = {}

            def push_opA_fillers(qg):
                # first half (co=0) of qg3's out-proj: available right after
                # pair 0's transpose, staged to SBUF f32; the co=1 half plus
                # an add happens in the tail
                for nh in range(2):
                    yA = ypool.tile([128, 4, 512], f32, tag="yA", bufs=2,
                                    name=f"yA_{qg}_{nh}")
                    yA_box[(qg, nh)] = yA
                    for ti in range(4):
                        fillers.append(
                            (("op", qg), 512 * PE_NS,
                             lambda qg=qg, nh=nh, ti=ti, yA=yA:
                                 emit_op_half(qg, nh, ti, yA))
                        )

            def push_op_fillers(qg, split=False):
                dma_split = dma_splits[qg]
                for nh in range(2):
                    ysb = ypool.tile([128, 4, 512], bf16, tag="y",
                                     name=f"ysb_{qg}_{nh}")
                    yA = yA_box.get((qg, nh)) if split else None
                    for ti in range(4):
                        fillers.append(
                            (("op", qg), (512 if split else 1024) * PE_NS,
                             lambda qg=qg, nh=nh, ti=ti, ysb=ysb,
                                    ds=dma_split, yA=yA:
                                 emit_op_chunk(qg, nh, ti, ysb, ds, yA))
                        )

            def push_z_phase(qg, h, pts, kt2g, zq_box):
                """Queue head h's z chains + normalize (+ transpose) at the
                FRONT of the filler queue; they drain during head h+1's S
                phase (one-head software pipeline)."""
                box = {}

                def chain(qt):
                    if qt == 0:
                        box["zp"] = ps_z.tile([128, 4, 65], f32, tag="z",
                                              name=f"zp_{h}_{qg}")
                        if h % 2 == 0:
                            zq_box[h // 2] = zqpool.tile(
                                [128, 4, 128], bf16, tag="zq",
                                name=f"zq_{qg}_{h // 2}")
                    emit_zchain(h, qg, qt, pts, kt2g, box["zp"])

                def norm():
                    emit_norm(h, qg, box["zp"], zq_box[h // 2])
                    if h % 2 == 1:
                        emit_transpose(qg, h // 2, zq_box[h // 2])
                        if h == 1 and qg == 3 and op3_split:
                            push_opA_fillers(qg)
                        if h == HLOC - 1:
                            # quarter finished: queue its out-proj (reads
                            # zT(qg), complete as of this point) and the
                            # next token group's QKV
                            push_op_fillers(qg, split=(qg == 3 and op3_split))
                            if qg + 2 <= 3:
                                push_qkv_fillers(qg + 2)

                # the previous head's z thunks must fully precede this
                # head's (ps_z rotation + zq pair ordering)
                drain_class("z")
                thunks = []
                for qt in range(4):
                    ncols = (4 * qg + qt + 1) * 65
                    thunks.append(
                        (("z", qg, h), ncols * PE_NS,
                         lambda qt=qt: chain(qt))
                    )
                thunks.append((("z", qg, h), 0.0, norm))
                fillers[0:0] = thunks
                z_pushed_at[0] = fill_count[0]

            # ---- program ----
            emit_qkv(0)
            push_qkv_fillers(1)

            zq_box = {}
            for qg in range(4):
                groups = _groups(qg, sreg_w)
                kt2g = {}
                for gi, grp in enumerate(groups):
                    for kt, off, w in grp:
                        kt2g[kt] = (gi, off, w)
                if qg > 0:
                    # barrier: this quarter's S/z read QT/KT/VA of tg=qg
                    drain(("qkv", qg))
                for h in range(HLOC):
                    if h == 1:
                        drain(("v", qg))
                    pts = []
                    # emit S groups in pairs (back-to-back on PE) so ACT's
                    # exp stream has at most one bubble per pair, not per
                    # group; the 2-buffer S rotation permits exactly 2 ahead
                    qsc, qpad = ((fill_scale, fill_pad) if fill_cfg is None
                                 else fill_cfg[qg])
                    step = 2 if pair_s else 1
                    for g0i in range(0, len(groups), step):
                        pair = groups[g0i : g0i + step]
                        budget = 0.0
                        for gi, grp in zip(range(g0i, g0i + step), pair):
                            pts.append(emit_sgrp(h, qg, gi, grp))
                            cum = grp[-1][1] + grp[-1][2]
                            budget += ((cum * ACT_NS + 185.0) * qsc
                                       + qpad - cum * PE_NS)
                        fill(max(0.0, budget))
                    push_z_phase(qg, h, pts, kt2g, zq_box)

            # drain the tail (queue can grow while draining)
            while fillers:
                _, _, thunk = _pop_next()
                thunk()

    nc.compile()
    return nc


def _pack_w(w):
    # [DM, C] -> [128, MO, C]: partition p holds rows {mo*128 + p}
    return np.ascontiguousarray(
        w.reshape(MO, 128, w.shape[1]).transpose(1, 0, 2)
    ).astype(ml_dtypes.bfloat16)


def make_in_maps(x, w_qkv, b_qkv, w_out):
    # multiplicative post-exp mask: 1 where k <= q (upper incl diag), else 0
    tri = np.tri(128, 128, 0, dtype=np.float32).T.astype(ml_dtypes.bfloat16)
    ident = np.eye(128, dtype=np.float32).astype(ml_dtypes.bfloat16)
    in_maps = []
    for core in range(8):
        b = core // 4
        hg = core % 4
        c0 = hg * CLOC
        csl = slice(c0, c0 + CLOC)

        # packed consts: [128, 264] bf16-typed raw columns
        cst = np.zeros((128, 264), np.uint16)
        cst[:, 0:128] = tri.view(np.uint16)
        cst[:, 128:256] = ident.view(np.uint16)
        bq = np.ascontiguousarray(
            b_qkv[csl].astype(np.float32).reshape(2, 128).T
        )
        bk = np.ascontiguousarray(
            b_qkv[DM + c0 : DM + c0 + CLOC].astype(np.float32).reshape(2, 128).T
        )
        cst[:, 256:260] = bq.view(np.uint16).reshape(128, 4)
        cst[:, 260:264] = bk.view(np.uint16).reshape(128, 4)

        wq_p = _pack_w(w_qkv[:, csl])
        wk_p = _pack_w(w_qkv[:, DM + c0 : DM + c0 + CLOC])
        # [128, ctj, MO, 128]: ctj = ct*2 + j (j=0 -> Q, j=1 -> K)
        wqk = np.stack(
            [wq_p[:, :, 0:128], wk_p[:, :, 0:128],
             wq_p[:, :, 128:256], wk_p[:, :, 128:256]],
            axis=1,
        )
        in_maps.append(
            {
                "xT": _pack_w(np.ascontiguousarray(x[b].T)),
                "wqk": np.ascontiguousarray(wqk),
                "wv": _pack_w(w_qkv[:, 2 * DM + c0 : 2 * DM + c0 + CLOC]),
                # wo: [CLOC, DM] -> [128, 2, DM]
                "wo": np.ascontiguousarray(
                    w_out[csl, :].reshape(2, 128, DM).transpose(1, 0, 2)
                ).astype(ml_dtypes.bfloat16),
                "cst": cst.view(ml_dtypes.bfloat16),
            }
        )
    return in_maps


def gather(results, b_qkv, w_out, b_out):
    # device skips the V bias; z_norm + b_v projects to a constant row:
    # y += b_v @ w_out, folded into the output bias here
    b_eff = (
        b_out.astype(np.float32)
        + b_qkv[2 * DM :].astype(np.float32) @ w_out.astype(np.float32)
    )
    out = np.empty((B, S, DM), np.float32)
    for b in range(B):
        acc = results[4 * b]["y"].astype(np.float32)
        for j in range(1, 4):
            acc = acc + results[4 * b + j]["y"]
        out[b] = acc + b_eff[None, :]
    return out


def kernel(x, w_qkv, b_qkv, w_out, b_out):
    x = np.asarray(x)
    w_qkv = np.asarray(w_qkv)
    b_qkv = np.asarray(b_qkv)
    w_out = np.asarray(w_out)
    b_out = np.asarray(b_out)

    if "nc" not in _CACHE:
        _CACHE["nc"] = build()
    nc = _CACHE["nc"]

    in_maps = make_in_maps(x, w_qkv, b_qkv, w_out)
    res = run_bass_kernel_spmd(nc, in_maps, core_ids=list(range(8)))
    return gather(res.results, b_qkv, w_out, b_out)



# revision 9
# speedup vs baseline: 1.0126x; 1.0126x over previous
"""Causal multi-head attention block (B=2, S=2048, D=1024, H=16) on 8 TRN2 cores.

Sharding: core i handles batch b = i//4 and head group hg = i%4 (4 heads =
256 model dims). Each core computes its heads' attention and a partial
output projection; the host sums the 4 partials per batch and adds b_out.

Per-core device pipeline (bf16 matmuls, fp32 PSUM accumulation):
  1. QKV. Q^T,K^T land as [head_cols, tokens] (lhsT = W, rhs = x^T);
     V lands as [tokens, head_cols] (lhsT = x^T tiles, rhs = W_v), stored
     augmented with a ones column so the z-matmul also produces softmax
     row sums.
  2. Attention per head, flash-style in the S^T = K.Q^T orientation over
     the causal lower triangle only: S^T[k_tile, q_span] -> exp on ScalarE
     (scale=1/8, no max subtraction; logits ~N(0,1)) -> P^T bf16 ->
     multiplicative 0/1 mask on diagonal blocks -> z[q_tile, 65]
     += P^T_chunk^T @ V_aug accumulated over k tiles in PSUM. The [q, d+1]
     z orientation makes each z matmul only 65 PE columns (vs a full
     q-span) and puts the softmax row sum in PSUM column 64 of the same
     partition as its query, so normalization is a per-partition
     tensor_scalar multiply fused into the PSUM->SBUF copy.
  3. z[q,d] tiles are transposed back to z^T[d,q] via PE transpose
     (identity matmul, 128 cols per 2-head tile) for the out-projection.
  4. Out-proj: y_partial[t, n] accumulated over the 256 local dims.

Program order is a fine-grained software pipeline: the attention loop is
a flat sequence over (q-quarter, head, k-group) with the z matmuls
lagging one group behind the S matmuls, and a filler queue (next token
group's QKV chains, previous quarters' out-proj chunks) drained between
S and z so the PE never waits on ScalarE exp. Host pre-packs all inputs
into SBUF layouts (bf16); the V bias is folded into the output bias on
the host (b_v @ w_out).
"""

import numpy as np
import ml_dtypes

import concourse.mybir as mybir
import concourse.tile as tile
from concourse import bacc
from concourse.bass_utils import run_bass_kernel_spmd

B = 2
S = 2048
DM = 1024
HD = 64
HLOC = 4                 # heads per core
CLOC = HLOC * HD         # local model dims (256)
MO = DM // 128           # 8 k-subtiles of the model dim
NKT = S // 128           # 16 key tiles

f32 = mybir.dt.float32
bf16 = mybir.dt.bfloat16
f8 = mybir.dt.float8e4
DR = mybir.MatmulPerfMode.DoubleRow
EXP = mybir.ActivationFunctionType.Exp

ACT_NS = 0.8333333333333334
PE_NS = 0.4166666666666667

_CACHE = {}


def _groups(qg, cap=1024):
    """Pack the causal k-tile spans of query quarter qg into exp groups of
    <= cap columns. Returns list of groups; each group is a list of
    (kt, offset_in_group, width)."""
    g0 = qg * 512
    last_kt = 4 * qg + 3
    groups, cur, cum = [], [], 0
    for kt in range(last_kt + 1):
        w = g0 + 512 - max(kt * 128, g0)
        if cum + w > cap:
            groups.append(cur)
            cur, cum = [], 0
        cur.append((kt, cum, w))
        cum += w
    groups.append(cur)
    return groups


def build(pt_bufs=20, zq_bufs=4, y_bufs=4, sreg_w=1024, fill_scale=1.0,
          fill_pad=0.0, dma_splits=(2, 2, 2, 2), tri_engine="dve",
          z_delay=1, defer_v=True, copy_split=False, rr_pop=False,
          pair_s=False, op3_split=False, schr_groups=(0, 0, 0, 0),
          fill_cfg=((1.025, 53.0), (0.995, 56.0), (1.014, 5.0), (0.974, -158.0))):
    nc = bacc.Bacc("TRN2", target_bir_lowering=False, debug=False)

    # x and the QKV weights are fp8 e4m3 hi/lo pairs (weights pre-scaled x32
    # on the host so e4m3's mantissa is used; the x32 scales cancel in the
    # exp scale and the x32 ones-column). Q = xh@wh + xh@wl + xl@wh via
    # DoubleRow matmuls: contraction 256/matmul at 0.5 cycles/col.
    xh_d = nc.dram_tensor("xTh", [128, MO, S], f8, kind="ExternalInput")
    xl_d = nc.dram_tensor("xTl", [128, MO, S], f8, kind="ExternalInput")
    # wqk grouped per QKV chain (ctj = ct*2+j) so each chain's weights are
    # one contiguous DMA
    wqkh_d = nc.dram_tensor("wqkh", [128, 4, MO, 128], f8, kind="ExternalInput")
    wqkl_d = nc.dram_tensor("wqkl", [128, 4, MO, 128], f8, kind="ExternalInput")
    wvh_d = nc.dram_tensor("wvh", [128, MO, CLOC], f8, kind="ExternalInput")
    wvl_d = nc.dram_tensor("wvl", [128, MO, CLOC], f8, kind="ExternalInput")
    wo_d = nc.dram_tensor("wo", [128, 2, DM], bf16, kind="ExternalInput")
    # consts packed as raw bf16 columns: tri[0:128], identity[128:256],
    # bq[256:260], bk[260:264] (f32 values bit-split across bf16 pairs)
    cst_d = nc.dram_tensor("cst", [128, 264], bf16, kind="ExternalInput")
    y_d = nc.dram_tensor("y", [S, DM], bf16, kind="ExternalOutput")

    with tile.TileContext(nc) as tc:
        with (
            tc.tile_pool(name="consts", bufs=1) as consts,
            tc.tile_pool(name="acts", bufs=1) as apool,
            tc.tile_pool(name="pt", bufs=pt_bufs) as ppool,
            tc.tile_pool(name="zq", bufs=zq_bufs) as zqpool,
            tc.tile_pool(name="norm", bufs=4) as spool,
            tc.tile_pool(name="ycopy", bufs=y_bufs) as ypool,
            # 8 PSUM banks: ps_s 2x[128,1024]=4 (QK logits), ps_z
            # 2x[128,4,65]=2 (z accumulators), ps_b 2x[128,512]=2
            # (QKV / V / out-proj chains and z transposes)
            tc.tile_pool(name="ps_s", bufs=2, space="PSUM") as ps_s,
            tc.tile_pool(name="ps_z", bufs=2, space="PSUM") as ps_z,
            tc.tile_pool(name="ps_b", bufs=2, space="PSUM") as ps_b,
        ):
            csb = consts.tile([128, 264], bf16)
            wqkh = consts.tile([128, 4, MO, 128], f8)
            wqkl = consts.tile([128, 4, MO, 128], f8)
            wvh = consts.tile([128, MO, CLOC], f8)
            wvl = consts.tile([128, MO, CLOC], f8)
            wo = consts.tile([128, 2, DM], bf16)
            xh = apool.tile([128, MO, S], f8)
            xl = apool.tile([128, MO, S], f8)

            # DMA order = consumption order (transfers serialize on the DMA
            # engines): first QKV chain's weights, then x mo-pairs at the
            # pace the first chain consumes them, then the rest in bulk.
            # Term order within a chain is xh@wh, xh@wl, xl@wh, so the hi
            # halves lead and xl/wl follow.
            nc.sync.dma_start(wqkh[:, 0, :, :], wqkh_d[:, 0, :, :])
            nc.scalar.dma_start(xh[:, 0:2, 0:512], xh_d[:, 0:2, 0:512])
            nc.sync.dma_start(csb[:], cst_d[:])
            nc.scalar.dma_start(xh[:, 2:4, 0:512], xh_d[:, 2:4, 0:512])
            nc.sync.dma_start(wqkl[:, 0, :, :], wqkl_d[:, 0, :, :])
            nc.scalar.dma_start(xh[:, 4:8, 0:512], xh_d[:, 4:8, 0:512])
            nc.sync.dma_start(wqkh[:, 1, :, :], wqkh_d[:, 1, :, :])
            nc.scalar.dma_start(xl[:, :, 0:512], xl_d[:, :, 0:512])
            nc.sync.dma_start(wqkl[:, 1, :, :], wqkl_d[:, 1, :, :])
            nc.sync.dma_start(wqkh[:, 2:4, :, :], wqkh_d[:, 2:4, :, :])
            nc.sync.dma_start(wqkl[:, 2:4, :, :], wqkl_d[:, 2:4, :, :])
            nc.sync.dma_start(wvh[:], wvh_d[:])
            nc.sync.dma_start(wvl[:], wvl_d[:])
            nc.scalar.dma_start(xh[:, :, 512:1024], xh_d[:, :, 512:1024])
            nc.sync.dma_start(xl[:, :, 512:1024], xl_d[:, :, 512:1024])
            nc.scalar.dma_start(xh[:, :, 1024:1536], xh_d[:, :, 1024:1536])
            nc.sync.dma_start(xl[:, :, 1024:1536], xl_d[:, :, 1024:1536])
            nc.scalar.dma_start(xh[:, :, 1536:2048], xh_d[:, :, 1536:2048])
            nc.sync.dma_start(xl[:, :, 1536:2048], xl_d[:, :, 1536:2048])
            nc.sync.dma_start(wo[:], wo_d[:])

            tri = csb[:, 0:128]
            ident = csb[:, 128:256]
            bq_sb = csb[:, 256:260].bitcast(f32)
            bk_sb = csb[:, 260:264].bitcast(f32)

            QT = apool.tile([128, 2, S], bf16)
            KT = apool.tile([128, 2, S], bf16)
            # V augmented: [t-part, kt, h, 0:64] = v dims (x32), col 64 = 32
            # so the rowsum scale matches the v columns and the x32 cancels
            # in the normalization
            VA = apool.tile([128, NKT, HLOC, 72], bf16)
            nc.vector.memset(VA[:, :, :, 64:65], 32.0)
            zT = apool.tile([128, 2, S], bf16)

            # ---- filler queue: PE work units drained while ScalarE exps ----
            fillers = []          # list of (key, pe_ns, thunk)
            fill_debt = [0.0]
            fill_count = [0]      # fill() invocations, for z-pop delay
            z_pushed_at = [0]
            rr_state = [0]

            def _pop_next():
                """z thunks once ScalarE has had time to produce their exp
                inputs (z_delay fill periods after push), then QKV (needed
                by the next quarter anyway), out-proj last (the only filler
                class available during the final ACT-bound quarter)."""
                z_ready = fill_count[0] - z_pushed_at[0] >= z_delay
                if rr_pop:
                    # rotate class preference so short op chunks interleave
                    # with z/v/qkv work and their copies hide
                    base = ("z", "v", "qkv", "op") if z_ready else \
                        ("v", "qkv", "op", "z")
                    r = rr_state[0] % 4
                    order = base[r:] + base[:r] if z_ready else base
                    if z_ready:
                        rr_state[0] += 1
                    for cls in order:
                        if cls == "z" and not z_ready:
                            continue
                        for i, (k, ns, thunk) in enumerate(fillers):
                            if k[0] == cls:
                                return fillers.pop(i)
                    return fillers.pop(0)
                order = ("z", "v", "qkv") if z_ready else ("v", "qkv")
                for cls in order:
                    for i, (k, ns, thunk) in enumerate(fillers):
                        if k[0] == cls:
                            return fillers.pop(i)
                for i, (k, ns, thunk) in enumerate(fillers):
                    if k[0] != "z":
                        return fillers.pop(i)
                return fillers.pop(0)

            def fill(budget_ns):
                fill_count[0] += 1
                budget = budget_ns + fill_debt[0]
                spent = 0.0
                while fillers and spent < budget:
                    _, ns, thunk = _pop_next()
                    thunk()
                    spent += ns
                fill_debt[0] = budget - spent if fillers else 0.0

            def drain(key):
                """Force-emit queued fillers matching key (dependency
                barrier: attention on quarter qg needs all of QKV(tg=qg))."""
                rest = []
                for k, ns, thunk in fillers:
                    if k == key:
                        thunk()
                    else:
                        rest.append((k, ns, thunk))
                fillers[:] = rest

            def drain_class(cls):
                rest = []
                for k, ns, thunk in fillers:
                    if k[0] == cls:
                        thunk()
                    else:
                        rest.append((k, ns, thunk))
                fillers[:] = rest

            qkv_terms = ((0, 0), (0, 1), (1, 0))  # (x lo?, w lo?) per term

            def emit_qk_chain(tg, ct, j):
                tsl = slice(tg * 512, (tg + 1) * 512)
                dst, b_sb = ((QT, bq_sb), (KT, bk_sb))[j]
                ctj = ct * 2 + j
                ps = ps_b.tile([128, 512], f32, tag="b",
                               name=f"qk_{tg}_{ct}_{j}")
                for sub in range(2):
                    t0 = tg * 512 + sub * 256
                    i = 0
                    for xlo, wlo in qkv_terms:
                        xs = (xh, xl)[xlo]
                        ws = (wqkh, wqkl)[wlo]
                        for c in range(MO // 2):
                            nc.tensor.matmul(
                                ps[:, sub * 256 : sub * 256 + 256],
                                ws[:, ctj, 2 * c : 2 * c + 2, :],
                                xs[:, 2 * c : 2 * c + 2, t0 : t0 + 256],
                                start=(i == 0),
                                stop=(i == 11),
                                perf_mode=DR,
                            )
                            i += 1
                nc.vector.tensor_scalar_add(
                    dst[:, ct, tsl], ps[:], b_sb[:, ct : ct + 1]
                )

            def emit_v_chain(tg, ti):
                tt = tg * 4 + ti
                ps = ps_b.tile([128, 512], f32, tag="b", name=f"v_{tg}_{ti}")
                i = 0
                for xlo, wlo in qkv_terms:
                    xs = (xh, xl)[xlo]
                    ws = (wvh, wvl)[wlo]
                    for c in range(MO // 2):
                        nc.tensor.matmul(
                            ps[:, 0:CLOC],
                            xs[:, 2 * c : 2 * c + 2, tt * 128 : (tt + 1) * 128],
                            ws[:, 2 * c : 2 * c + 2, :],
                            start=(i == 0),
                            stop=(i == 11),
                            perf_mode=DR,
                        )
                        i += 1
                nc.vector.tensor_copy(
                    VA[:, tt, :, 0:64],
                    ps[:, 0:CLOC].rearrange("p (h d) -> p h d", d=64),
                )

            def emit_qkv(tg):
                for ct in range(2):
                    for j in range(2):
                        emit_qk_chain(tg, ct, j)
                for ti in range(4):
                    emit_v_chain(tg, ti)

            def push_qkv_fillers(tg):
                for ct in range(2):
                    for j in range(2):
                        fillers.append(
                            (("qkv", tg), 3072 * PE_NS,
                             lambda tg=tg, ct=ct, j=j: emit_qk_chain(tg, ct, j))
                        )
                vcls = "v" if defer_v else "qkv"
                for ti in range(4):
                    fillers.append(
                        ((vcls, tg), 1536 * PE_NS,
                         lambda tg=tg, ti=ti: emit_v_chain(tg, ti))
                    )

            # ---- attention ----
            def emit_sgrp(h, qg, gi, grp):
                """S^T matmuls for one exp group + the exp + diag masks."""
                hp = (h % 2) * 64
                ct = h // 2
                g0 = qg * 512
                cum = grp[-1][1] + grp[-1][2]
                sreg = ps_s.tile([128, sreg_w], f32, tag="s",
                                 name=f"s_{h}_{qg}_{gi}")
                for kt, off, w in grp:
                    q0 = g0 + 512 - w
                    c0 = off
                    while c0 < off + w:
                        cw = min(off + w - c0, 512 - c0 % 512)
                        nc.tensor.matmul(
                            sreg[:, c0 : c0 + cw],
                            KT[hp : hp + 64, ct, kt * 128 : (kt + 1) * 128],
                            QT[hp : hp + 64, ct,
                               q0 + c0 - off : q0 + c0 - off + cw],
                        )
                        c0 += cw
                pT = ppool.tile([128, sreg_w], bf16, tag="pT",
                                name=f"pT_{h}_{qg}_{gi}")
                no_diag = all(kt * 128 < g0 for kt, _, _ in grp)
                if gi < schr_groups[qg] and no_diag:
                    # Schraudolph exp on DVE: bf16 bits of exp(x*0.125) ~=
                    # int16(x*(0.125*128/ln2) + (127*128 - 5.6)); offloads
                    # the ACT engine (the attention-band pacer) at ~2% rms
                    # error on this group's P entries. sreg is 1024x (Q,K
                    # each carry the x32 weight prescale).
                    nc.vector.tensor_scalar(
                        pT[:, :cum].bitcast(mybir.dt.int16), sreg[:, :cum],
                        scalar1=23.0831253 / 1024.0, op0=mybir.AluOpType.mult,
                        scalar2=16250.4, op1=mybir.AluOpType.add)
                else:
                    nc.scalar.activation(pT[:, :cum], sreg[:, :cum], EXP,
                                         scale=0.125 / 1024.0)
                teng = nc.gpsimd if tri_engine == "gpsimd" else nc.vector
                for kt, off, w in grp:
                    if kt * 128 >= g0:  # diagonal block leads the span
                        teng.tensor_mul(
                            pT[:, off : off + 128],
                            pT[:, off : off + 128],
                            tri[:],
                        )
                return pT

            def emit_zchain(h, qg, qt, pts, kt2g, zp):
                """z[q,65] = sum_kt pT_chunk^T @ V_aug: one sequential PSUM
                accumulation chain per q-tile (a PSUM bank supports only one
                open accumulation group at a time)."""
                g0 = qg * 512
                qa = 4 * qg + qt
                for kt in range(qa + 1):
                    gi, off, w = kt2g[kt]
                    q0 = g0 + 512 - w
                    c0 = off + (g0 + qt * 128) - q0
                    nc.tensor.matmul(
                        zp[:, qt, 0:65],
                        pts[gi][:, c0 : c0 + 128],
                        VA[:, kt, h, 0:65],
                        start=(kt == 0),
                        stop=(kt == qa),
                    )

            def emit_norm(h, qg, zp, zq):
                """1/rowsum fused into the PSUM->SBUF copy of z."""
                hp = (h % 2) * 64
                rec = spool.tile([128, 4, 1], f32, tag="rec",
                                 name=f"rec_{h}_{qg}")
                nc.vector.reciprocal(rec[:], zp[:, :, 64:65])
                with nc.allow_low_precision(reason="attn out to bf16"):
                    for qt in range(4):
                        nc.vector.tensor_scalar_mul(
                            zq[:, qt, hp : hp + 64],
                            zp[:, qt, 0:64],
                            rec[:, qt, :],
                        )

            def emit_transpose(qg, pair, zq):
                """zq [q,128d] -> zT [128d, q] via PE transpose of 4 tiles."""
                quad = ps_b.tile([128, 4, 128], bf16, tag="b",
                                 name=f"tq_{qg}_{pair}")
                for qt in range(4):
                    nc.tensor.transpose(quad[:, qt, :], zq[:, qt, :], ident)
                with nc.allow_low_precision(reason="zT copy"):
                    nc.vector.tensor_copy(
                        zT[:, pair, qg * 512 : (qg + 1) * 512],
                        quad[:].rearrange("p a b -> p (a b)"),
                    )

            def emit_op_half(qg, nh, ti, yA):
                tt = qg * 4 + ti
                ps = ps_b.tile([128, 512], f32, tag="b",
                               name=f"opA_{qg}_{nh}_{ti}")
                nc.tensor.matmul(
                    ps[:],
                    zT[:, 0, tt * 128 : (tt + 1) * 128],
                    wo[:, 0, nh * 512 : (nh + 1) * 512],
                )
                nc.vector.tensor_copy(yA[:, ti, :], ps[:])

            def emit_op_chunk(qg, nh, ti, ysb, dma_split, yA=None):
                tt = qg * 4 + ti
                ps = ps_b.tile([128, 512], f32, tag="b",
                               name=f"op_{qg}_{nh}_{ti}")
                cos = (1,) if yA is not None else (0, 1)
                for co in cos:
                    nc.tensor.matmul(
                        ps[:],
                        zT[:, co, tt * 128 : (tt + 1) * 128],
                        wo[:, co, nh * 512 : (nh + 1) * 512],
                        start=(co == cos[0]),
                        stop=(co == 1),
                    )
                with nc.allow_low_precision(reason="y partial to bf16"):
                    if yA is not None:
                        nc.vector.tensor_add(ysb[:, ti, :], ps[:],
                                             yA[:, ti, :])
                    elif qg == 3:
                        # tail: alternate engines so the copy stream (612ns
                        # each) does not pace the final 427ns-chunk unroll
                        (nc.scalar.copy if ti % 2 == 0
                         else nc.vector.tensor_copy)(ysb[:, ti, :], ps[:])
                    else:
                        nc.vector.tensor_copy(ysb[:, ti, :], ps[:])
                nper = 4 // dma_split
                if ti % nper == nper - 1:
                    t0 = tt - nper + 1
                    deng = nc.sync if (ti // nper + nh) % 2 == 0 else nc.scalar
                    deng.dma_start(
                        y_d[t0 * 128 : (tt + 1) * 128,
                            nh * 512 : (nh + 1) * 512].rearrange(
                            "(ti p) n -> p ti n", p=128
                        ),
                        ysb[:, ti - nper + 1 : ti + 1, :],
                    )

            yA_box = {}

            def push_opA_fillers(qg):
                # first half (co=0) of qg3's out-proj: available right after
                # pair 0's transpose, staged to SBUF f32; the co=1 half plus
                # an add happens in the tail
                for nh in range(2):
                    yA = ypool.tile([128, 4, 512], f32, tag="yA", bufs=2,
                                    name=f"yA_{qg}_{nh}")
                    yA_box[(qg, nh)] = yA
                    for ti in range(4):
                        fillers.append(
                            (("op", qg), 512 * PE_NS,
                             lambda qg=qg, nh=nh, ti=ti, yA=yA:
                                 emit_op_half(qg, nh, ti, yA))
                        )

            def push_op_fillers(qg, split=False):
                dma_split = dma_splits[qg]
                for nh in range(2):
                    ysb = ypool.tile([128, 4, 512], bf16, tag="y",
                                     name=f"ysb_{qg}_{nh}")
                    yA = yA_box.get((qg, nh)) if split else None
                    for ti in range(4):
                        fillers.append(
                            (("op", qg), (512 if split else 1024) * PE_NS,
                             lambda qg=qg, nh=nh, ti=ti, ysb=ysb,
                                    ds=dma_split, yA=yA:
                                 emit_op_chunk(qg, nh, ti, ysb, ds, yA))
                        )

            def push_z_phase(qg, h, pts, kt2g, zq_box):
                """Queue head h's z chains + normalize (+ transpose) at the
                FRONT of the filler queue; they drain during head h+1's S
                phase (one-head software pipeline)."""
                box = {}

                def chain(qt):
                    if qt == 0:
                        box["zp"] = ps_z.tile([128, 4, 65], f32, tag="z",
                                              name=f"zp_{h}_{qg}")
                        if h % 2 == 0:
                            zq_box[h // 2] = zqpool.tile(
                                [128, 4, 128], bf16, tag="zq",
                                name=f"zq_{qg}_{h // 2}")
                    emit_zchain(h, qg, qt, pts, kt2g, box["zp"])

                def norm():
                    emit_norm(h, qg, box["zp"], zq_box[h // 2])
                    if h % 2 == 1:
                        emit_transpose(qg, h // 2, zq_box[h // 2])
                        if h == 1 and qg == 3 and op3_split:
                            push_opA_fillers(qg)
                        if h == HLOC - 1:
                            # quarter finished: queue its out-proj (reads
                            # zT(qg), complete as of this point) and the
                            # next token group's QKV
                            push_op_fillers(qg, split=(qg == 3 and op3_split))
                            if qg + 2 <= 3:
                                push_qkv_fillers(qg + 2)

                # the previous head's z thunks must fully precede this
                # head's (ps_z rotation + zq pair ordering)
                drain_class("z")
                thunks = []
                for qt in range(4):
                    ncols = (4 * qg + qt + 1) * 65
                    thunks.append(
                        (("z", qg, h), ncols * PE_NS,
                         lambda qt=qt: chain(qt))
                    )
                thunks.append((("z", qg, h), 0.0, norm))
                fillers[0:0] = thunks
                z_pushed_at[0] = fill_count[0]

            # ---- program ----
            emit_qkv(0)
            push_qkv_fillers(1)

            zq_box = {}
            for qg in range(4):
                groups = _groups(qg, sreg_w)
                kt2g = {}
                for gi, grp in enumerate(groups):
                    for kt, off, w in grp:
                        kt2g[kt] = (gi, off, w)
                if qg > 0:
                    # barrier: this quarter's S/z read QT/KT/VA of tg=qg
                    drain(("qkv", qg))
                for h in range(HLOC):
                    if h == 1:
                        drain(("v", qg))
                    pts = []
                    # emit S groups in pairs (back-to-back on PE) so ACT's
                    # exp stream has at most one bubble per pair, not per
                    # group; the 2-buffer S rotation permits exactly 2 ahead
                    qsc, qpad = ((fill_scale, fill_pad) if fill_cfg is None
                                 else fill_cfg[qg])
                    step = 2 if pair_s else 1
                    for g0i in range(0, len(groups), step):
                        pair = groups[g0i : g0i + step]
                        budget = 0.0
                        for gi, grp in zip(range(g0i, g0i + step), pair):
                            pts.append(emit_sgrp(h, qg, gi, grp))
                            cum = grp[-1][1] + grp[-1][2]
                            budget += ((cum * ACT_NS + 185.0) * qsc
                                       + qpad - cum * PE_NS)
                        fill(max(0.0, budget))
                    push_z_phase(qg, h, pts, kt2g, zq_box)

            # drain the tail (queue can grow while draining)
            while fillers:
                _, _, thunk = _pop_next()
                thunk()

    nc.compile()
    return nc


def _pack_w(w):
    # [DM, C] -> [128, MO, C] f32: partition p holds rows {mo*128 + p}
    return np.ascontiguousarray(
        w.reshape(MO, 128, w.shape[1]).transpose(1, 0, 2)
    ).astype(np.float32)


def _split8(a):
    # f32 array -> (hi, lo) e4m3 pair with hi + lo ~= a to ~0.1%
    ah = a.astype(ml_dtypes.float8_e4m3)
    al = (a - ah.astype(np.float32)).astype(ml_dtypes.float8_e4m3)
    return np.ascontiguousarray(ah), np.ascontiguousarray(al)


def make_in_maps(x, w_qkv, b_qkv, w_out):
    # multiplicative post-exp mask: 1 where k <= q (upper incl diag), else 0
    tri = np.tri(128, 128, 0, dtype=np.float32).T.astype(ml_dtypes.bfloat16)
    ident = np.eye(128, dtype=np.float32).astype(ml_dtypes.bfloat16)
    in_maps = []
    for core in range(8):
        b = core // 4
        hg = core % 4
        c0 = hg * CLOC
        csl = slice(c0, c0 + CLOC)

        # packed consts: [128, 264] bf16-typed raw columns. Biases carry the
        # x32 weight prescale (Q,K live at 32x on device).
        cst = np.zeros((128, 264), np.uint16)
        cst[:, 0:128] = tri.view(np.uint16)
        cst[:, 128:256] = ident.view(np.uint16)
        bq = np.ascontiguousarray(
            32.0 * b_qkv[csl].astype(np.float32).reshape(2, 128).T
        )
        bk = np.ascontiguousarray(
            32.0 * b_qkv[DM + c0 : DM + c0 + CLOC].astype(np.float32)
            .reshape(2, 128).T
        )
        cst[:, 256:260] = bq.view(np.uint16).reshape(128, 4)
        cst[:, 260:264] = bk.view(np.uint16).reshape(128, 4)

        wq_p = _pack_w(32.0 * w_qkv[:, csl])
        wk_p = _pack_w(32.0 * w_qkv[:, DM + c0 : DM + c0 + CLOC])
        # [128, ctj, MO, 128]: ctj = ct*2 + j (j=0 -> Q, j=1 -> K)
        wqk = np.concatenate(
            [wq_p[:, None, :, 0:128], wk_p[:, None, :, 0:128],
             wq_p[:, None, :, 128:256], wk_p[:, None, :, 128:256]],
            axis=1,
        )
        wqkh, wqkl = _split8(wqk)
        wvh, wvl = _split8(
            _pack_w(32.0 * w_qkv[:, 2 * DM + c0 : 2 * DM + c0 + CLOC]))
        xTh, xTl = _split8(_pack_w(np.ascontiguousarray(x[b].T)))
        in_maps.append(
            {
                "xTh": xTh,
                "xTl": xTl,
                "wqkh": wqkh,
                "wqkl": wqkl,
                "wvh": wvh,
                "wvl": wvl,
                # wo: [CLOC, DM] -> [128, 2, DM]
                "wo": np.ascontiguousarray(
                    w_out[csl, :].reshape(2, 128, DM).transpose(1, 0, 2)
                ).astype(ml_dtypes.bfloat16),
                "cst": cst.view(ml_dtypes.bfloat16),
            }
        )
    return in_maps


def gather(results, b_qkv, w_out, b_out):
    # device skips the V bias; z_norm + b_v projects to a constant row:
    # y += b_v @ w_out, folded into the output bias here
    b_eff = (
        b_out.astype(np.float32)
        + b_qkv[2 * DM :].astype(np.float32) @ w_out.astype(np.float32)
    )
    out = np.empty((B, S, DM), np.float32)
    for b in range(B):
        acc = results[4 * b]["y"].astype(np.float32)
        for j in range(1, 4):
            acc = acc + results[4 * b + j]["y"]
        out[b] = acc + b_eff[None, :]
    return out


def kernel(x, w_qkv, b_qkv, w_out, b_out):
    x = np.asarray(x)
    w_qkv = np.asarray(w_qkv)
    b_qkv = np.asarray(b_qkv)
    w_out = np.asarray(w_out)
    b_out = np.asarray(b_out)

    if "nc" not in _CACHE:
        _CACHE["nc"] = build()
    nc = _CACHE["nc"]

    in_maps = make_in_maps(x, w_qkv, b_qkv, w_out)
    res = run_bass_kernel_spmd(nc, in_maps, core_ids=list(range(8)))
    return gather(res.results, b_qkv, w_out, b_out)



# revision 14
# speedup vs baseline: 1.0443x; 1.0313x over previous
"""Causal multi-head attention block (B=2, S=2048, D=1024, H=16) on 8 TRN2 cores.

Sharding: core i handles batch b = i//4 and head group hg = i%4 (4 heads =
256 model dims). Each core computes its heads' attention and a partial
output projection; the host sums the 4 partials per batch and adds b_out.

Per-core device pipeline (bf16 matmuls, fp32 PSUM accumulation):
  1. QKV. Q^T,K^T land as [head_cols, tokens] (lhsT = W, rhs = x^T);
     V lands as [tokens, head_cols] (lhsT = x^T tiles, rhs = W_v), stored
     augmented with a ones column so the z-matmul also produces softmax
     row sums.
  2. Attention per head, flash-style in the S^T = K.Q^T orientation over
     the causal lower triangle only: S^T[k_tile, q_span] -> exp on ScalarE
     (scale=1/8, no max subtraction; logits ~N(0,1)) -> P^T bf16 ->
     multiplicative 0/1 mask on diagonal blocks -> z[q_tile, 65]
     += P^T_chunk^T @ V_aug accumulated over k tiles in PSUM. The [q, d+1]
     z orientation makes each z matmul only 65 PE columns (vs a full
     q-span) and puts the softmax row sum in PSUM column 64 of the same
     partition as its query, so normalization is a per-partition
     tensor_scalar multiply fused into the PSUM->SBUF copy.
  3. z[q,d] tiles are transposed back to z^T[d,q] via PE transpose
     (identity matmul, 128 cols per 2-head tile) for the out-projection.
  4. Out-proj: y_partial[t, n] accumulated over the 256 local dims.

Program order is a fine-grained software pipeline: the attention loop is
a flat sequence over (q-quarter, head, k-group) with the z matmuls
lagging one group behind the S matmuls, and a filler queue (next token
group's QKV chains, previous quarters' out-proj chunks) drained between
S and z so the PE never waits on ScalarE exp. Host pre-packs all inputs
into SBUF layouts (bf16); the V bias is folded into the output bias on
the host (b_v @ w_out).
"""

import numpy as np
import ml_dtypes

import concourse.mybir as mybir
import concourse.tile as tile
from concourse import bacc
from concourse.bass_utils import run_bass_kernel_spmd

B = 2
S = 2048
DM = 1024
HD = 64
HLOC = 4                 # heads per core
CLOC = HLOC * HD         # local model dims (256)
MO = DM // 128           # 8 k-subtiles of the model dim
NKT = S // 128           # 16 key tiles

f32 = mybir.dt.float32
bf16 = mybir.dt.bfloat16
f8 = mybir.dt.float8e4
DR = mybir.MatmulPerfMode.DoubleRow
EXP = mybir.ActivationFunctionType.Exp

ACT_NS = 0.8333333333333334
PE_NS = 0.4166666666666667

_CACHE = {}


def _groups(qg, cap=1024):
    """Pack the causal k-tile spans of query quarter qg into exp groups of
    <= cap columns. Returns list of groups; each group is a list of
    (kt, offset_in_group, width)."""
    g0 = qg * 512
    last_kt = 4 * qg + 3
    groups, cur, cum = [], [], 0
    for kt in range(last_kt + 1):
        w = g0 + 512 - max(kt * 128, g0)
        if cum + w > cap:
            groups.append(cur)
            cur, cum = [], 0
        cur.append((kt, cum, w))
        cum += w
    groups.append(cur)
    return groups


def build(pt_bufs=20, zq_bufs=4, y_bufs=4, sreg_w=1024, fill_scale=1.0,
          fill_pad=0.0, dma_splits=(2, 2, 2, 4), tri_engine="dve",
          z_delay=1, defer_v=True, copy_split=False, rr_pop=False,
          pair_s=False, op3_split=False, schr_groups=(0, 0, 0, 0),
          fill_cfg=((1.025, 53.0), (0.995, 56.0), (1.014, 5.0), (0.974, -158.0))):
    nc = bacc.Bacc("TRN2", target_bir_lowering=False, debug=False)

    # x and the QKV weights are fp8 e4m3 hi/lo pairs (weights pre-scaled x32
    # on the host so e4m3's mantissa is used; the x32 scales cancel in the
    # exp scale and the x32 ones-column). Q = xh@wh + xh@wl + xl@wh via
    # DoubleRow matmuls: contraction 256/matmul at 0.5 cycles/col.
    xh_d = nc.dram_tensor("xTh", [128, MO, S], f8, kind="ExternalInput")
    xl_d = nc.dram_tensor("xTl", [128, MO, S], f8, kind="ExternalInput")
    # wqk grouped per QKV chain (ctj = ct*2+j) so each chain's weights are
    # one contiguous DMA
    wqkh_d = nc.dram_tensor("wqkh", [128, 4, MO, 128], f8, kind="ExternalInput")
    wqkl_d = nc.dram_tensor("wqkl", [128, 4, MO, 128], f8, kind="ExternalInput")
    wvh_d = nc.dram_tensor("wvh", [128, MO, CLOC], f8, kind="ExternalInput")
    wvl_d = nc.dram_tensor("wvl", [128, MO, CLOC], f8, kind="ExternalInput")
    wo_d = nc.dram_tensor("wo", [128, 2, DM], bf16, kind="ExternalInput")
    # consts packed as raw bf16 columns: tri[0:128], identity[128:256],
    # bq[256:260], bk[260:264] (f32 values bit-split across bf16 pairs)
    cst_d = nc.dram_tensor("cst", [128, 264], bf16, kind="ExternalInput")
    y_d = nc.dram_tensor("y", [S, DM], bf16, kind="ExternalOutput")

    with tile.TileContext(nc) as tc:
        with (
            tc.tile_pool(name="consts", bufs=1) as consts,
            tc.tile_pool(name="acts", bufs=1) as apool,
            tc.tile_pool(name="pt", bufs=pt_bufs) as ppool,
            tc.tile_pool(name="zq", bufs=zq_bufs) as zqpool,
            tc.tile_pool(name="norm", bufs=4) as spool,
            tc.tile_pool(name="ycopy", bufs=y_bufs) as ypool,
            # 8 PSUM banks: ps_s 2x[128,1024]=4 (QK logits), ps_z
            # 2x[128,4,65]=2 (z accumulators), ps_b 2x[128,512]=2
            # (QKV / V / out-proj chains and z transposes)
            tc.tile_pool(name="ps_s", bufs=2, space="PSUM") as ps_s,
            tc.tile_pool(name="ps_z", bufs=2, space="PSUM") as ps_z,
            tc.tile_pool(name="ps_b", bufs=2, space="PSUM") as ps_b,
        ):
            csb = consts.tile([128, 264], bf16)
            wqkh = consts.tile([128, 4, MO, 128], f8)
            wqkl = consts.tile([128, 4, MO, 128], f8)
            wvh = consts.tile([128, MO, CLOC], f8)
            wvl = consts.tile([128, MO, CLOC], f8)
            wo = consts.tile([128, 2, DM], bf16)
            xh = apool.tile([128, MO, S], f8)
            xl = apool.tile([128, MO, S], f8)

            # DMA order = consumption order (transfers serialize on the DMA
            # engines). Startup fans out over three queues (SP: weights,
            # ACT: xh, Pool/SWDGE: xl) so the first QK chain can start ~2.6us in;
            # bulk transfers stay off the ACT queue once the exp stream
            # starts (each dma_start occupies the sequencer ~660ns).
            nc.sync.dma_start(wqkh[:, 0, :, :], wqkh_d[:, 0, :, :])
            nc.scalar.dma_start(xh[:, 0:2, 0:512], xh_d[:, 0:2, 0:512])
            nc.gpsimd.dma_start(xl[:, 0:2, 0:512], xl_d[:, 0:2, 0:512])
            nc.sync.dma_start(wqkl[:, 0, :, :], wqkl_d[:, 0, :, :])
            nc.scalar.dma_start(xh[:, 2:4, 0:512], xh_d[:, 2:4, 0:512])
            nc.gpsimd.dma_start(xl[:, 2:4, 0:512], xl_d[:, 2:4, 0:512])
            nc.sync.dma_start(csb[:], cst_d[:])
            nc.scalar.dma_start(xh[:, 4:8, 0:512], xh_d[:, 4:8, 0:512])
            nc.gpsimd.dma_start(xl[:, 4:8, 0:512], xl_d[:, 4:8, 0:512])
            nc.sync.dma_start(wqkh[:, 1, :, :], wqkh_d[:, 1, :, :])
            nc.sync.dma_start(wqkl[:, 1, :, :], wqkl_d[:, 1, :, :])
            nc.sync.dma_start(wqkh[:, 2:4, :, :], wqkh_d[:, 2:4, :, :])
            nc.sync.dma_start(wqkl[:, 2:4, :, :], wqkl_d[:, 2:4, :, :])
            nc.sync.dma_start(wvh[:], wvh_d[:])
            nc.sync.dma_start(wvl[:], wvl_d[:])
            nc.scalar.dma_start(xh[:, :, 512:1024], xh_d[:, :, 512:1024])
            nc.sync.dma_start(xl[:, :, 512:1024], xl_d[:, :, 512:1024])
            nc.sync.dma_start(xh[:, :, 1024:1536], xh_d[:, :, 1024:1536])
            nc.sync.dma_start(xl[:, :, 1024:1536], xl_d[:, :, 1024:1536])
            nc.sync.dma_start(xh[:, :, 1536:2048], xh_d[:, :, 1536:2048])
            nc.sync.dma_start(xl[:, :, 1536:2048], xl_d[:, :, 1536:2048])
            nc.sync.dma_start(wo[:], wo_d[:])

            tri = csb[:, 0:128]
            ident = csb[:, 128:256]
            bq_sb = csb[:, 256:260].bitcast(f32)
            bk_sb = csb[:, 260:264].bitcast(f32)

            QT = apool.tile([128, 2, S], bf16)
            KT = apool.tile([128, 2, S], bf16)
            # V augmented: [t-part, kt, h, 0:64] = v dims (x32), col 64 = 32
            # so the rowsum scale matches the v columns and the x32 cancels
            # in the normalization
            VA = apool.tile([128, NKT, HLOC, 72], bf16)
            nc.vector.memset(VA[:, :, :, 64:65], 32.0)
            zT = apool.tile([128, 2, S], bf16)

            # ---- filler queue: PE work units drained while ScalarE exps ----
            fillers = []          # list of (key, pe_ns, thunk)
            fill_debt = [0.0]
            fill_count = [0]      # fill() invocations, for z-pop delay
            z_pushed_at = [0]
            rr_state = [0]

            def _pop_next():
                """z thunks once ScalarE has had time to produce their exp
                inputs (z_delay fill periods after push), then QKV (needed
                by the next quarter anyway), out-proj last (the only filler
                class available during the final ACT-bound quarter)."""
                z_ready = fill_count[0] - z_pushed_at[0] >= z_delay
                if rr_pop:
                    # rotate class preference so short op chunks interleave
                    # with z/v/qkv work and their copies hide
                    base = ("z", "v", "qkv", "op") if z_ready else \
                        ("v", "qkv", "op", "z")
                    r = rr_state[0] % 4
                    order = base[r:] + base[:r] if z_ready else base
                    if z_ready:
                        rr_state[0] += 1
                    for cls in order:
                        if cls == "z" and not z_ready:
                            continue
                        for i, (k, ns, thunk) in enumerate(fillers):
                            if k[0] == cls:
                                return fillers.pop(i)
                    return fillers.pop(0)
                order = ("z", "v", "qkv") if z_ready else ("v", "qkv")
                for cls in order:
                    for i, (k, ns, thunk) in enumerate(fillers):
                        if k[0] == cls:
                            return fillers.pop(i)
                for i, (k, ns, thunk) in enumerate(fillers):
                    if k[0] != "z":
                        return fillers.pop(i)
                return fillers.pop(0)

            def fill(budget_ns):
                fill_count[0] += 1
                budget = budget_ns + fill_debt[0]
                spent = 0.0
                while fillers and spent < budget:
                    _, ns, thunk = _pop_next()
                    thunk()
                    spent += ns
                fill_debt[0] = budget - spent if fillers else 0.0

            def drain(key):
                """Force-emit queued fillers matching key (dependency
                barrier: attention on quarter qg needs all of QKV(tg=qg))."""
                rest = []
                for k, ns, thunk in fillers:
                    if k == key:
                        thunk()
                    else:
                        rest.append((k, ns, thunk))
                fillers[:] = rest

            def drain_class(cls):
                rest = []
                for k, ns, thunk in fillers:
                    if k[0] == cls:
                        thunk()
                    else:
                        rest.append((k, ns, thunk))
                fillers[:] = rest

            qkv_terms = ((0, 0), (0, 1), (1, 0))  # (x lo?, w lo?) per term

            def emit_qk_chain(tg, ct, j):
                tsl = slice(tg * 512, (tg + 1) * 512)
                dst, b_sb = ((QT, bq_sb), (KT, bk_sb))[j]
                ctj = ct * 2 + j
                ps = ps_b.tile([128, 512], f32, tag="b",
                               name=f"qk_{tg}_{ct}_{j}")
                for sub in range(2):
                    t0 = tg * 512 + sub * 256
                    i = 0
                    for xlo, wlo in qkv_terms:
                        xs = (xh, xl)[xlo]
                        ws = (wqkh, wqkl)[wlo]
                        for c in range(MO // 2):
                            nc.tensor.matmul(
                                ps[:, sub * 256 : sub * 256 + 256],
                                ws[:, ctj, 2 * c : 2 * c + 2, :],
                                xs[:, 2 * c : 2 * c + 2, t0 : t0 + 256],
                                start=(i == 0),
                                stop=(i == 11),
                                perf_mode=DR,
                            )
                            i += 1
                nc.vector.tensor_scalar_add(
                    dst[:, ct, tsl], ps[:], b_sb[:, ct : ct + 1]
                )

            def emit_v_chain(tg, ti):
                tt = tg * 4 + ti
                ps = ps_b.tile([128, 512], f32, tag="b", name=f"v_{tg}_{ti}")
                i = 0
                for xlo, wlo in qkv_terms:
                    xs = (xh, xl)[xlo]
                    ws = (wvh, wvl)[wlo]
                    for c in range(MO // 2):
                        nc.tensor.matmul(
                            ps[:, 0:CLOC],
                            xs[:, 2 * c : 2 * c + 2, tt * 128 : (tt + 1) * 128],
                            ws[:, 2 * c : 2 * c + 2, :],
                            start=(i == 0),
                            stop=(i == 11),
                            perf_mode=DR,
                        )
                        i += 1
                nc.vector.tensor_copy(
                    VA[:, tt, :, 0:64],
                    ps[:, 0:CLOC].rearrange("p (h d) -> p h d", d=64),
                )

            def emit_qkv(tg):
                for ct in range(2):
                    for j in range(2):
                        emit_qk_chain(tg, ct, j)
                for ti in range(4):
                    emit_v_chain(tg, ti)

            def push_qkv_fillers(tg):
                for ct in range(2):
                    for j in range(2):
                        fillers.append(
                            (("qkv", tg), 3072 * PE_NS,
                             lambda tg=tg, ct=ct, j=j: emit_qk_chain(tg, ct, j))
                        )
                vcls = "v" if defer_v else "qkv"
                for ti in range(4):
                    fillers.append(
                        ((vcls, tg), 1536 * PE_NS,
                         lambda tg=tg, ti=ti: emit_v_chain(tg, ti))
                    )

            # ---- attention ----
            def emit_sgrp(h, qg, gi, grp):
                """S^T matmuls for one exp group + the exp + diag masks."""
                hp = (h % 2) * 64
                ct = h // 2
                g0 = qg * 512
                cum = grp[-1][1] + grp[-1][2]
                sreg = ps_s.tile([128, sreg_w], f32, tag="s",
                                 name=f"s_{h}_{qg}_{gi}")
                for kt, off, w in grp:
                    q0 = g0 + 512 - w
                    c0 = off
                    while c0 < off + w:
                        cw = min(off + w - c0, 512 - c0 % 512)
                        nc.tensor.matmul(
                            sreg[:, c0 : c0 + cw],
                            KT[hp : hp + 64, ct, kt * 128 : (kt + 1) * 128],
                            QT[hp : hp + 64, ct,
                               q0 + c0 - off : q0 + c0 - off + cw],
                        )
                        c0 += cw
                pT = ppool.tile([128, sreg_w], bf16, tag="pT",
                                name=f"pT_{h}_{qg}_{gi}")
                no_diag = all(kt * 128 < g0 for kt, _, _ in grp)
                if gi < schr_groups[qg] and no_diag:
                    # Schraudolph exp on DVE: bf16 bits of exp(x*0.125) ~=
                    # int16(x*(0.125*128/ln2) + (127*128 - 5.6)); offloads
                    # the ACT engine (the attention-band pacer) at ~2% rms
                    # error on this group's P entries. sreg is 1024x (Q,K
                    # each carry the x32 weight prescale).
                    nc.vector.tensor_scalar(
                        pT[:, :cum].bitcast(mybir.dt.int16), sreg[:, :cum],
                        scalar1=23.0831253 / 1024.0, op0=mybir.AluOpType.mult,
                        scalar2=16250.4, op1=mybir.AluOpType.add)
                else:
                    nc.scalar.activation(pT[:, :cum], sreg[:, :cum], EXP,
                                         scale=0.125 / 1024.0)
                teng = nc.gpsimd if tri_engine == "gpsimd" else nc.vector
                for kt, off, w in grp:
                    if kt * 128 >= g0:  # diagonal block leads the span
                        teng.tensor_mul(
                            pT[:, off : off + 128],
                            pT[:, off : off + 128],
                            tri[:],
                        )
                return pT

            def emit_zchain(h, qg, qt, pts, kt2g, zp):
                """z[q,65] = sum_kt pT_chunk^T @ V_aug: one sequential PSUM
                accumulation chain per q-tile (a PSUM bank supports only one
                open accumulation group at a time)."""
                g0 = qg * 512
                qa = 4 * qg + qt
                for kt in range(qa + 1):
                    gi, off, w = kt2g[kt]
                    q0 = g0 + 512 - w
                    c0 = off + (g0 + qt * 128) - q0
                    nc.tensor.matmul(
                        zp[:, qt, 0:65],
                        pts[gi][:, c0 : c0 + 128],
                        VA[:, kt, h, 0:65],
                        start=(kt == 0),
                        stop=(kt == qa),
                    )

            def emit_norm(h, qg, zp, zq):
                """1/rowsum fused into the PSUM->SBUF copy of z."""
                hp = (h % 2) * 64
                rec = spool.tile([128, 4, 1], f32, tag="rec",
                                 name=f"rec_{h}_{qg}")
                nc.vector.reciprocal(rec[:], zp[:, :, 64:65])
                with nc.allow_low_precision(reason="attn out to bf16"):
                    for qt in range(4):
                        nc.vector.tensor_scalar_mul(
                            zq[:, qt, hp : hp + 64],
                            zp[:, qt, 0:64],
                            rec[:, qt, :],
                        )

            def emit_transpose(qg, pair, zq):
                """zq [q,128d] -> zT [128d, q] via PE transpose of 4 tiles."""
                quad = ps_b.tile([128, 4, 128], bf16, tag="b",
                                 name=f"tq_{qg}_{pair}")
                for qt in range(4):
                    nc.tensor.transpose(quad[:, qt, :], zq[:, qt, :], ident)
                with nc.allow_low_precision(reason="zT copy"):
                    nc.vector.tensor_copy(
                        zT[:, pair, qg * 512 : (qg + 1) * 512],
                        quad[:].rearrange("p a b -> p (a b)"),
                    )

            def emit_op_half(qg, nh, ti, yA):
                tt = qg * 4 + ti
                ps = ps_b.tile([128, 512], f32, tag="b",
                               name=f"opA_{qg}_{nh}_{ti}")
                nc.tensor.matmul(
                    ps[:],
                    zT[:, 0, tt * 128 : (tt + 1) * 128],
                    wo[:, 0, nh * 512 : (nh + 1) * 512],
                )
                nc.vector.tensor_copy(yA[:, ti, :], ps[:])

            def emit_op_chunk(qg, nh, ti, ysb, dma_split, yA=None):
                tt = qg * 4 + ti
                ps = ps_b.tile([128, 512], f32, tag="b",
                               name=f"op_{qg}_{nh}_{ti}")
                cos = (1,) if yA is not None else (0, 1)
                for co in cos:
                    nc.tensor.matmul(
                        ps[:],
                        zT[:, co, tt * 128 : (tt + 1) * 128],
                        wo[:, co, nh * 512 : (nh + 1) * 512],
                        start=(co == cos[0]),
                        stop=(co == 1),
                    )
                with nc.allow_low_precision(reason="y partial to bf16"):
                    if yA is not None:
                        nc.vector.tensor_add(ysb[:, ti, :], ps[:],
                                             yA[:, ti, :])
                    elif qg == 3:
                        # tail: alternate engines so the copy stream (612ns
                        # each) does not pace the final 427ns-chunk unroll
                        (nc.scalar.copy if ti % 2 == 0
                         else nc.vector.tensor_copy)(ysb[:, ti, :], ps[:])
                    else:
                        nc.vector.tensor_copy(ysb[:, ti, :], ps[:])
                nper = 4 // dma_split
                if ti % nper == nper - 1:
                    t0 = tt - nper + 1
                    deng = nc.sync if (ti // nper + nh) % 2 == 0 else nc.scalar
                    deng.dma_start(
                        y_d[t0 * 128 : (tt + 1) * 128,
                            nh * 512 : (nh + 1) * 512].rearrange(
                            "(ti p) n -> p ti n", p=128
                        ),
                        ysb[:, ti - nper + 1 : ti + 1, :],
                    )

            yA_box = {}

            def push_opA_fillers(qg):
                # first half (co=0) of qg3's out-proj: available right after
                # pair 0's transpose, staged to SBUF f32; the co=1 half plus
                # an add happens in the tail
                for nh in range(2):
                    yA = ypool.tile([128, 4, 512], f32, tag="yA", bufs=2,
                                    name=f"yA_{qg}_{nh}")
                    yA_box[(qg, nh)] = yA
                    for ti in range(4):
                        fillers.append(
                            (("op", qg), 512 * PE_NS,
                             lambda qg=qg, nh=nh, ti=ti, yA=yA:
                                 emit_op_half(qg, nh, ti, yA))
                        )

            def push_op_fillers(qg, split=False):
                dma_split = dma_splits[qg]
                for nh in range(2):
                    ysb = ypool.tile([128, 4, 512], bf16, tag="y",
                                     name=f"ysb_{qg}_{nh}")
                    yA = yA_box.get((qg, nh)) if split else None
                    for ti in range(4):
                        fillers.append(
                            (("op", qg), (512 if split else 1024) * PE_NS,
                             lambda qg=qg, nh=nh, ti=ti, ysb=ysb,
                                    ds=dma_split, yA=yA:
                                 emit_op_chunk(qg, nh, ti, ysb, ds, yA))
                        )

            def push_z_phase(qg, h, pts, kt2g, zq_box):
                """Queue head h's z chains + normalize (+ transpose) at the
                FRONT of the filler queue; they drain during head h+1's S
                phase (one-head software pipeline)."""
                box = {}

                def chain(qt):
                    if qt == 0:
                        box["zp"] = ps_z.tile([128, 4, 65], f32, tag="z",
                                              name=f"zp_{h}_{qg}")
                        if h % 2 == 0:
                            zq_box[h // 2] = zqpool.tile(
                                [128, 4, 128], bf16, tag="zq",
                                name=f"zq_{qg}_{h // 2}")
                    emit_zchain(h, qg, qt, pts, kt2g, box["zp"])

                def norm():
                    emit_norm(h, qg, box["zp"], zq_box[h // 2])
                    if h % 2 == 1:
                        emit_transpose(qg, h // 2, zq_box[h // 2])
                        if h == 1 and qg == 3 and op3_split:
                            push_opA_fillers(qg)
                        if h == HLOC - 1:
                            # quarter finished: queue its out-proj (reads
                            # zT(qg), complete as of this point) and the
                            # next token group's QKV
                            push_op_fillers(qg, split=(qg == 3 and op3_split))
                            if qg + 2 <= 3:
                                push_qkv_fillers(qg + 2)

                # the previous head's z thunks must fully precede this
                # head's (ps_z rotation + zq pair ordering)
                drain_class("z")
                thunks = []
                for qt in range(4):
                    ncols = (4 * qg + qt + 1) * 65
                    thunks.append(
                        (("z", qg, h), ncols * PE_NS,
                         lambda qt=qt: chain(qt))
                    )
                thunks.append((("z", qg, h), 0.0, norm))
                fillers[0:0] = thunks
                z_pushed_at[0] = fill_count[0]

            # ---- program ----
            # ct0's Q,K chains (heads 0,1) emit directly so quarter 0's S/exp
            # stream starts as soon as possible; ct1 + V chains become
            # fillers drained during h0/h1's exp (barriers: v at h==1,
            # qkv(ct1) at h==2).
            for j in range(2):
                emit_qk_chain(0, 0, j)
            for jj in range(2):
                fillers.append(
                    (("qkv", 0), 3072 * PE_NS,
                     lambda jj=jj: emit_qk_chain(0, 1, jj))
                )
            for ti in range(4):
                fillers.append(
                    (("v", 0), 1536 * PE_NS,
                     lambda ti=ti: emit_v_chain(0, ti))
                )
            push_qkv_fillers(1)

            zq_box = {}
            for qg in range(4):
                groups = _groups(qg, sreg_w)
                kt2g = {}
                for gi, grp in enumerate(groups):
                    for kt, off, w in grp:
                        kt2g[kt] = (gi, off, w)
                if qg > 0:
                    # barrier: this quarter's S/z read QT/KT/VA of tg=qg
                    drain(("qkv", qg))
                for h in range(HLOC):
                    if h == 1:
                        drain(("v", qg))
                    if h == 2 and qg == 0:
                        drain(("qkv", 0))  # ct1 chains gate heads 2,3
                    pts = []
                    # emit S groups in pairs (back-to-back on PE) so ACT's
                    # exp stream has at most one bubble per pair, not per
                    # group; the 2-buffer S rotation permits exactly 2 ahead
                    qsc, qpad = ((fill_scale, fill_pad) if fill_cfg is None
                                 else fill_cfg[qg])
                    step = 2 if pair_s else 1
                    for g0i in range(0, len(groups), step):
                        pair = groups[g0i : g0i + step]
                        budget = 0.0
                        for gi, grp in zip(range(g0i, g0i + step), pair):
                            pts.append(emit_sgrp(h, qg, gi, grp))
                            cum = grp[-1][1] + grp[-1][2]
                            budget += ((cum * ACT_NS + 185.0) * qsc
                                       + qpad - cum * PE_NS)
                        fill(max(0.0, budget))
                    push_z_phase(qg, h, pts, kt2g, zq_box)

            # drain the tail (queue can grow while draining)
            while fillers:
                _, _, thunk = _pop_next()
                thunk()

    nc.compile()
    return nc


def _pack_w(w):
    # [DM, C] -> [128, MO, C] f32: partition p holds rows {mo*128 + p}
    return np.ascontiguousarray(
        w.reshape(MO, 128, w.shape[1]).transpose(1, 0, 2)
    ).astype(np.float32)


def _split8(a):
    # f32 array -> (hi, lo) e4m3 pair with hi + lo ~= a to ~0.1%
    ah = a.astype(ml_dtypes.float8_e4m3)
    al = (a - ah.astype(np.float32)).astype(ml_dtypes.float8_e4m3)
    return np.ascontiguousarray(ah), np.ascontiguousarray(al)


def make_in_maps(x, w_qkv, b_qkv, w_out):
    # multiplicative post-exp mask: 1 where k <= q (upper incl diag), else 0
    tri = np.tri(128, 128, 0, dtype=np.float32).T.astype(ml_dtypes.bfloat16)
    ident = np.eye(128, dtype=np.float32).astype(ml_dtypes.bfloat16)
    in_maps = []
    for core in range(8):
        b = core // 4
        hg = core % 4
        c0 = hg * CLOC
        csl = slice(c0, c0 + CLOC)

        # packed consts: [128, 264] bf16-typed raw columns. Biases carry the
        # x32 weight prescale (Q,K live at 32x on device).
        cst = np.zeros((128, 264), np.uint16)
        cst[:, 0:128] = tri.view(np.uint16)
        cst[:, 128:256] = ident.view(np.uint16)
        bq = np.ascontiguousarray(
            32.0 * b_qkv[csl].astype(np.float32).reshape(2, 128).T
        )
        bk = np.ascontiguousarray(
            32.0 * b_qkv[DM + c0 : DM + c0 + CLOC].astype(np.float32)
            .reshape(2, 128).T
        )
        cst[:, 256:260] = bq.view(np.uint16).reshape(128, 4)
        cst[:, 260:264] = bk.view(np.uint16).reshape(128, 4)

        wq_p = _pack_w(32.0 * w_qkv[:, csl])
        wk_p = _pack_w(32.0 * w_qkv[:, DM + c0 : DM + c0 + CLOC])
        # [128, ctj, MO, 128]: ctj = ct*2 + j (j=0 -> Q, j=1 -> K)
        wqk = np.concatenate(
            [wq_p[:, None, :, 0:128], wk_p[:, None, :, 0:128],
             wq_p[:, None, :, 128:256], wk_p[:, None, :, 128:256]],
            axis=1,
        )
        wqkh, wqkl = _split8(wqk)
        wvh, wvl = _split8(
            _pack_w(32.0 * w_qkv[:, 2 * DM + c0 : 2 * DM + c0 + CLOC]))
        xTh, xTl = _split8(_pack_w(np.ascontiguousarray(x[b].T)))
        in_maps.append(
            {
                "xTh": xTh,
                "xTl": xTl,
                "wqkh": wqkh,
                "wqkl": wqkl,
                "wvh": wvh,
                "wvl": wvl,
                # wo: [CLOC, DM] -> [128, 2, DM]
                "wo": np.ascontiguousarray(
                    w_out[csl, :].reshape(2, 128, DM).transpose(1, 0, 2)
                ).astype(ml_dtypes.bfloat16),
                "cst": cst.view(ml_dtypes.bfloat16),
            }
        )
    return in_maps


def gather(results, b_qkv, w_out, b_out):
    # device skips the V bias; z_norm + b_v projects to a constant row:
    # y += b_v @ w_out, folded into the output bias here
    b_eff = (
        b_out.astype(np.float32)
        + b_qkv[2 * DM :].astype(np.float32) @ w_out.astype(np.float32)
    )
    out = np.empty((B, S, DM), np.float32)
    for b in range(B):
        acc = results[4 * b]["y"].astype(np.float32)
        for j in range(1, 4):
            acc = acc + results[4 * b + j]["y"]
        out[b] = acc + b_eff[None, :]
    return out


def kernel(x, w_qkv, b_qkv, w_out, b_out):
    x = np.asarray(x)
    w_qkv = np.asarray(w_qkv)
    b_qkv = np.asarray(b_qkv)
    w_out = np.asarray(w_out)
    b_out = np.asarray(b_out)

    if "nc" not in _CACHE:
        _CACHE["nc"] = build()
    nc = _CACHE["nc"]

    in_maps = make_in_maps(x, w_qkv, b_qkv, w_out)
    res = run_bass_kernel_spmd(nc, in_maps, core_ids=list(range(8)))
    return gather(res.results, b_qkv, w_out, b_out)



# revision 44
# speedup vs baseline: 1.0861x; 1.0401x over previous
"""Causal multi-head attention block (B=2, S=2048, D=1024, H=16) on 8 TRN2 cores.

Sharding: core i handles batch b = i//4 and head group hg = i%4 (4 heads =
256 model dims). Each core computes its heads' attention and a partial
output projection; the host sums the 4 partials per batch and adds b_out.

Per-core device pipeline (fp32 PSUM accumulation):
  1. QKV in compensated fp8: x and the QKV weights are hi/lo e4m3 pairs
     (hosts pre-scales weights x32 so e4m3's mantissa range is used;
     hi+lo carries ~11 mantissa bits, more precise than bf16). Each
     chain is xh@wh + xh@wl + xl@wh via DoubleRow matmuls (contraction
     256/matmul at 0.5 cycles/col = 3x bf16 throughput). Q^T,K^T land
     as [head_cols, tokens] at 32x scale (absorbed by the exp scale
     2^-13); V lands as [tokens, head_cols] at 32x, stored augmented
     with a 32.0 column so the z-matmul's row sums carry the same scale
     and the normalization cancels it exactly.
  2. Attention per head in bf16 (fp8 here fails the 2e-2 tolerance),
     flash-style in the S^T = K.Q^T orientation over the causal lower
     triangle only: S^T[k_tile, q_span] -> exp on ScalarE
     (scale=2^-13, no max subtraction; logits ~N(0,1)) -> P^T bf16 ->
     multiplicative 0/1 mask on diagonal blocks -> z[q_tile, 65]
     += P^T_chunk^T @ V_aug accumulated over k tiles in PSUM. The [q, d+1]
     z orientation makes each z matmul only 65 PE columns (vs a full
     q-span) and puts the softmax row sum in PSUM column 64 of the same
     partition as its query, so normalization is a per-partition
     tensor_scalar multiply fused into the PSUM->SBUF copy.
  3. z[q,d] tiles are transposed back to z^T[d,q] via PE transpose
     (identity matmul, 128 cols per 2-head tile) for the out-projection.
  4. Out-proj: y_partial[t, n] accumulated over the 256 local dims.

Program order is a fine-grained software pipeline: the attention loop is
a flat sequence over (q-quarter, head, k-group) with the z matmuls
lagging one group behind the S matmuls, and a filler queue (next token
group's QKV chains, previous quarters' out-proj chunks) drained between
S and z so the PE never waits on ScalarE exp. The final (ACT-bound)
quarter's last head runs a per-q-tile norm -> transpose -> out-proj
pipeline so the tail doesn't serialize, and y DMAs ride the Pool
engine's SWDGE path (+SP/ACT in the tail) to keep descriptor
generation off the exp-critical sequencers. Host pre-packs all inputs
into SBUF layouts; the V bias is folded into the output bias on the
host (b_v @ w_out).
"""

import numpy as np
import ml_dtypes

import concourse.mybir as mybir
import concourse.tile as tile
from concourse import bacc
from concourse.bass_utils import run_bass_kernel_spmd

B = 2
S = 2048
DM = 1024
HD = 64
HLOC = 4                 # heads per core
CLOC = HLOC * HD         # local model dims (256)
MO = DM // 128           # 8 k-subtiles of the model dim
NKT = S // 128           # 16 key tiles

f32 = mybir.dt.float32
bf16 = mybir.dt.bfloat16
f8 = mybir.dt.float8e4
DR = mybir.MatmulPerfMode.DoubleRow
EXP = mybir.ActivationFunctionType.Exp

ACT_NS = 0.8333333333333334
PE_NS = 0.4166666666666667

_CACHE = {}


def _groups(qg, cap=1024):
    """Pack the causal k-tile spans of query quarter qg into exp groups of
    <= cap columns. Returns list of groups; each group is a list of
    (kt, offset_in_group, width)."""
    g0 = qg * 512
    last_kt = 4 * qg + 3
    groups, cur, cum = [], [], 0
    for kt in range(last_kt + 1):
        w = g0 + 512 - max(kt * 128, g0)
        if cum + w > cap:
            groups.append(cur)
            cur, cum = [], 0
        cur.append((kt, cum, w))
        cum += w
    groups.append(cur)
    return groups


def build(pt_bufs=20, zq_bufs=4, y_bufs=4, sreg_w=1024, fill_scale=1.0,
          fill_pad=0.0, dma_splits=(2, 2, 4, 4), tri_engine="dve", op_from=2,
          zp_bufs=2,
          z_delay=1, defer_v=True, copy_split=False, rr_pop=False,
          pair_s=(0, 0, 0, 1), op3_split=False, schr_groups=(0, 0, 0, 0),
          fill_cfg=((1.025, 53.0), (0.995, 56.0), (1.014, 5.0), (0.974, -100.0))):
    nc = bacc.Bacc("TRN2", target_bir_lowering=False, debug=False)

    # x and the QKV weights are fp8 e4m3 hi/lo pairs (weights pre-scaled x32
    # on the host so e4m3's mantissa is used; the x32 scales cancel in the
    # exp scale and the x32 ones-column). Q = xh@wh + xh@wl + xl@wh via
    # DoubleRow matmuls: contraction 256/matmul at 0.5 cycles/col.
    xh_d = nc.dram_tensor("xTh", [128, MO, S], f8, kind="ExternalInput")
    xl_d = nc.dram_tensor("xTl", [128, MO, S], f8, kind="ExternalInput")
    # wqk grouped per QKV chain (ctj = ct*2+j) so each chain's weights are
    # one contiguous DMA
    wqkh_d = nc.dram_tensor("wqkh", [128, 4, MO, 128], f8, kind="ExternalInput")
    wqkl_d = nc.dram_tensor("wqkl", [128, 4, MO, 128], f8, kind="ExternalInput")
    wvh_d = nc.dram_tensor("wvh", [128, MO, CLOC], f8, kind="ExternalInput")
    wvl_d = nc.dram_tensor("wvl", [128, MO, CLOC], f8, kind="ExternalInput")
    wo_d = nc.dram_tensor("wo", [128, 2, DM], bf16, kind="ExternalInput")
    # consts packed as raw bf16 columns: tri[0:128], identity[128:256],
    # bq[256:260], bk[260:264] (f32 values bit-split across bf16 pairs)
    cst_d = nc.dram_tensor("cst", [128, 264], bf16, kind="ExternalInput")
    y_d = nc.dram_tensor("y", [S, DM], bf16, kind="ExternalOutput")

    with tile.TileContext(nc) as tc:
        with (
            tc.tile_pool(name="consts", bufs=1) as consts,
            tc.tile_pool(name="acts", bufs=1) as apool,
            tc.tile_pool(name="pt", bufs=pt_bufs) as ppool,
            tc.tile_pool(name="zq", bufs=zq_bufs) as zqpool,
            tc.tile_pool(name="norm", bufs=4) as spool,
            tc.tile_pool(name="ycopy", bufs=y_bufs) as ypool,
            # 8 PSUM banks: ps_s 2x[128,1024]=4 (QK logits), ps_z
            # 2x[128,4,65]=2 (z accumulators), ps_b 2x[128,512]=2
            # (QKV / V / out-proj chains and z transposes)
            tc.tile_pool(name="ps_s", bufs=2, space="PSUM") as ps_s,
            tc.tile_pool(name="ps_z", bufs=zp_bufs, space="PSUM") as ps_z,
            tc.tile_pool(name="ps_b", bufs=2, space="PSUM") as ps_b,
        ):
            csb = consts.tile([128, 264], bf16)
            wqkh = consts.tile([128, 4, MO, 128], f8)
            wqkl = consts.tile([128, 4, MO, 128], f8)
            wvh = consts.tile([128, MO, CLOC], f8)
            wvl = consts.tile([128, MO, CLOC], f8)
            wo = consts.tile([128, 2, DM], bf16)
            xh = apool.tile([128, MO, S], f8)
            xl = apool.tile([128, MO, S], f8)

            # DMA order = consumption order (transfers serialize on the DMA
            # engines). Startup fans out over three queues (SP: weights,
            # ACT: xh, Pool/SWDGE: xl) so the first QK chain can start ~2.6us in;
            # bulk transfers stay off the ACT queue once the exp stream
            # starts (each dma_start occupies the sequencer ~660ns).
            nc.sync.dma_start(wqkh[:, 0, :, :], wqkh_d[:, 0, :, :])
            nc.scalar.dma_start(xh[:, 0:2, 0:512], xh_d[:, 0:2, 0:512])
            nc.gpsimd.dma_start(xl[:, 0:2, 0:512], xl_d[:, 0:2, 0:512])
            nc.sync.dma_start(wqkl[:, 0, :, :], wqkl_d[:, 0, :, :])
            nc.scalar.dma_start(xh[:, 2:4, 0:512], xh_d[:, 2:4, 0:512])
            nc.gpsimd.dma_start(xl[:, 2:4, 0:512], xl_d[:, 2:4, 0:512])
            nc.sync.dma_start(csb[:], cst_d[:])
            nc.scalar.dma_start(xh[:, 4:8, 0:512], xh_d[:, 4:8, 0:512])
            nc.gpsimd.dma_start(xl[:, 4:8, 0:512], xl_d[:, 4:8, 0:512])
            nc.sync.dma_start(wqkh[:, 1, :, :], wqkh_d[:, 1, :, :])
            nc.sync.dma_start(wqkl[:, 1, :, :], wqkl_d[:, 1, :, :])
            nc.sync.dma_start(wqkh[:, 2:4, :, :], wqkh_d[:, 2:4, :, :])
            nc.sync.dma_start(wqkl[:, 2:4, :, :], wqkl_d[:, 2:4, :, :])
            nc.sync.dma_start(wvh[:], wvh_d[:])
            nc.sync.dma_start(wvl[:], wvl_d[:])
            nc.sync.dma_start(xh[:, :, 512:1024], xh_d[:, :, 512:1024])
            nc.sync.dma_start(xl[:, :, 512:1024], xl_d[:, :, 512:1024])
            nc.sync.dma_start(xh[:, :, 1024:1536], xh_d[:, :, 1024:1536])
            nc.sync.dma_start(xl[:, :, 1024:1536], xl_d[:, :, 1024:1536])
            nc.sync.dma_start(xh[:, :, 1536:2048], xh_d[:, :, 1536:2048])
            nc.sync.dma_start(xl[:, :, 1536:2048], xl_d[:, :, 1536:2048])
            nc.sync.dma_start(wo[:], wo_d[:])

            tri = csb[:, 0:128]
            ident = csb[:, 128:256]
            bq_sb = csb[:, 256:260].bitcast(f32)
            bk_sb = csb[:, 260:264].bitcast(f32)

            QT = apool.tile([128, 2, S], bf16)
            KT = apool.tile([128, 2, S], bf16)
            # V augmented: [t-part, kt, h, 0:64] = v dims (x32), col 64 = 32
            # so the rowsum scale matches the v columns and the x32 cancels
            # in the normalization
            VA = apool.tile([128, NKT, HLOC, 72], bf16)
            nc.vector.memset(VA[:, :, :, 64:65], 32.0)
            zT = apool.tile([128, 2, S], bf16)

            # ---- filler queue: PE work units drained while ScalarE exps ----
            fillers = []          # list of (key, pe_ns, thunk)
            fill_debt = [0.0]
            fill_count = [0]      # fill() invocations, for z-pop delay
            z_pushed_at = {}      # (qg, h) -> fill_count at push
            rr_state = [0]

            def _z_ok(k):
                return (fill_count[0] - z_pushed_at.get((k[1], k[2]), -(1 << 30))
                        >= z_delay)

            cur_qg = [0]

            def _pop_next(op_ok=True):
                """z thunks once ScalarE has had time to produce their exp
                inputs (z_delay fill periods after push), then QKV (needed
                by the next quarter anyway), out-proj last. op thunks are
                hoarded for the late ACT-bound quarters (qg >= op_from),
                where they are the only filler class left."""
                for cls in ("z", "v", "qkv"):
                    for i, (k, ns, thunk) in enumerate(fillers):
                        if k[0] == cls and (cls != "z" or _z_ok(k)):
                            return fillers.pop(i)
                if op_ok:
                    for i, (k, ns, thunk) in enumerate(fillers):
                        if k[0] == "op":
                            return fillers.pop(i)
                    return fillers.pop(0) if fillers else None
                return None

            def fill(budget_ns, op_ok=True):
                fill_count[0] += 1
                budget = budget_ns + fill_debt[0]
                spent = 0.0
                while fillers and spent < budget:
                    nxt = _pop_next(op_ok)
                    if nxt is None:
                        # only hoarded op (or unready z) left: stop without
                        # banking debt so qg2's first fill doesn't burst
                        fill_debt[0] = 0.0
                        return
                    _, ns, thunk = nxt
                    thunk()
                    spent += ns
                fill_debt[0] = budget - spent if fillers else 0.0

            def drain(key):
                """Force-emit queued fillers matching key (dependency
                barrier: attention on quarter qg needs all of QKV(tg=qg))."""
                rest = []
                for k, ns, thunk in fillers:
                    if k == key:
                        thunk()
                    else:
                        rest.append((k, ns, thunk))
                fillers[:] = rest

            def drain_class(cls):
                rest = []
                for k, ns, thunk in fillers:
                    if k[0] == cls:
                        thunk()
                    else:
                        rest.append((k, ns, thunk))
                fillers[:] = rest

            qkv_terms = ((0, 0), (0, 1), (1, 0))  # (x lo?, w lo?) per term

            def emit_qk_chain(tg, ct, j):
                tsl = slice(tg * 512, (tg + 1) * 512)
                dst, b_sb = ((QT, bq_sb), (KT, bk_sb))[j]
                ctj = ct * 2 + j
                ps = ps_b.tile([128, 512], f32, tag="b",
                               name=f"qk_{tg}_{ct}_{j}")
                for sub in range(2):
                    t0 = tg * 512 + sub * 256
                    i = 0
                    for xlo, wlo in qkv_terms:
                        xs = (xh, xl)[xlo]
                        ws = (wqkh, wqkl)[wlo]
                        for c in range(MO // 2):
                            nc.tensor.matmul(
                                ps[:, sub * 256 : sub * 256 + 256],
                                ws[:, ctj, 2 * c : 2 * c + 2, :],
                                xs[:, 2 * c : 2 * c + 2, t0 : t0 + 256],
                                start=(i == 0),
                                stop=(i == 11),
                                perf_mode=DR,
                            )
                            i += 1
                nc.vector.tensor_scalar_add(
                    dst[:, ct, tsl], ps[:], b_sb[:, ct : ct + 1]
                )

            def emit_v_chain(tg, ti):
                tt = tg * 4 + ti
                ps = ps_b.tile([128, 512], f32, tag="b", name=f"v_{tg}_{ti}")
                i = 0
                for xlo, wlo in qkv_terms:
                    xs = (xh, xl)[xlo]
                    ws = (wvh, wvl)[wlo]
                    for c in range(MO // 2):
                        nc.tensor.matmul(
                            ps[:, 0:CLOC],
                            xs[:, 2 * c : 2 * c + 2, tt * 128 : (tt + 1) * 128],
                            ws[:, 2 * c : 2 * c + 2, :],
                            start=(i == 0),
                            stop=(i == 11),
                            perf_mode=DR,
                        )
                        i += 1
                nc.vector.tensor_copy(
                    VA[:, tt, :, 0:64],
                    ps[:, 0:CLOC].rearrange("p (h d) -> p h d", d=64),
                )

            def emit_qkv(tg):
                for ct in range(2):
                    for j in range(2):
                        emit_qk_chain(tg, ct, j)
                for ti in range(4):
                    emit_v_chain(tg, ti)

            def push_qkv_fillers(tg):
                for ct in range(2):
                    for j in range(2):
                        fillers.append(
                            (("qkv", tg), 3072 * PE_NS,
                             lambda tg=tg, ct=ct, j=j: emit_qk_chain(tg, ct, j))
                        )
                vcls = "v" if defer_v else "qkv"
                for ti in range(4):
                    fillers.append(
                        ((vcls, tg), 1536 * PE_NS,
                         lambda tg=tg, ti=ti: emit_v_chain(tg, ti))
                    )

            # ---- attention ----
            def emit_sgrp(h, qg, gi, grp):
                """S^T matmuls for one exp group + the exp + diag masks."""
                hp = (h % 2) * 64
                ct = h // 2
                g0 = qg * 512
                cum = grp[-1][1] + grp[-1][2]
                sreg = ps_s.tile([128, sreg_w], f32, tag="s",
                                 name=f"s_{h}_{qg}_{gi}")
                for kt, off, w in grp:
                    q0 = g0 + 512 - w
                    c0 = off
                    while c0 < off + w:
                        cw = min(off + w - c0, 512 - c0 % 512)
                        nc.tensor.matmul(
                            sreg[:, c0 : c0 + cw],
                            KT[hp : hp + 64, ct, kt * 128 : (kt + 1) * 128],
                            QT[hp : hp + 64, ct,
                               q0 + c0 - off : q0 + c0 - off + cw],
                        )
                        c0 += cw
                pT = ppool.tile([128, sreg_w], bf16, tag="pT",
                                name=f"pT_{h}_{qg}_{gi}")
                no_diag = all(kt * 128 < g0 for kt, _, _ in grp)
                if gi < schr_groups[qg] and no_diag:
                    # Schraudolph exp on DVE: bf16 bits of exp(x*0.125) ~=
                    # int16(x*(0.125*128/ln2) + (127*128 - 5.6)); offloads
                    # the ACT engine (the attention-band pacer) at ~2% rms
                    # error on this group's P entries. sreg is 1024x (Q,K
                    # each carry the x32 weight prescale).
                    nc.vector.tensor_scalar(
                        pT[:, :cum].bitcast(mybir.dt.int16), sreg[:, :cum],
                        scalar1=23.0831253 / 1024.0, op0=mybir.AluOpType.mult,
                        scalar2=16250.4, op1=mybir.AluOpType.add)
                else:
                    nc.scalar.activation(pT[:, :cum], sreg[:, :cum], EXP,
                                         scale=0.125 / 1024.0)
                teng = nc.gpsimd if tri_engine == "gpsimd" else nc.vector
                for kt, off, w in grp:
                    if kt * 128 >= g0:  # diagonal block leads the span
                        teng.tensor_mul(
                            pT[:, off : off + 128],
                            pT[:, off : off + 128],
                            tri[:],
                        )
                return pT

            def emit_zchain(h, qg, qt, pts, kt2g, zp):
                """z[q,65] = sum_kt pT_chunk^T @ V_aug: one sequential PSUM
                accumulation chain per q-tile (a PSUM bank supports only one
                open accumulation group at a time)."""
                g0 = qg * 512
                qa = 4 * qg + qt
                for kt in range(qa + 1):
                    gi, off, w = kt2g[kt]
                    q0 = g0 + 512 - w
                    c0 = off + (g0 + qt * 128) - q0
                    nc.tensor.matmul(
                        zp[:, qt, 0:65],
                        pts[gi][:, c0 : c0 + 128],
                        VA[:, kt, h, 0:65],
                        start=(kt == 0),
                        stop=(kt == qa),
                    )

            def emit_norm(h, qg, zp, zq):
                """1/rowsum fused into the PSUM->SBUF copy of z."""
                hp = (h % 2) * 64
                rec = spool.tile([128, 4, 1], f32, tag="rec",
                                 name=f"rec_{h}_{qg}")
                nc.vector.reciprocal(rec[:], zp[:, :, 64:65])
                with nc.allow_low_precision(reason="attn out to bf16"):
                    for qt in range(4):
                        nc.vector.tensor_scalar_mul(
                            zq[:, qt, hp : hp + 64],
                            zp[:, qt, 0:64],
                            rec[:, qt, :],
                        )

            def emit_transpose(qg, pair, zq):
                """zq [q,128d] -> zT [128d, q] via PE transpose of 4 tiles."""
                quad = ps_b.tile([128, 4, 128], bf16, tag="b",
                                 name=f"tq_{qg}_{pair}")
                for qt in range(4):
                    nc.tensor.transpose(quad[:, qt, :], zq[:, qt, :], ident)
                with nc.allow_low_precision(reason="zT copy"):
                    nc.vector.tensor_copy(
                        zT[:, pair, qg * 512 : (qg + 1) * 512],
                        quad[:].rearrange("p a b -> p (a b)"),
                    )

            def emit_op_half(qg, nh, ti, yA):
                tt = qg * 4 + ti
                ps = ps_b.tile([128, 512], f32, tag="b",
                               name=f"opA_{qg}_{nh}_{ti}")
                nc.tensor.matmul(
                    ps[:],
                    zT[:, 0, tt * 128 : (tt + 1) * 128],
                    wo[:, 0, nh * 512 : (nh + 1) * 512],
                )
                nc.vector.tensor_copy(yA[:, ti, :], ps[:])

            def emit_op_chunk(qg, nh, ti, ysb, dma_split, yA=None):
                tt = qg * 4 + ti
                ps = ps_b.tile([128, 512], f32, tag="b",
                               name=f"op_{qg}_{nh}_{ti}")
                cos = (1,) if yA is not None else (0, 1)
                for co in cos:
                    nc.tensor.matmul(
                        ps[:],
                        zT[:, co, tt * 128 : (tt + 1) * 128],
                        wo[:, co, nh * 512 : (nh + 1) * 512],
                        start=(co == cos[0]),
                        stop=(co == 1),
                    )
                with nc.allow_low_precision(reason="y partial to bf16"):
                    if yA is not None:
                        nc.vector.tensor_add(ysb[:, ti, :], ps[:],
                                             yA[:, ti, :])
                    elif qg == 3:
                        # tail: alternate engines so the copy stream (612ns
                        # each) does not pace the final 427ns-chunk unroll
                        (nc.scalar.copy if ti % 2 == 0
                         else nc.vector.tensor_copy)(ysb[:, ti, :], ps[:])
                    else:
                        nc.vector.tensor_copy(ysb[:, ti, :], ps[:])
                nper = 4 // dma_split
                if ti % nper == nper - 1:
                    t0 = tt - nper + 1
                    # y DMA queue: Pool/SWDGE keeps descriptor gen off the
                    # ACT/SP sequencers and HWDGE during the exp-critical
                    # band; in the qg3 tail (exp done) rotate across all
                    # three so gens parallelize
                    if qg == 3:
                        deng = (nc.gpsimd, nc.sync, nc.scalar)[
                            (ti // nper + nh * dma_split) % 3]
                    else:
                        deng = (nc.gpsimd, nc.sync)[(ti // nper + nh) % 2]
                    deng.dma_start(
                        y_d[t0 * 128 : (tt + 1) * 128,
                            nh * 512 : (nh + 1) * 512].rearrange(
                            "(ti p) n -> p ti n", p=128
                        ),
                        ysb[:, ti - nper + 1 : ti + 1, :],
                    )

            yA_box = {}

            def push_opA_fillers(qg):
                # first half (co=0) of qg3's out-proj: available right after
                # pair 0's transpose, staged to SBUF f32; the co=1 half plus
                # an add happens in the tail
                for nh in range(2):
                    yA = ypool.tile([128, 4, 512], f32, tag="yA", bufs=2,
                                    name=f"yA_{qg}_{nh}")
                    yA_box[(qg, nh)] = yA
                    for ti in range(4):
                        fillers.append(
                            (("op", qg), 512 * PE_NS,
                             lambda qg=qg, nh=nh, ti=ti, yA=yA:
                                 emit_op_half(qg, nh, ti, yA))
                        )

            def push_op_fillers(qg, split=False):
                dma_split = dma_splits[qg]
                for nh in range(2):
                    ysb = ypool.tile([128, 4, 512], bf16, tag="y",
                                     name=f"ysb_{qg}_{nh}")
                    yA = yA_box.get((qg, nh)) if split else None
                    for ti in range(4):
                        fillers.append(
                            (("op", qg), (512 if split else 1024) * PE_NS,
                             lambda qg=qg, nh=nh, ti=ti, ysb=ysb,
                                    ds=dma_split, yA=yA:
                                 emit_op_chunk(qg, nh, ti, ysb, ds, yA))
                        )

            def push_z_phase(qg, h, pts, kt2g, zq_box):
                """Queue head h's z chains + normalize (+ transpose) at the
                FRONT of the filler queue; they drain during head h+1's S
                phase (one-head software pipeline)."""
                box = {}
                # final unit (qg3 h3): per-q-tile norm -> transpose -> zT
                # copy -> that tile's out-proj chunks, so the tail pipelines
                # instead of serializing behind the full head
                tailpipe = qg == 3 and h == HLOC - 1

                def chain(qt):
                    if qt == 0:
                        box["zp"] = ps_z.tile([128, 4, 65], f32, tag="z",
                                              name=f"zp_{h}_{qg}")
                        if h % 2 == 0:
                            zq_box[h // 2] = zqpool.tile(
                                [128, 4, 128], bf16, tag="zq",
                                name=f"zq_{qg}_{h // 2}")
                    emit_zchain(h, qg, qt, pts, kt2g, box["zp"])
                    if tailpipe:
                        zq = zq_box[h // 2]
                        if qt == 0:
                            box["rec"] = spool.tile([128, 4, 1], f32,
                                                    tag="rec",
                                                    name=f"rec_{h}_{qg}")
                            box["ysb"] = [
                                ypool.tile([128, 4, 512], bf16, tag="y",
                                           name=f"ysb_3_{nh}")
                                for nh in range(2)
                            ]
                        rec = box["rec"]
                        zp = box["zp"]
                        nc.vector.reciprocal(rec[:, qt, :], zp[:, qt, 64:65])
                        with nc.allow_low_precision(reason="attn out bf16"):
                            nc.vector.tensor_scalar_mul(
                                zq[:, qt, 64:128], zp[:, qt, 0:64],
                                rec[:, qt, :])
                        quad = ps_b.tile([128, 128], bf16, tag="b",
                                         name=f"tq3_{qt}")
                        nc.tensor.transpose(quad[:], zq[:, qt, :], ident)
                        tt = qg * 4 + qt
                        with nc.allow_low_precision(reason="zT copy"):
                            nc.vector.tensor_copy(
                                zT[:, 1, tt * 128 : (tt + 1) * 128], quad[:])
                        for nh in range(2):
                            fillers.append(
                                (("op", 3), 1024 * PE_NS,
                                 lambda nh=nh, ti=qt:
                                     emit_op_chunk(3, nh, ti,
                                                   box["ysb"][nh],
                                                   dma_splits[3]))
                            )

                def norm():
                    if tailpipe:
                        return
                    emit_norm(h, qg, box["zp"], zq_box[h // 2])
                    if h % 2 == 1:
                        emit_transpose(qg, h // 2, zq_box[h // 2])
                        if h == 1 and qg == 3 and op3_split:
                            push_opA_fillers(qg)
                        if h == HLOC - 1:
                            # quarter finished: queue its out-proj (reads
                            # zT(qg), complete as of this point) and the
                            # next token group's QKV
                            push_op_fillers(qg, split=(qg == 3 and op3_split))
                            if qg + 2 <= 3:
                                push_qkv_fillers(qg + 2)

                # z thunks of heads <= h-2 must fully precede this head's
                # (ps_z has 2 bufs, so h-1's z may still be queued); h's
                # thunks append after h-1's so class-FIFO order holds
                prev = ("z", qg, h - 1)
                rest = []
                for k, ns, thunk in fillers:
                    if k[0] == "z" and k != prev:
                        thunk()
                    else:
                        rest.append((k, ns, thunk))
                fillers[:] = rest
                thunks = []
                for qt in range(4):
                    ncols = (4 * qg + qt + 1) * 65
                    thunks.append(
                        (("z", qg, h), ncols * PE_NS,
                         lambda qt=qt: chain(qt))
                    )
                thunks.append((("z", qg, h), 0.0, norm))
                idx = 0
                for i, (k, _, _) in enumerate(fillers):
                    if k[0] == "z":
                        idx = i + 1
                fillers[idx:idx] = thunks
                z_pushed_at[(qg, h)] = fill_count[0]

            # ---- program ----
            # ct0's Q,K chains (heads 0,1) emit directly so quarter 0's S/exp
            # stream starts as soon as possible; ct1 + V chains become
            # fillers drained during h0/h1's exp (barriers: v at h==1,
            # qkv(ct1) at h==2).
            for j in range(2):
                emit_qk_chain(0, 0, j)
            for jj in range(2):
                fillers.append(
                    (("qkv", 0), 3072 * PE_NS,
                     lambda jj=jj: emit_qk_chain(0, 1, jj))
                )
            for ti in range(4):
                fillers.append(
                    (("v", 0), 1536 * PE_NS,
                     lambda ti=ti: emit_v_chain(0, ti))
                )
            push_qkv_fillers(1)

            zq_box = {}
            for qg in range(4):
                groups = _groups(qg, sreg_w)
                kt2g = {}
                for gi, grp in enumerate(groups):
                    for kt, off, w in grp:
                        kt2g[kt] = (gi, off, w)
                if qg > 0:
                    # barrier: this quarter's S/z read QT/KT/VA of tg=qg
                    drain(("qkv", qg))
                for h in range(HLOC):
                    if h == 1:
                        drain(("v", qg))
                    if h == 2 and qg == 0:
                        drain(("qkv", 0))  # ct1 chains gate heads 2,3
                    pts = []
                    # emit S groups in pairs (back-to-back on PE) so ACT's
                    # exp stream has at most one bubble per pair, not per
                    # group; the 2-buffer S rotation permits exactly 2 ahead
                    qsc, qpad = ((fill_scale, fill_pad) if fill_cfg is None
                                 else fill_cfg[qg])
                    pqg = pair_s if isinstance(pair_s, bool) else pair_s[qg]
                    step = 2 if pqg else 1
                    for g0i in range(0, len(groups), step):
                        pair = groups[g0i : g0i + step]
                        budget = 0.0
                        for gi, grp in zip(range(g0i, g0i + step), pair):
                            pts.append(emit_sgrp(h, qg, gi, grp))
                            cum = grp[-1][1] + grp[-1][2]
                            budget += ((cum * ACT_NS + 185.0) * qsc
                                       + qpad - cum * PE_NS)
                        fill(max(0.0, budget), op_ok=(qg >= op_from))
                    push_z_phase(qg, h, pts, kt2g, zq_box)

            # drain the tail (queue can grow while draining)
            while fillers:
                nxt = _pop_next(True)
                _, _, thunk = nxt if nxt else fillers.pop(0)
                thunk()

    nc.compile()
    return nc


def _pack_w(w):
    # [DM, C] -> [128, MO, C] f32: partition p holds rows {mo*128 + p}
    return np.ascontiguousarray(
        w.reshape(MO, 128, w.shape[1]).transpose(1, 0, 2)
    ).astype(np.float32)


def _split8(a):
    # f32 array -> (hi, lo) e4m3 pair with hi + lo ~= a to ~0.1%
    ah = a.astype(ml_dtypes.float8_e4m3)
    al = (a - ah.astype(np.float32)).astype(ml_dtypes.float8_e4m3)
    return np.ascontiguousarray(ah), np.ascontiguousarray(al)


def make_in_maps(x, w_qkv, b_qkv, w_out):
    # multiplicative post-exp mask: 1 where k <= q (upper incl diag), else 0
    tri = np.tri(128, 128, 0, dtype=np.float32).T.astype(ml_dtypes.bfloat16)
    ident = np.eye(128, dtype=np.float32).astype(ml_dtypes.bfloat16)
    in_maps = []
    for core in range(8):
        b = core // 4
        hg = core % 4
        c0 = hg * CLOC
        csl = slice(c0, c0 + CLOC)

        # packed consts: [128, 264] bf16-typed raw columns. Biases carry the
        # x32 weight prescale (Q,K live at 32x on device).
        cst = np.zeros((128, 264), np.uint16)
        cst[:, 0:128] = tri.view(np.uint16)
        cst[:, 128:256] = ident.view(np.uint16)
        bq = np.ascontiguousarray(
            32.0 * b_qkv[csl].astype(np.float32).reshape(2, 128).T
        )
        bk = np.ascontiguousarray(
            32.0 * b_qkv[DM + c0 : DM + c0 + CLOC].astype(np.float32)
            .reshape(2, 128).T
        )
        cst[:, 256:260] = bq.view(np.uint16).reshape(128, 4)
        cst[:, 260:264] = bk.view(np.uint16).reshape(128, 4)

        wq_p = _pack_w(32.0 * w_qkv[:, csl])
        wk_p = _pack_w(32.0 * w_qkv[:, DM + c0 : DM + c0 + CLOC])
        # [128, ctj, MO, 128]: ctj = ct*2 + j (j=0 -> Q, j=1 -> K)
        wqk = np.concatenate(
            [wq_p[:, None, :, 0:128], wk_p[:, None, :, 0:128],
             wq_p[:, None, :, 128:256], wk_p[:, None, :, 128:256]],
            axis=1,
        )
        wqkh, wqkl = _split8(wqk)
        wvh, wvl = _split8(
            _pack_w(32.0 * w_qkv[:, 2 * DM + c0 : 2 * DM + c0 + CLOC]))
        xTh, xTl = _split8(_pack_w(np.ascontiguousarray(x[b].T)))
        in_maps.append(
            {
                "xTh": xTh,
                "xTl": xTl,
                "wqkh": wqkh,
                "wqkl": wqkl,
                "wvh": wvh,
                "wvl": wvl,
                # wo: [CLOC, DM] -> [128, 2, DM]
                "wo": np.ascontiguousarray(
                    w_out[csl, :].reshape(2, 128, DM).transpose(1, 0, 2)
                ).astype(ml_dtypes.bfloat16),
                "cst": cst.view(ml_dtypes.bfloat16),
            }
        )
    return in_maps


def gather(results, b_qkv, w_out, b_out):
    # device skips the V bias; z_norm + b_v projects to a constant row:
    # y += b_v @ w_out, folded into the output bias here
    b_eff = (
        b_out.astype(np.float32)
        + b_qkv[2 * DM :].astype(np.float32) @ w_out.astype(np.float32)
    )
    out = np.empty((B, S, DM), np.float32)
    for b in range(B):
        acc = results[4 * b]["y"].astype(np.float32)
        for j in range(1, 4):
            acc = acc + results[4 * b + j]["y"]
        out[b] = acc + b_eff[None, :]
    return out


def kernel(x, w_qkv, b_qkv, w_out, b_out):
    x = np.asarray(x)
    w_qkv = np.asarray(w_qkv)
    b_qkv = np.asarray(b_qkv)
    w_out = np.asarray(w_out)
    b_out = np.asarray(b_out)

    if "nc" not in _CACHE:
        _CACHE["nc"] = build()
    nc = _CACHE["nc"]

    in_maps = make_in_maps(x, w_qkv, b_qkv, w_out)
    res = run_bass_kernel_spmd(nc, in_maps, core_ids=list(range(8)))
    return gather(res.results, b_qkv, w_out, b_out)



# revision 46
# speedup vs baseline: 1.1258x; 1.0366x over previous
"""Causal multi-head attention block (B=2, S=2048, D=1024, H=16) on 8 TRN2 cores.

Sharding: core i handles batch b = i//4 and head group hg = i%4 (4 heads =
256 model dims). Each core computes its heads' attention and a partial
output projection; the host sums the 4 partials per batch and adds b_out.

Per-core device pipeline (fp32 PSUM accumulation):
  1. QKV in compensated fp8: x and the QKV weights are hi/lo e4m3 pairs
     (hosts pre-scales weights x32 so e4m3's mantissa range is used;
     hi+lo carries ~11 mantissa bits, more precise than bf16). Each
     chain is xh@wh + xh@wl + xl@wh via DoubleRow matmuls (contraction
     256/matmul at 0.5 cycles/col = 3x bf16 throughput). Q^T,K^T land
     as [head_cols, tokens] at 32x scale (absorbed by the exp scale
     2^-13); V lands as [tokens, head_cols] at 32x, stored augmented
     with a 32.0 column so the z-matmul's row sums carry the same scale
     and the normalization cancels it exactly.
  2. Attention per head in bf16 (fp8 here fails the 2e-2 tolerance),
     flash-style in the S^T = K.Q^T orientation over the causal lower
     triangle only: S^T[k_tile, q_span] -> exp on ScalarE
     (scale=2^-13, no max subtraction; logits ~N(0,1)) -> P^T bf16 ->
     multiplicative 0/1 mask on diagonal blocks -> z[q_tile, 65]
     += P^T_chunk^T @ V_aug accumulated over k tiles in PSUM. The [q, d+1]
     z orientation makes each z matmul only 65 PE columns (vs a full
     q-span) and puts the softmax row sum in PSUM column 64 of the same
     partition as its query, so normalization is a per-partition
     tensor_scalar multiply fused into the PSUM->SBUF copy.
  3. z[q,d] tiles are transposed back to z^T[d,q] via PE transpose
     (identity matmul, 128 cols per 2-head tile) for the out-projection.
  4. Out-proj: y_partial[t, n] accumulated over the 256 local dims.

Program order is a fine-grained software pipeline: the attention loop is
a flat sequence over (q-quarter, head, k-group) with the z matmuls
lagging one group behind the S matmuls, and a filler queue (next token
group's QKV chains, previous quarters' out-proj chunks) drained between
S and z so the PE never waits on ScalarE exp. The final (ACT-bound)
quarter's last head runs a per-q-tile norm -> transpose -> out-proj
pipeline so the tail doesn't serialize, and y DMAs ride the Pool
engine's SWDGE path (+SP/ACT in the tail) to keep descriptor
generation off the exp-critical sequencers. Host pre-packs all inputs
into SBUF layouts; the V bias is folded into the output bias on the
host (b_v @ w_out).
"""

import numpy as np
import ml_dtypes

import concourse.mybir as mybir
import concourse.tile as tile
from concourse import bacc
from concourse.bass_utils import run_bass_kernel_spmd

B = 2
S = 2048
DM = 1024
HD = 64
HLOC = 4                 # heads per core
CLOC = HLOC * HD         # local model dims (256)
MO = DM // 128           # 8 k-subtiles of the model dim
NKT = S // 128           # 16 key tiles

f32 = mybir.dt.float32
bf16 = mybir.dt.bfloat16
f8 = mybir.dt.float8e4
DR = mybir.MatmulPerfMode.DoubleRow
EXP = mybir.ActivationFunctionType.Exp

ACT_NS = 0.8333333333333334
PE_NS = 0.4166666666666667

_CACHE = {}


def _groups(qg, cap=1024):
    """Pack the causal k-tile spans of query quarter qg into exp groups of
    <= cap columns. Returns list of groups; each group is a list of
    (kt, offset_in_group, width)."""
    g0 = qg * 512
    last_kt = 4 * qg + 3
    groups, cur, cum = [], [], 0
    for kt in range(last_kt + 1):
        w = g0 + 512 - max(kt * 128, g0)
        if cum + w > cap:
            groups.append(cur)
            cur, cum = [], 0
        cur.append((kt, cum, w))
        cum += w
    groups.append(cur)
    return groups


def build(pt_bufs=20, zq_bufs=4, y_bufs=4, sreg_w=1024, fill_scale=1.0,
          fill_pad=0.0, dma_splits=(2, 2, 4, 4), tri_engine="dve", op_from=2,
          zp_bufs=2, dma_transpose=True,
          z_delay=1, defer_v=True, copy_split=False, rr_pop=False,
          pair_s=(0, 0, 0, 1), op3_split=False, schr_groups=(0, 0, 0, 0),
          fill_cfg=((1.025, 53.0), (0.995, 56.0), (1.014, 5.0), (0.974, -100.0))):
    nc = bacc.Bacc("TRN2", target_bir_lowering=False, debug=False)

    # x and the QKV weights are fp8 e4m3 hi/lo pairs (weights pre-scaled x32
    # on the host so e4m3's mantissa is used; the x32 scales cancel in the
    # exp scale and the x32 ones-column). Q = xh@wh + xh@wl + xl@wh via
    # DoubleRow matmuls: contraction 256/matmul at 0.5 cycles/col.
    xh_d = nc.dram_tensor("xTh", [128, MO, S], f8, kind="ExternalInput")
    xl_d = nc.dram_tensor("xTl", [128, MO, S], f8, kind="ExternalInput")
    # wqk grouped per QKV chain (ctj = ct*2+j) so each chain's weights are
    # one contiguous DMA
    wqkh_d = nc.dram_tensor("wqkh", [128, 4, MO, 128], f8, kind="ExternalInput")
    wqkl_d = nc.dram_tensor("wqkl", [128, 4, MO, 128], f8, kind="ExternalInput")
    wvh_d = nc.dram_tensor("wvh", [128, MO, CLOC], f8, kind="ExternalInput")
    wvl_d = nc.dram_tensor("wvl", [128, MO, CLOC], f8, kind="ExternalInput")
    wo_d = nc.dram_tensor("wo", [128, 2, DM], bf16, kind="ExternalInput")
    # consts packed as raw bf16 columns: tri[0:128], identity[128:256],
    # bq[256:260], bk[260:264] (f32 values bit-split across bf16 pairs)
    cst_d = nc.dram_tensor("cst", [128, 264], bf16, kind="ExternalInput")
    y_d = nc.dram_tensor("y", [S, DM], bf16, kind="ExternalOutput")

    with tile.TileContext(nc) as tc:
        with (
            tc.tile_pool(name="consts", bufs=1) as consts,
            tc.tile_pool(name="acts", bufs=1) as apool,
            tc.tile_pool(name="pt", bufs=pt_bufs) as ppool,
            tc.tile_pool(name="zq", bufs=zq_bufs) as zqpool,
            tc.tile_pool(name="norm", bufs=4) as spool,
            tc.tile_pool(name="ycopy", bufs=y_bufs) as ypool,
            # 8 PSUM banks: ps_s 2x[128,1024]=4 (QK logits), ps_z
            # 2x[128,4,65]=2 (z accumulators), ps_b 2x[128,512]=2
            # (QKV / V / out-proj chains and z transposes)
            tc.tile_pool(name="ps_s", bufs=2, space="PSUM") as ps_s,
            tc.tile_pool(name="ps_z", bufs=zp_bufs, space="PSUM") as ps_z,
            tc.tile_pool(name="ps_b", bufs=2, space="PSUM") as ps_b,
        ):
            csb = consts.tile([128, 264], bf16)
            wqkh = consts.tile([128, 4, MO, 128], f8)
            wqkl = consts.tile([128, 4, MO, 128], f8)
            wvh = consts.tile([128, MO, CLOC], f8)
            wvl = consts.tile([128, MO, CLOC], f8)
            wo = consts.tile([128, 2, DM], bf16)
            xh = apool.tile([128, MO, S], f8)
            xl = apool.tile([128, MO, S], f8)

            # DMA order = consumption order (transfers serialize on the DMA
            # engines). Startup fans out over three queues (SP: weights,
            # ACT: xh, Pool/SWDGE: xl) so the first QK chain can start ~2.6us in;
            # bulk transfers stay off the ACT queue once the exp stream
            # starts (each dma_start occupies the sequencer ~660ns).
            nc.sync.dma_start(wqkh[:, 0, :, :], wqkh_d[:, 0, :, :])
            nc.scalar.dma_start(xh[:, 0:2, 0:512], xh_d[:, 0:2, 0:512])
            nc.gpsimd.dma_start(xl[:, 0:2, 0:512], xl_d[:, 0:2, 0:512])
            nc.sync.dma_start(wqkl[:, 0, :, :], wqkl_d[:, 0, :, :])
            nc.scalar.dma_start(xh[:, 2:4, 0:512], xh_d[:, 2:4, 0:512])
            nc.gpsimd.dma_start(xl[:, 2:4, 0:512], xl_d[:, 2:4, 0:512])
            nc.sync.dma_start(csb[:], cst_d[:])
            nc.scalar.dma_start(xh[:, 4:8, 0:512], xh_d[:, 4:8, 0:512])
            nc.gpsimd.dma_start(xl[:, 4:8, 0:512], xl_d[:, 4:8, 0:512])
            nc.sync.dma_start(wqkh[:, 1, :, :], wqkh_d[:, 1, :, :])
            nc.sync.dma_start(wqkl[:, 1, :, :], wqkl_d[:, 1, :, :])
            nc.sync.dma_start(wqkh[:, 2:4, :, :], wqkh_d[:, 2:4, :, :])
            nc.sync.dma_start(wqkl[:, 2:4, :, :], wqkl_d[:, 2:4, :, :])
            nc.sync.dma_start(wvh[:], wvh_d[:])
            nc.sync.dma_start(wvl[:], wvl_d[:])
            nc.sync.dma_start(xh[:, :, 512:1024], xh_d[:, :, 512:1024])
            nc.sync.dma_start(xl[:, :, 512:1024], xl_d[:, :, 512:1024])
            nc.sync.dma_start(xh[:, :, 1024:1536], xh_d[:, :, 1024:1536])
            nc.sync.dma_start(xl[:, :, 1024:1536], xl_d[:, :, 1024:1536])
            nc.sync.dma_start(xh[:, :, 1536:2048], xh_d[:, :, 1536:2048])
            nc.sync.dma_start(xl[:, :, 1536:2048], xl_d[:, :, 1536:2048])
            nc.sync.dma_start(wo[:], wo_d[:])

            tri = csb[:, 0:128]
            ident = csb[:, 128:256]
            bq_sb = csb[:, 256:260].bitcast(f32)
            bk_sb = csb[:, 260:264].bitcast(f32)

            QT = apool.tile([128, 2, S], bf16)
            KT = apool.tile([128, 2, S], bf16)
            # V augmented: [t-part, kt, h, 0:64] = v dims (x32), col 64 = 32
            # so the rowsum scale matches the v columns and the x32 cancels
            # in the normalization
            VA = apool.tile([128, NKT, HLOC, 72], bf16)
            nc.vector.memset(VA[:, :, :, 64:65], 32.0)
            zT = apool.tile([128, 2, S], bf16)

            # ---- filler queue: PE work units drained while ScalarE exps ----
            fillers = []          # list of (key, pe_ns, thunk)
            fill_debt = [0.0]
            fill_count = [0]      # fill() invocations, for z-pop delay
            z_pushed_at = {}      # (qg, h) -> fill_count at push
            rr_state = [0]

            def _z_ok(k):
                return (fill_count[0] - z_pushed_at.get((k[1], k[2]), -(1 << 30))
                        >= z_delay)

            cur_qg = [0]

            def _pop_next(op_ok=True):
                """z thunks once ScalarE has had time to produce their exp
                inputs (z_delay fill periods after push), then QKV (needed
                by the next quarter anyway), out-proj last. op thunks are
                hoarded for the late ACT-bound quarters (qg >= op_from),
                where they are the only filler class left."""
                for cls in ("z", "v", "qkv"):
                    for i, (k, ns, thunk) in enumerate(fillers):
                        if k[0] == cls and (cls != "z" or _z_ok(k)):
                            return fillers.pop(i)
                if op_ok:
                    for i, (k, ns, thunk) in enumerate(fillers):
                        if k[0] == "op":
                            return fillers.pop(i)
                    return fillers.pop(0) if fillers else None
                return None

            def fill(budget_ns, op_ok=True):
                fill_count[0] += 1
                budget = budget_ns + fill_debt[0]
                spent = 0.0
                while fillers and spent < budget:
                    nxt = _pop_next(op_ok)
                    if nxt is None:
                        # only hoarded op (or unready z) left: stop without
                        # banking debt so qg2's first fill doesn't burst
                        fill_debt[0] = 0.0
                        return
                    _, ns, thunk = nxt
                    thunk()
                    spent += ns
                fill_debt[0] = budget - spent if fillers else 0.0

            def drain(key):
                """Force-emit queued fillers matching key (dependency
                barrier: attention on quarter qg needs all of QKV(tg=qg))."""
                rest = []
                for k, ns, thunk in fillers:
                    if k == key:
                        thunk()
                    else:
                        rest.append((k, ns, thunk))
                fillers[:] = rest

            def drain_class(cls):
                rest = []
                for k, ns, thunk in fillers:
                    if k[0] == cls:
                        thunk()
                    else:
                        rest.append((k, ns, thunk))
                fillers[:] = rest

            qkv_terms = ((0, 0), (0, 1), (1, 0))  # (x lo?, w lo?) per term

            def emit_qk_chain(tg, ct, j):
                tsl = slice(tg * 512, (tg + 1) * 512)
                dst, b_sb = ((QT, bq_sb), (KT, bk_sb))[j]
                ctj = ct * 2 + j
                ps = ps_b.tile([128, 512], f32, tag="b",
                               name=f"qk_{tg}_{ct}_{j}")
                for sub in range(2):
                    t0 = tg * 512 + sub * 256
                    i = 0
                    for xlo, wlo in qkv_terms:
                        xs = (xh, xl)[xlo]
                        ws = (wqkh, wqkl)[wlo]
                        for c in range(MO // 2):
                            nc.tensor.matmul(
                                ps[:, sub * 256 : sub * 256 + 256],
                                ws[:, ctj, 2 * c : 2 * c + 2, :],
                                xs[:, 2 * c : 2 * c + 2, t0 : t0 + 256],
                                start=(i == 0),
                                stop=(i == 11),
                                perf_mode=DR,
                            )
                            i += 1
                nc.vector.tensor_scalar_add(
                    dst[:, ct, tsl], ps[:], b_sb[:, ct : ct + 1]
                )

            def emit_v_chain(tg, ti):
                tt = tg * 4 + ti
                ps = ps_b.tile([128, 512], f32, tag="b", name=f"v_{tg}_{ti}")
                i = 0
                for xlo, wlo in qkv_terms:
                    xs = (xh, xl)[xlo]
                    ws = (wvh, wvl)[wlo]
                    for c in range(MO // 2):
                        nc.tensor.matmul(
                            ps[:, 0:CLOC],
                            xs[:, 2 * c : 2 * c + 2, tt * 128 : (tt + 1) * 128],
                            ws[:, 2 * c : 2 * c + 2, :],
                            start=(i == 0),
                            stop=(i == 11),
                            perf_mode=DR,
                        )
                        i += 1
                nc.vector.tensor_copy(
                    VA[:, tt, :, 0:64],
                    ps[:, 0:CLOC].rearrange("p (h d) -> p h d", d=64),
                )

            def emit_qkv(tg):
                for ct in range(2):
                    for j in range(2):
                        emit_qk_chain(tg, ct, j)
                for ti in range(4):
                    emit_v_chain(tg, ti)

            def push_qkv_fillers(tg):
                for ct in range(2):
                    for j in range(2):
                        fillers.append(
                            (("qkv", tg), 3072 * PE_NS,
                             lambda tg=tg, ct=ct, j=j: emit_qk_chain(tg, ct, j))
                        )
                vcls = "v" if defer_v else "qkv"
                for ti in range(4):
                    fillers.append(
                        ((vcls, tg), 1536 * PE_NS,
                         lambda tg=tg, ti=ti: emit_v_chain(tg, ti))
                    )

            # ---- attention ----
            def emit_sgrp(h, qg, gi, grp):
                """S^T matmuls for one exp group + the exp + diag masks."""
                hp = (h % 2) * 64
                ct = h // 2
                g0 = qg * 512
                cum = grp[-1][1] + grp[-1][2]
                sreg = ps_s.tile([128, sreg_w], f32, tag="s",
                                 name=f"s_{h}_{qg}_{gi}")
                for kt, off, w in grp:
                    q0 = g0 + 512 - w
                    c0 = off
                    while c0 < off + w:
                        cw = min(off + w - c0, 512 - c0 % 512)
                        nc.tensor.matmul(
                            sreg[:, c0 : c0 + cw],
                            KT[hp : hp + 64, ct, kt * 128 : (kt + 1) * 128],
                            QT[hp : hp + 64, ct,
                               q0 + c0 - off : q0 + c0 - off + cw],
                        )
                        c0 += cw
                pT = ppool.tile([128, sreg_w], bf16, tag="pT",
                                name=f"pT_{h}_{qg}_{gi}")
                no_diag = all(kt * 128 < g0 for kt, _, _ in grp)
                if gi < schr_groups[qg] and no_diag:
                    # Schraudolph exp on DVE: bf16 bits of exp(x*0.125) ~=
                    # int16(x*(0.125*128/ln2) + (127*128 - 5.6)); offloads
                    # the ACT engine (the attention-band pacer) at ~2% rms
                    # error on this group's P entries. sreg is 1024x (Q,K
                    # each carry the x32 weight prescale).
                    nc.vector.tensor_scalar(
                        pT[:, :cum].bitcast(mybir.dt.int16), sreg[:, :cum],
                        scalar1=23.0831253 / 1024.0, op0=mybir.AluOpType.mult,
                        scalar2=16250.4, op1=mybir.AluOpType.add)
                else:
                    nc.scalar.activation(pT[:, :cum], sreg[:, :cum], EXP,
                                         scale=0.125 / 1024.0)
                teng = nc.gpsimd if tri_engine == "gpsimd" else nc.vector
                for kt, off, w in grp:
                    if kt * 128 >= g0:  # diagonal block leads the span
                        teng.tensor_mul(
                            pT[:, off : off + 128],
                            pT[:, off : off + 128],
                            tri[:],
                        )
                return pT

            def emit_zchain(h, qg, qt, pts, kt2g, zp):
                """z[q,65] = sum_kt pT_chunk^T @ V_aug: one sequential PSUM
                accumulation chain per q-tile (a PSUM bank supports only one
                open accumulation group at a time)."""
                g0 = qg * 512
                qa = 4 * qg + qt
                for kt in range(qa + 1):
                    gi, off, w = kt2g[kt]
                    q0 = g0 + 512 - w
                    c0 = off + (g0 + qt * 128) - q0
                    nc.tensor.matmul(
                        zp[:, qt, 0:65],
                        pts[gi][:, c0 : c0 + 128],
                        VA[:, kt, h, 0:65],
                        start=(kt == 0),
                        stop=(kt == qa),
                    )

            def emit_norm(h, qg, zp, zq):
                """1/rowsum fused into the PSUM->SBUF copy of z."""
                hp = (h % 2) * 64
                rec = spool.tile([128, 4, 1], f32, tag="rec",
                                 name=f"rec_{h}_{qg}")
                nc.vector.reciprocal(rec[:], zp[:, :, 64:65])
                with nc.allow_low_precision(reason="attn out to bf16"):
                    for qt in range(4):
                        nc.vector.tensor_scalar_mul(
                            zq[:, qt, hp : hp + 64],
                            zp[:, qt, 0:64],
                            rec[:, qt, :],
                        )

            def emit_transpose(qg, pair, zq):
                """zq [q,128d] -> zT [128d, q]. All these transposes are
                latency-tolerant (op(qg) reads zT several us later), so they
                ride the DMA xbar instead of PE+DVE; only the tail-critical
                qg3/pair1 path (in the tailpipe) uses the PE transpose."""
                if dma_transpose:
                    for qt in range(4):
                        tt = qg * 4 + qt
                        nc.sync.dma_start_transpose(
                            zT[:, pair, tt * 128 : (tt + 1) * 128],
                            zq[:, qt, :],
                        )
                    return
                quad = ps_b.tile([128, 4, 128], bf16, tag="b",
                                 name=f"tq_{qg}_{pair}")
                for qt in range(4):
                    nc.tensor.transpose(quad[:, qt, :], zq[:, qt, :], ident)
                with nc.allow_low_precision(reason="zT copy"):
                    nc.vector.tensor_copy(
                        zT[:, pair, qg * 512 : (qg + 1) * 512],
                        quad[:].rearrange("p a b -> p (a b)"),
                    )

            def emit_op_half(qg, nh, ti, yA):
                tt = qg * 4 + ti
                ps = ps_b.tile([128, 512], f32, tag="b",
                               name=f"opA_{qg}_{nh}_{ti}")
                nc.tensor.matmul(
                    ps[:],
                    zT[:, 0, tt * 128 : (tt + 1) * 128],
                    wo[:, 0, nh * 512 : (nh + 1) * 512],
                )
                nc.vector.tensor_copy(yA[:, ti, :], ps[:])

            def emit_op_chunk(qg, nh, ti, ysb, dma_split, yA=None):
                tt = qg * 4 + ti
                ps = ps_b.tile([128, 512], f32, tag="b",
                               name=f"op_{qg}_{nh}_{ti}")
                cos = (1,) if yA is not None else (0, 1)
                for co in cos:
                    nc.tensor.matmul(
                        ps[:],
                        zT[:, co, tt * 128 : (tt + 1) * 128],
                        wo[:, co, nh * 512 : (nh + 1) * 512],
                        start=(co == cos[0]),
                        stop=(co == 1),
                    )
                with nc.allow_low_precision(reason="y partial to bf16"):
                    if yA is not None:
                        nc.vector.tensor_add(ysb[:, ti, :], ps[:],
                                             yA[:, ti, :])
                    elif qg == 3:
                        # tail: alternate engines so the copy stream (612ns
                        # each) does not pace the final 427ns-chunk unroll
                        (nc.scalar.copy if ti % 2 == 0
                         else nc.vector.tensor_copy)(ysb[:, ti, :], ps[:])
                    else:
                        nc.vector.tensor_copy(ysb[:, ti, :], ps[:])
                nper = 4 // dma_split
                if ti % nper == nper - 1:
                    t0 = tt - nper + 1
                    # y DMA queue: Pool/SWDGE keeps descriptor gen off the
                    # ACT/SP sequencers and HWDGE during the exp-critical
                    # band; in the qg3 tail (exp done) rotate across all
                    # three so gens parallelize
                    if qg == 3:
                        deng = (nc.gpsimd, nc.sync, nc.scalar)[
                            (ti // nper + nh * dma_split) % 3]
                    else:
                        deng = (nc.gpsimd, nc.sync)[(ti // nper + nh) % 2]
                    deng.dma_start(
                        y_d[t0 * 128 : (tt + 1) * 128,
                            nh * 512 : (nh + 1) * 512].rearrange(
                            "(ti p) n -> p ti n", p=128
                        ),
                        ysb[:, ti - nper + 1 : ti + 1, :],
                    )

            yA_box = {}

            def push_opA_fillers(qg):
                # first half (co=0) of qg3's out-proj: available right after
                # pair 0's transpose, staged to SBUF f32; the co=1 half plus
                # an add happens in the tail
                for nh in range(2):
                    yA = ypool.tile([128, 4, 512], f32, tag="yA", bufs=2,
                                    name=f"yA_{qg}_{nh}")
                    yA_box[(qg, nh)] = yA
                    for ti in range(4):
                        fillers.append(
                            (("op", qg), 512 * PE_NS,
                             lambda qg=qg, nh=nh, ti=ti, yA=yA:
                                 emit_op_half(qg, nh, ti, yA))
                        )

            def push_op_fillers(qg, split=False):
                dma_split = dma_splits[qg]
                for nh in range(2):
                    ysb = ypool.tile([128, 4, 512], bf16, tag="y",
                                     name=f"ysb_{qg}_{nh}")
                    yA = yA_box.get((qg, nh)) if split else None
                    for ti in range(4):
                        fillers.append(
                            (("op", qg), (512 if split else 1024) * PE_NS,
                             lambda qg=qg, nh=nh, ti=ti, ysb=ysb,
                                    ds=dma_split, yA=yA:
                                 emit_op_chunk(qg, nh, ti, ysb, ds, yA))
                        )

            def push_z_phase(qg, h, pts, kt2g, zq_box):
                """Queue head h's z chains + normalize (+ transpose) at the
                FRONT of the filler queue; they drain during head h+1's S
                phase (one-head software pipeline)."""
                box = {}
                # final unit (qg3 h3): per-q-tile norm -> transpose -> zT
                # copy -> that tile's out-proj chunks, so the tail pipelines
                # instead of serializing behind the full head
                tailpipe = qg == 3 and h == HLOC - 1

                def chain(qt):
                    if qt == 0:
                        box["zp"] = ps_z.tile([128, 4, 65], f32, tag="z",
                                              name=f"zp_{h}_{qg}")
                        if h % 2 == 0:
                            zq_box[h // 2] = zqpool.tile(
                                [128, 4, 128], bf16, tag="zq",
                                name=f"zq_{qg}_{h // 2}")
                    emit_zchain(h, qg, qt, pts, kt2g, box["zp"])
                    if tailpipe:
                        zq = zq_box[h // 2]
                        if qt == 0:
                            box["rec"] = spool.tile([128, 4, 1], f32,
                                                    tag="rec",
                                                    name=f"rec_{h}_{qg}")
                            box["ysb"] = [
                                ypool.tile([128, 4, 512], bf16, tag="y",
                                           name=f"ysb_3_{nh}")
                                for nh in range(2)
                            ]
                        rec = box["rec"]
                        zp = box["zp"]
                        nc.vector.reciprocal(rec[:, qt, :], zp[:, qt, 64:65])
                        with nc.allow_low_precision(reason="attn out bf16"):
                            nc.vector.tensor_scalar_mul(
                                zq[:, qt, 64:128], zp[:, qt, 0:64],
                                rec[:, qt, :])
                        quad = ps_b.tile([128, 128], bf16, tag="b",
                                         name=f"tq3_{qt}")
                        nc.tensor.transpose(quad[:], zq[:, qt, :], ident)
                        tt = qg * 4 + qt
                        with nc.allow_low_precision(reason="zT copy"):
                            nc.vector.tensor_copy(
                                zT[:, 1, tt * 128 : (tt + 1) * 128], quad[:])
                        for nh in range(2):
                            fillers.append(
                                (("op", 3), 1024 * PE_NS,
                                 lambda nh=nh, ti=qt:
                                     emit_op_chunk(3, nh, ti,
                                                   box["ysb"][nh],
                                                   dma_splits[3]))
                            )

                def norm():
                    if tailpipe:
                        return
                    emit_norm(h, qg, box["zp"], zq_box[h // 2])
                    if h % 2 == 1:
                        emit_transpose(qg, h // 2, zq_box[h // 2])
                        if h == 1 and qg == 3 and op3_split:
                            push_opA_fillers(qg)
                        if h == HLOC - 1:
                            # quarter finished: queue its out-proj (reads
                            # zT(qg), complete as of this point) and the
                            # next token group's QKV
                            push_op_fillers(qg, split=(qg == 3 and op3_split))
                            if qg + 2 <= 3:
                                push_qkv_fillers(qg + 2)

                # z thunks of heads <= h-2 must fully precede this head's
                # (ps_z has 2 bufs, so h-1's z may still be queued); h's
                # thunks append after h-1's so class-FIFO order holds
                prev = ("z", qg, h - 1)
                rest = []
                for k, ns, thunk in fillers:
                    if k[0] == "z" and k != prev:
                        thunk()
                    else:
                        rest.append((k, ns, thunk))
                fillers[:] = rest
                thunks = []
                for qt in range(4):
                    ncols = (4 * qg + qt + 1) * 65
                    thunks.append(
                        (("z", qg, h), ncols * PE_NS,
                         lambda qt=qt: chain(qt))
                    )
                thunks.append((("z", qg, h), 0.0, norm))
                idx = 0
                for i, (k, _, _) in enumerate(fillers):
                    if k[0] == "z":
                        idx = i + 1
                fillers[idx:idx] = thunks
                z_pushed_at[(qg, h)] = fill_count[0]

            # ---- program ----
            # ct0's Q,K chains (heads 0,1) emit directly so quarter 0's S/exp
            # stream starts as soon as possible; ct1 + V chains become
            # fillers drained during h0/h1's exp (barriers: v at h==1,
            # qkv(ct1) at h==2).
            for j in range(2):
                emit_qk_chain(0, 0, j)
            for jj in range(2):
                fillers.append(
                    (("qkv", 0), 3072 * PE_NS,
                     lambda jj=jj: emit_qk_chain(0, 1, jj))
                )
            for ti in range(4):
                fillers.append(
                    (("v", 0), 1536 * PE_NS,
                     lambda ti=ti: emit_v_chain(0, ti))
                )
            push_qkv_fillers(1)

            zq_box = {}
            for qg in range(4):
                groups = _groups(qg, sreg_w)
                kt2g = {}
                for gi, grp in enumerate(groups):
                    for kt, off, w in grp:
                        kt2g[kt] = (gi, off, w)
                if qg > 0:
                    # barrier: this quarter's S/z read QT/KT/VA of tg=qg
                    drain(("qkv", qg))
                for h in range(HLOC):
                    if h == 1:
                        drain(("v", qg))
                    if h == 2 and qg == 0:
                        drain(("qkv", 0))  # ct1 chains gate heads 2,3
                    pts = []
                    # emit S groups in pairs (back-to-back on PE) so ACT's
                    # exp stream has at most one bubble per pair, not per
                    # group; the 2-buffer S rotation permits exactly 2 ahead
                    qsc, qpad = ((fill_scale, fill_pad) if fill_cfg is None
                                 else fill_cfg[qg])
                    pqg = pair_s if isinstance(pair_s, bool) else pair_s[qg]
                    step = 2 if pqg else 1
                    for g0i in range(0, len(groups), step):
                        pair = groups[g0i : g0i + step]
                        budget = 0.0
                        for gi, grp in zip(range(g0i, g0i + step), pair):
                            pts.append(emit_sgrp(h, qg, gi, grp))
                            cum = grp[-1][1] + grp[-1][2]
                            budget += ((cum * ACT_NS + 185.0) * qsc
                                       + qpad - cum * PE_NS)
                        fill(max(0.0, budget), op_ok=(qg >= op_from))
                    push_z_phase(qg, h, pts, kt2g, zq_box)

            # drain the tail (queue can grow while draining)
            while fillers:
                nxt = _pop_next(True)
                _, _, thunk = nxt if nxt else fillers.pop(0)
                thunk()

    nc.compile()
    return nc


def _pack_w(w):
    # [DM, C] -> [128, MO, C] f32: partition p holds rows {mo*128 + p}
    return np.ascontiguousarray(
        w.reshape(MO, 128, w.shape[1]).transpose(1, 0, 2)
    ).astype(np.float32)


def _split8(a):
    # f32 array -> (hi, lo) e4m3 pair with hi + lo ~= a to ~0.1%
    ah = a.astype(ml_dtypes.float8_e4m3)
    al = (a - ah.astype(np.float32)).astype(ml_dtypes.float8_e4m3)
    return np.ascontiguousarray(ah), np.ascontiguousarray(al)


def make_in_maps(x, w_qkv, b_qkv, w_out):
    # multiplicative post-exp mask: 1 where k <= q (upper incl diag), else 0
    tri = np.tri(128, 128, 0, dtype=np.float32).T.astype(ml_dtypes.bfloat16)
    ident = np.eye(128, dtype=np.float32).astype(ml_dtypes.bfloat16)
    in_maps = []
    for core in range(8):
        b = core // 4
        hg = core % 4
        c0 = hg * CLOC
        csl = slice(c0, c0 + CLOC)

        # packed consts: [128, 264] bf16-typed raw columns. Biases carry the
        # x32 weight prescale (Q,K live at 32x on device).
        cst = np.zeros((128, 264), np.uint16)
        cst[:, 0:128] = tri.view(np.uint16)
        cst[:, 128:256] = ident.view(np.uint16)
        bq = np.ascontiguousarray(
            32.0 * b_qkv[csl].astype(np.float32).reshape(2, 128).T
        )
        bk = np.ascontiguousarray(
            32.0 * b_qkv[DM + c0 : DM + c0 + CLOC].astype(np.float32)
            .reshape(2, 128).T
        )
        cst[:, 256:260] = bq.view(np.uint16).reshape(128, 4)
        cst[:, 260:264] = bk.view(np.uint16).reshape(128, 4)

        wq_p = _pack_w(32.0 * w_qkv[:, csl])
        wk_p = _pack_w(32.0 * w_qkv[:, DM + c0 : DM + c0 + CLOC])
        # [128, ctj, MO, 128]: ctj = ct*2 + j (j=0 -> Q, j=1 -> K)
        wqk = np.concatenate(
            [wq_p[:, None, :, 0:128], wk_p[:, None, :, 0:128],
             wq_p[:, None, :, 128:256], wk_p[:, None, :, 128:256]],
            axis=1,
        )
        wqkh, wqkl = _split8(wqk)
        wvh, wvl = _split8(
            _pack_w(32.0 * w_qkv[:, 2 * DM + c0 : 2 * DM + c0 + CLOC]))
        xTh, xTl = _split8(_pack_w(np.ascontiguousarray(x[b].T)))
        in_maps.append(
            {
                "xTh": xTh,
                "xTl": xTl,
                "wqkh": wqkh,
                "wqkl": wqkl,
                "wvh": wvh,
                "wvl": wvl,
                # wo: [CLOC, DM] -> [128, 2, DM]
                "wo": np.ascontiguousarray(
                    w_out[csl, :].reshape(2, 128, DM).transpose(1, 0, 2)
                ).astype(ml_dtypes.bfloat16),
                "cst": cst.view(ml_dtypes.bfloat16),
            }
        )
    return in_maps


def gather(results, b_qkv, w_out, b_out):
    # device skips the V bias; z_norm + b_v projects to a constant row:
    # y += b_v @ w_out, folded into the output bias here
    b_eff = (
        b_out.astype(np.float32)
        + b_qkv[2 * DM :].astype(np.float32) @ w_out.astype(np.float32)
    )
    out = np.empty((B, S, DM), np.float32)
    for b in range(B):
        acc = results[4 * b]["y"].astype(np.float32)
        for j in range(1, 4):
            acc = acc + results[4 * b + j]["y"]
        out[b] = acc + b_eff[None, :]
    return out


def kernel(x, w_qkv, b_qkv, w_out, b_out):
    x = np.asarray(x)
    w_qkv = np.asarray(w_qkv)
    b_qkv = np.asarray(b_qkv)
    w_out = np.asarray(w_out)
    b_out = np.asarray(b_out)

    if "nc" not in _CACHE:
        _CACHE["nc"] = build()
    nc = _CACHE["nc"]

    in_maps = make_in_maps(x, w_qkv, b_qkv, w_out)
    res = run_bass_kernel_spmd(nc, in_maps, core_ids=list(range(8)))
    return gather(res.results, b_qkv, w_out, b_out)



# revision 47
# speedup vs baseline: 1.1422x; 1.0146x over previous
"""Causal multi-head attention block (B=2, S=2048, D=1024, H=16) on 8 TRN2 cores.

Sharding: core i handles batch b = i//4 and head group hg = i%4 (4 heads =
256 model dims). Each core computes its heads' attention and a partial
output projection; the host sums the 4 partials per batch and adds b_out.

Per-core device pipeline (fp32 PSUM accumulation):
  1. QKV in compensated fp8: x and the QKV weights are hi/lo e4m3 pairs
     (hosts pre-scales weights x32 so e4m3's mantissa range is used;
     hi+lo carries ~11 mantissa bits, more precise than bf16). Each
     chain is xh@wh + xh@wl + xl@wh via DoubleRow matmuls (contraction
     256/matmul at 0.5 cycles/col = 3x bf16 throughput). Q^T,K^T land
     as [head_cols, tokens] at 32x scale (absorbed by the exp scale
     2^-13); V lands as [tokens, head_cols] at 32x, stored augmented
     with a 32.0 column so the z-matmul's row sums carry the same scale
     and the normalization cancels it exactly.
  2. Attention per head in bf16 (fp8 here fails the 2e-2 tolerance),
     flash-style in the S^T = K.Q^T orientation over the causal lower
     triangle only: S^T[k_tile, q_span] -> exp on ScalarE
     (scale=2^-13, no max subtraction; logits ~N(0,1)) -> P^T bf16 ->
     multiplicative 0/1 mask on diagonal blocks -> z[q_tile, 65]
     += P^T_chunk^T @ V_aug accumulated over k tiles in PSUM. The [q, d+1]
     z orientation makes each z matmul only 65 PE columns (vs a full
     q-span) and puts the softmax row sum in PSUM column 64 of the same
     partition as its query, so normalization is a per-partition
     tensor_scalar multiply fused into the PSUM->SBUF copy.
  3. z[q,d] tiles are transposed back to z^T[d,q] via PE transpose
     (identity matmul, 128 cols per 2-head tile) for the out-projection.
  4. Out-proj: y_partial[t, n] accumulated over the 256 local dims.

Program order is a fine-grained software pipeline: the attention loop is
a flat sequence over (q-quarter, head, k-group) with the z matmuls
lagging one group behind the S matmuls, and a filler queue (next token
group's QKV chains, previous quarters' out-proj chunks) drained between
S and z so the PE never waits on ScalarE exp. The final (ACT-bound)
quarter's last head runs a per-q-tile norm -> transpose -> out-proj
pipeline so the tail doesn't serialize, and y DMAs ride the Pool
engine's SWDGE path (+SP/ACT in the tail) to keep descriptor
generation off the exp-critical sequencers. Host pre-packs all inputs
into SBUF layouts; the V bias is folded into the output bias on the
host (b_v @ w_out).
"""

import numpy as np
import ml_dtypes

import concourse.mybir as mybir
import concourse.tile as tile
from concourse import bacc
from concourse.bass_utils import run_bass_kernel_spmd

B = 2
S = 2048
DM = 1024
HD = 64
HLOC = 4                 # heads per core
CLOC = HLOC * HD         # local model dims (256)
MO = DM // 128           # 8 k-subtiles of the model dim
NKT = S // 128           # 16 key tiles

f32 = mybir.dt.float32
bf16 = mybir.dt.bfloat16
f8 = mybir.dt.float8e4
DR = mybir.MatmulPerfMode.DoubleRow
EXP = mybir.ActivationFunctionType.Exp

ACT_NS = 0.8333333333333334
PE_NS = 0.4166666666666667

_CACHE = {}


def _groups(qg, cap=1024):
    """Pack the causal k-tile spans of query quarter qg into exp groups of
    <= cap columns. Returns list of groups; each group is a list of
    (kt, offset_in_group, width)."""
    g0 = qg * 512
    last_kt = 4 * qg + 3
    groups, cur, cum = [], [], 0
    for kt in range(last_kt + 1):
        w = g0 + 512 - max(kt * 128, g0)
        if cum + w > cap:
            groups.append(cur)
            cur, cum = [], 0
        cur.append((kt, cum, w))
        cum += w
    groups.append(cur)
    return groups


def build(pt_bufs=20, zq_bufs=4, y_bufs=4, sreg_w=1024, fill_scale=1.0,
          fill_pad=0.0, dma_splits=(2, 2, 4, 4), tri_engine="dve", op_from=2,
          zp_bufs=2, dma_transpose=True,
          z_delay=1, defer_v=True, copy_split=False, rr_pop=False,
          pair_s=(0, 0, 0, 1), op3_split=False, schr_groups=(0, 0, 0, 0),
          fill_cfg=((1.025, 53.0), (0.995, 56.0), (1.014, 5.0), (0.974, -100.0))):
    nc = bacc.Bacc("TRN2", target_bir_lowering=False, debug=False)

    # x and the QKV weights are fp8 e4m3 hi/lo pairs (weights pre-scaled x32
    # on the host so e4m3's mantissa is used; the x32 scales cancel in the
    # exp scale and the x32 ones-column). Q = xh@wh + xh@wl + xl@wh via
    # DoubleRow matmuls: contraction 256/matmul at 0.5 cycles/col.
    xh_d = nc.dram_tensor("xTh", [128, MO, S], f8, kind="ExternalInput")
    xl_d = nc.dram_tensor("xTl", [128, MO, S], f8, kind="ExternalInput")
    # wqk grouped per QKV chain (ctj = ct*2+j) so each chain's weights are
    # one contiguous DMA
    wqkh_d = nc.dram_tensor("wqkh", [128, 4, MO, 128], f8, kind="ExternalInput")
    wqkl_d = nc.dram_tensor("wqkl", [128, 4, MO, 128], f8, kind="ExternalInput")
    wvh_d = nc.dram_tensor("wvh", [128, MO, CLOC], f8, kind="ExternalInput")
    wvl_d = nc.dram_tensor("wvl", [128, MO, CLOC], f8, kind="ExternalInput")
    wo_d = nc.dram_tensor("wo", [128, 2, DM], bf16, kind="ExternalInput")
    # consts packed as raw bf16 columns: tri[0:128], identity[128:256],
    # bq[256:260], bk[260:264] (f32 values bit-split across bf16 pairs)
    cst_d = nc.dram_tensor("cst", [128, 264], bf16, kind="ExternalInput")
    y_d = nc.dram_tensor("y", [S, DM], bf16, kind="ExternalOutput")

    with tile.TileContext(nc) as tc:
        with (
            tc.tile_pool(name="consts", bufs=1) as consts,
            tc.tile_pool(name="acts", bufs=1) as apool,
            tc.tile_pool(name="pt", bufs=pt_bufs) as ppool,
            tc.tile_pool(name="zq", bufs=zq_bufs) as zqpool,
            tc.tile_pool(name="norm", bufs=4) as spool,
            tc.tile_pool(name="ycopy", bufs=y_bufs) as ypool,
            # 8 PSUM banks: ps_s 2x[128,1024]=4 (QK logits), ps_z
            # 2x[128,4,65]=2 (z accumulators), ps_b 2x[128,512]=2
            # (QKV / V / out-proj chains and z transposes)
            tc.tile_pool(name="ps_s", bufs=2, space="PSUM") as ps_s,
            tc.tile_pool(name="ps_z", bufs=zp_bufs, space="PSUM") as ps_z,
            tc.tile_pool(name="ps_b", bufs=2, space="PSUM") as ps_b,
        ):
            csb = consts.tile([128, 264], bf16)
            wqkh = consts.tile([128, 4, MO, 128], f8)
            wqkl = consts.tile([128, 4, MO, 128], f8)
            wvh = consts.tile([128, MO, CLOC], f8)
            wvl = consts.tile([128, MO, CLOC], f8)
            wo = consts.tile([128, 2, DM], bf16)
            xh = apool.tile([128, MO, S], f8)
            xl = apool.tile([128, MO, S], f8)

            # DMA order = consumption order (transfers serialize on the DMA
            # engines). Startup fans out over three queues (SP: weights,
            # ACT: xh, Pool/SWDGE: xl) so the first QK chain can start ~2.6us in;
            # bulk transfers stay off the ACT queue once the exp stream
            # starts (each dma_start occupies the sequencer ~660ns).
            nc.sync.dma_start(wqkh[:, 0, :, :], wqkh_d[:, 0, :, :])
            nc.scalar.dma_start(xh[:, 0:2, 0:512], xh_d[:, 0:2, 0:512])
            nc.gpsimd.dma_start(xl[:, 0:2, 0:512], xl_d[:, 0:2, 0:512])
            nc.sync.dma_start(wqkl[:, 0, :, :], wqkl_d[:, 0, :, :])
            nc.scalar.dma_start(xh[:, 2:4, 0:512], xh_d[:, 2:4, 0:512])
            nc.gpsimd.dma_start(xl[:, 2:4, 0:512], xl_d[:, 2:4, 0:512])
            nc.sync.dma_start(csb[:], cst_d[:])
            nc.scalar.dma_start(xh[:, 4:8, 0:512], xh_d[:, 4:8, 0:512])
            nc.gpsimd.dma_start(xl[:, 4:8, 0:512], xl_d[:, 4:8, 0:512])
            nc.sync.dma_start(wqkh[:, 1, :, :], wqkh_d[:, 1, :, :])
            nc.sync.dma_start(wqkl[:, 1, :, :], wqkl_d[:, 1, :, :])
            nc.sync.dma_start(wqkh[:, 2:4, :, :], wqkh_d[:, 2:4, :, :])
            nc.sync.dma_start(wqkl[:, 2:4, :, :], wqkl_d[:, 2:4, :, :])
            nc.sync.dma_start(wvh[:], wvh_d[:])
            nc.sync.dma_start(wvl[:], wvl_d[:])
            nc.sync.dma_start(xh[:, :, 512:1024], xh_d[:, :, 512:1024])
            nc.sync.dma_start(xl[:, :, 512:1024], xl_d[:, :, 512:1024])
            nc.sync.dma_start(xh[:, :, 1024:1536], xh_d[:, :, 1024:1536])
            nc.sync.dma_start(xl[:, :, 1024:1536], xl_d[:, :, 1024:1536])
            nc.sync.dma_start(xh[:, :, 1536:2048], xh_d[:, :, 1536:2048])
            nc.sync.dma_start(xl[:, :, 1536:2048], xl_d[:, :, 1536:2048])
            nc.sync.dma_start(wo[:], wo_d[:])

            tri = csb[:, 0:128]
            ident = csb[:, 128:256]
            bq_sb = csb[:, 256:260].bitcast(f32)
            bk_sb = csb[:, 260:264].bitcast(f32)

            QT = apool.tile([128, 2, S], bf16)
            KT = apool.tile([128, 2, S], bf16)
            # V augmented: [t-part, kt, h, 0:64] = v dims (x32), col 64 = 32
            # so the rowsum scale matches the v columns and the x32 cancels
            # in the normalization
            VA = apool.tile([128, NKT, HLOC, 72], bf16)
            nc.vector.memset(VA[:, :, :, 64:65], 32.0)
            zT = apool.tile([128, 2, S], bf16)

            # ---- filler queue: PE work units drained while ScalarE exps ----
            fillers = []          # list of (key, pe_ns, thunk)
            fill_debt = [0.0]
            fill_count = [0]      # fill() invocations, for z-pop delay
            z_pushed_at = {}      # (qg, h) -> fill_count at push
            rr_state = [0]

            def _z_ok(k):
                return (fill_count[0] - z_pushed_at.get((k[1], k[2]), -(1 << 30))
                        >= z_delay)

            cur_qg = [0]

            def _pop_next(op_ok=True):
                """z thunks once ScalarE has had time to produce their exp
                inputs (z_delay fill periods after push), then QKV (needed
                by the next quarter anyway), out-proj last. op thunks are
                hoarded for the late ACT-bound quarters (qg >= op_from),
                where they are the only filler class left."""
                for cls in ("z", "v", "qkv"):
                    for i, (k, ns, thunk) in enumerate(fillers):
                        if k[0] == cls and (cls != "z" or _z_ok(k)):
                            return fillers.pop(i)
                if op_ok:
                    for i, (k, ns, thunk) in enumerate(fillers):
                        if k[0] == "op":
                            return fillers.pop(i)
                    return fillers.pop(0) if fillers else None
                return None

            def fill(budget_ns, op_ok=True):
                fill_count[0] += 1
                budget = budget_ns + fill_debt[0]
                spent = 0.0
                while fillers and spent < budget:
                    nxt = _pop_next(op_ok)
                    if nxt is None:
                        # only hoarded op (or unready z) left: stop without
                        # banking debt so qg2's first fill doesn't burst
                        fill_debt[0] = 0.0
                        return
                    _, ns, thunk = nxt
                    thunk()
                    spent += ns
                fill_debt[0] = budget - spent if fillers else 0.0

            def drain(key):
                """Force-emit queued fillers matching key (dependency
                barrier: attention on quarter qg needs all of QKV(tg=qg))."""
                rest = []
                for k, ns, thunk in fillers:
                    if k == key:
                        thunk()
                    else:
                        rest.append((k, ns, thunk))
                fillers[:] = rest

            def drain_class(cls):
                rest = []
                for k, ns, thunk in fillers:
                    if k[0] == cls:
                        thunk()
                    else:
                        rest.append((k, ns, thunk))
                fillers[:] = rest

            qkv_terms = ((0, 0), (0, 1), (1, 0))  # (x lo?, w lo?) per term

            def emit_qk_chain(tg, ct, j):
                tsl = slice(tg * 512, (tg + 1) * 512)
                dst, b_sb = ((QT, bq_sb), (KT, bk_sb))[j]
                ctj = ct * 2 + j
                ps = ps_b.tile([128, 512], f32, tag="b",
                               name=f"qk_{tg}_{ct}_{j}")
                for sub in range(2):
                    t0 = tg * 512 + sub * 256
                    i = 0
                    for xlo, wlo in qkv_terms:
                        xs = (xh, xl)[xlo]
                        ws = (wqkh, wqkl)[wlo]
                        for c in range(MO // 2):
                            nc.tensor.matmul(
                                ps[:, sub * 256 : sub * 256 + 256],
                                ws[:, ctj, 2 * c : 2 * c + 2, :],
                                xs[:, 2 * c : 2 * c + 2, t0 : t0 + 256],
                                start=(i == 0),
                                stop=(i == 11),
                                perf_mode=DR,
                            )
                            i += 1
                nc.vector.tensor_scalar_add(
                    dst[:, ct, tsl], ps[:], b_sb[:, ct : ct + 1]
                )

            def emit_v_chain(tg, ti):
                tt = tg * 4 + ti
                ps = ps_b.tile([128, 512], f32, tag="b", name=f"v_{tg}_{ti}")
                i = 0
                for xlo, wlo in qkv_terms:
                    xs = (xh, xl)[xlo]
                    ws = (wvh, wvl)[wlo]
                    for c in range(MO // 2):
                        nc.tensor.matmul(
                            ps[:, 0:CLOC],
                            xs[:, 2 * c : 2 * c + 2, tt * 128 : (tt + 1) * 128],
                            ws[:, 2 * c : 2 * c + 2, :],
                            start=(i == 0),
                            stop=(i == 11),
                            perf_mode=DR,
                        )
                        i += 1
                nc.vector.tensor_copy(
                    VA[:, tt, :, 0:64],
                    ps[:, 0:CLOC].rearrange("p (h d) -> p h d", d=64),
                )

            def emit_qkv(tg):
                for ct in range(2):
                    for j in range(2):
                        emit_qk_chain(tg, ct, j)
                for ti in range(4):
                    emit_v_chain(tg, ti)

            def push_qkv_fillers(tg):
                for ct in range(2):
                    for j in range(2):
                        fillers.append(
                            (("qkv", tg), 3072 * PE_NS,
                             lambda tg=tg, ct=ct, j=j: emit_qk_chain(tg, ct, j))
                        )
                vcls = "v" if defer_v else "qkv"
                for ti in range(4):
                    fillers.append(
                        ((vcls, tg), 1536 * PE_NS,
                         lambda tg=tg, ti=ti: emit_v_chain(tg, ti))
                    )

            # ---- attention ----
            def emit_sgrp(h, qg, gi, grp):
                """S^T matmuls for one exp group + the exp + diag masks."""
                hp = (h % 2) * 64
                ct = h // 2
                g0 = qg * 512
                cum = grp[-1][1] + grp[-1][2]
                sreg = ps_s.tile([128, sreg_w], f32, tag="s",
                                 name=f"s_{h}_{qg}_{gi}")
                for kt, off, w in grp:
                    q0 = g0 + 512 - w
                    c0 = off
                    while c0 < off + w:
                        cw = min(off + w - c0, 512 - c0 % 512)
                        nc.tensor.matmul(
                            sreg[:, c0 : c0 + cw],
                            KT[hp : hp + 64, ct, kt * 128 : (kt + 1) * 128],
                            QT[hp : hp + 64, ct,
                               q0 + c0 - off : q0 + c0 - off + cw],
                        )
                        c0 += cw
                pT = ppool.tile([128, sreg_w], bf16, tag="pT",
                                name=f"pT_{h}_{qg}_{gi}")
                no_diag = all(kt * 128 < g0 for kt, _, _ in grp)
                if gi < schr_groups[qg] and no_diag:
                    # Schraudolph exp on DVE: bf16 bits of exp(x*0.125) ~=
                    # int16(x*(0.125*128/ln2) + (127*128 - 5.6)); offloads
                    # the ACT engine (the attention-band pacer) at ~2% rms
                    # error on this group's P entries. sreg is 1024x (Q,K
                    # each carry the x32 weight prescale).
                    nc.vector.tensor_scalar(
                        pT[:, :cum].bitcast(mybir.dt.int16), sreg[:, :cum],
                        scalar1=23.0831253 / 1024.0, op0=mybir.AluOpType.mult,
                        scalar2=16250.4, op1=mybir.AluOpType.add)
                else:
                    nc.scalar.activation(pT[:, :cum], sreg[:, :cum], EXP,
                                         scale=0.125 / 1024.0)
                teng = nc.gpsimd if tri_engine == "gpsimd" else nc.vector
                for kt, off, w in grp:
                    if kt * 128 >= g0:  # diagonal block leads the span
                        teng.tensor_mul(
                            pT[:, off : off + 128],
                            pT[:, off : off + 128],
                            tri[:],
                        )
                return pT

            def emit_zchain(h, qg, qt, pts, kt2g, zp):
                """z[q,65] = sum_kt pT_chunk^T @ V_aug: one sequential PSUM
                accumulation chain per q-tile (a PSUM bank supports only one
                open accumulation group at a time)."""
                g0 = qg * 512
                qa = 4 * qg + qt
                for kt in range(qa + 1):
                    gi, off, w = kt2g[kt]
                    q0 = g0 + 512 - w
                    c0 = off + (g0 + qt * 128) - q0
                    nc.tensor.matmul(
                        zp[:, qt, 0:65],
                        pts[gi][:, c0 : c0 + 128],
                        VA[:, kt, h, 0:65],
                        start=(kt == 0),
                        stop=(kt == qa),
                    )

            def emit_norm(h, qg, zp, zq):
                """1/rowsum fused into the PSUM->SBUF copy of z."""
                hp = (h % 2) * 64
                rec = spool.tile([128, 4, 1], f32, tag="rec",
                                 name=f"rec_{h}_{qg}")
                nc.vector.reciprocal(rec[:], zp[:, :, 64:65])
                with nc.allow_low_precision(reason="attn out to bf16"):
                    for qt in range(4):
                        nc.vector.tensor_scalar_mul(
                            zq[:, qt, hp : hp + 64],
                            zp[:, qt, 0:64],
                            rec[:, qt, :],
                        )

            def emit_transpose(qg, pair, zq):
                """zq [q,128d] -> zT [128d, q]. All these transposes are
                latency-tolerant (op(qg) reads zT several us later), so they
                ride the DMA xbar instead of PE+DVE; only the tail-critical
                qg3/pair1 path (in the tailpipe) uses the PE transpose."""
                if dma_transpose:
                    for qt in range(4):
                        tt = qg * 4 + qt
                        nc.sync.dma_start_transpose(
                            zT[:, pair, tt * 128 : (tt + 1) * 128],
                            zq[:, qt, :],
                        )
                    return
                quad = ps_b.tile([128, 4, 128], bf16, tag="b",
                                 name=f"tq_{qg}_{pair}")
                for qt in range(4):
                    nc.tensor.transpose(quad[:, qt, :], zq[:, qt, :], ident)
                with nc.allow_low_precision(reason="zT copy"):
                    nc.vector.tensor_copy(
                        zT[:, pair, qg * 512 : (qg + 1) * 512],
                        quad[:].rearrange("p a b -> p (a b)"),
                    )

            def emit_op_half(qg, nh, ti, yA):
                tt = qg * 4 + ti
                ps = ps_b.tile([128, 512], f32, tag="b",
                               name=f"opA_{qg}_{nh}_{ti}")
                nc.tensor.matmul(
                    ps[:],
                    zT[:, 0, tt * 128 : (tt + 1) * 128],
                    wo[:, 0, nh * 512 : (nh + 1) * 512],
                )
                nc.vector.tensor_copy(yA[:, ti, :], ps[:])

            def emit_op_chunk(qg, nh, ti, ysb, dma_split, yA=None,
                              pool=None):
                tt = qg * 4 + ti
                # tail op chunks borrow ps_s (free once the last exp group
                # is consumed) so 4 chunks pipeline instead of 2
                ps = (pool or ps_b).tile([128, 512], f32,
                                         tag="b" if pool is None else "s",
                                         name=f"op_{qg}_{nh}_{ti}")
                cos = (1,) if yA is not None else (0, 1)
                for co in cos:
                    nc.tensor.matmul(
                        ps[:],
                        zT[:, co, tt * 128 : (tt + 1) * 128],
                        wo[:, co, nh * 512 : (nh + 1) * 512],
                        start=(co == cos[0]),
                        stop=(co == 1),
                    )
                with nc.allow_low_precision(reason="y partial to bf16"):
                    if yA is not None:
                        nc.vector.tensor_add(ysb[:, ti, :], ps[:],
                                             yA[:, ti, :])
                    elif qg == 3:
                        # tail: alternate engines so the copy stream (612ns
                        # each) does not pace the final 427ns-chunk unroll
                        (nc.scalar.copy if ti % 2 == 0
                         else nc.vector.tensor_copy)(ysb[:, ti, :], ps[:])
                    else:
                        nc.vector.tensor_copy(ysb[:, ti, :], ps[:])
                nper = 4 // dma_split
                if ti % nper == nper - 1:
                    t0 = tt - nper + 1
                    # y DMA queue: Pool/SWDGE keeps descriptor gen off the
                    # ACT/SP sequencers and HWDGE during the exp-critical
                    # band; in the qg3 tail (exp done) rotate across all
                    # three so gens parallelize
                    if qg == 3:
                        deng = (nc.gpsimd, nc.sync, nc.scalar)[
                            (ti // nper + nh * dma_split) % 3]
                    else:
                        deng = (nc.gpsimd, nc.sync)[(ti // nper + nh) % 2]
                    deng.dma_start(
                        y_d[t0 * 128 : (tt + 1) * 128,
                            nh * 512 : (nh + 1) * 512].rearrange(
                            "(ti p) n -> p ti n", p=128
                        ),
                        ysb[:, ti - nper + 1 : ti + 1, :],
                    )

            yA_box = {}

            def push_opA_fillers(qg):
                # first half (co=0) of qg3's out-proj: available right after
                # pair 0's transpose, staged to SBUF f32; the co=1 half plus
                # an add happens in the tail
                for nh in range(2):
                    yA = ypool.tile([128, 4, 512], f32, tag="yA", bufs=2,
                                    name=f"yA_{qg}_{nh}")
                    yA_box[(qg, nh)] = yA
                    for ti in range(4):
                        fillers.append(
                            (("op", qg), 512 * PE_NS,
                             lambda qg=qg, nh=nh, ti=ti, yA=yA:
                                 emit_op_half(qg, nh, ti, yA))
                        )

            def push_op_fillers(qg, split=False):
                dma_split = dma_splits[qg]
                for nh in range(2):
                    ysb = ypool.tile([128, 4, 512], bf16, tag="y",
                                     name=f"ysb_{qg}_{nh}")
                    yA = yA_box.get((qg, nh)) if split else None
                    for ti in range(4):
                        fillers.append(
                            (("op", qg), (512 if split else 1024) * PE_NS,
                             lambda qg=qg, nh=nh, ti=ti, ysb=ysb,
                                    ds=dma_split, yA=yA:
                                 emit_op_chunk(qg, nh, ti, ysb, ds, yA))
                        )

            def push_z_phase(qg, h, pts, kt2g, zq_box):
                """Queue head h's z chains + normalize (+ transpose) at the
                FRONT of the filler queue; they drain during head h+1's S
                phase (one-head software pipeline)."""
                box = {}
                # final unit (qg3 h3): per-q-tile norm -> transpose -> zT
                # copy -> that tile's out-proj chunks, so the tail pipelines
                # instead of serializing behind the full head
                tailpipe = qg == 3 and h == HLOC - 1

                def chain(qt):
                    if qt == 0:
                        box["zp"] = ps_z.tile([128, 4, 65], f32, tag="z",
                                              name=f"zp_{h}_{qg}")
                        if h % 2 == 0:
                            zq_box[h // 2] = zqpool.tile(
                                [128, 4, 128], bf16, tag="zq",
                                name=f"zq_{qg}_{h // 2}")
                    emit_zchain(h, qg, qt, pts, kt2g, box["zp"])
                    if tailpipe:
                        zq = zq_box[h // 2]
                        if qt == 0:
                            box["rec"] = spool.tile([128, 4, 1], f32,
                                                    tag="rec",
                                                    name=f"rec_{h}_{qg}")
                            box["ysb"] = [
                                ypool.tile([128, 4, 512], bf16, tag="y",
                                           name=f"ysb_3_{nh}")
                                for nh in range(2)
                            ]
                        rec = box["rec"]
                        zp = box["zp"]
                        nc.vector.reciprocal(rec[:, qt, :], zp[:, qt, 64:65])
                        with nc.allow_low_precision(reason="attn out bf16"):
                            nc.vector.tensor_scalar_mul(
                                zq[:, qt, 64:128], zp[:, qt, 0:64],
                                rec[:, qt, :])
                        quad = ps_b.tile([128, 128], bf16, tag="b",
                                         name=f"tq3_{qt}")
                        nc.tensor.transpose(quad[:], zq[:, qt, :], ident)
                        tt = qg * 4 + qt
                        with nc.allow_low_precision(reason="zT copy"):
                            nc.vector.tensor_copy(
                                zT[:, 1, tt * 128 : (tt + 1) * 128], quad[:])
                        for nh in range(2):
                            fillers.append(
                                (("op", 3), 1024 * PE_NS,
                                 lambda nh=nh, ti=qt:
                                     emit_op_chunk(3, nh, ti,
                                                   box["ysb"][nh],
                                                   dma_splits[3],
                                                   pool=ps_s))
                            )

                def norm():
                    if tailpipe:
                        return
                    emit_norm(h, qg, box["zp"], zq_box[h // 2])
                    if h % 2 == 1:
                        emit_transpose(qg, h // 2, zq_box[h // 2])
                        if h == 1 and qg == 3 and op3_split:
                            push_opA_fillers(qg)
                        if h == HLOC - 1:
                            # quarter finished: queue its out-proj (reads
                            # zT(qg), complete as of this point) and the
                            # next token group's QKV
                            push_op_fillers(qg, split=(qg == 3 and op3_split))
                            if qg + 2 <= 3:
                                push_qkv_fillers(qg + 2)

                # z thunks of heads <= h-2 must fully precede this head's
                # (ps_z has 2 bufs, so h-1's z may still be queued); h's
                # thunks append after h-1's so class-FIFO order holds
                prev = ("z", qg, h - 1)
                rest = []
                for k, ns, thunk in fillers:
                    if k[0] == "z" and k != prev:
                        thunk()
                    else:
                        rest.append((k, ns, thunk))
                fillers[:] = rest
                thunks = []
                for qt in range(4):
                    ncols = (4 * qg + qt + 1) * 65
                    thunks.append(
                        (("z", qg, h), ncols * PE_NS,
                         lambda qt=qt: chain(qt))
                    )
                thunks.append((("z", qg, h), 0.0, norm))
                idx = 0
                for i, (k, _, _) in enumerate(fillers):
                    if k[0] == "z":
                        idx = i + 1
                fillers[idx:idx] = thunks
                z_pushed_at[(qg, h)] = fill_count[0]

            # ---- program ----
            # ct0's Q,K chains (heads 0,1) emit directly so quarter 0's S/exp
            # stream starts as soon as possible; ct1 + V chains become
            # fillers drained during h0/h1's exp (barriers: v at h==1,
            # qkv(ct1) at h==2).
            for j in range(2):
                emit_qk_chain(0, 0, j)
            for jj in range(2):
                fillers.append(
                    (("qkv", 0), 3072 * PE_NS,
                     lambda jj=jj: emit_qk_chain(0, 1, jj))
                )
            for ti in range(4):
                fillers.append(
                    (("v", 0), 1536 * PE_NS,
                     lambda ti=ti: emit_v_chain(0, ti))
                )
            push_qkv_fillers(1)

            zq_box = {}
            for qg in range(4):
                groups = _groups(qg, sreg_w)
                kt2g = {}
                for gi, grp in enumerate(groups):
                    for kt, off, w in grp:
                        kt2g[kt] = (gi, off, w)
                if qg > 0:
                    # barrier: this quarter's S/z read QT/KT/VA of tg=qg
                    drain(("qkv", qg))
                for h in range(HLOC):
                    if h == 1:
                        drain(("v", qg))
                    if h == 2 and qg == 0:
                        drain(("qkv", 0))  # ct1 chains gate heads 2,3
                    pts = []
                    # emit S groups in pairs (back-to-back on PE) so ACT's
                    # exp stream has at most one bubble per pair, not per
                    # group; the 2-buffer S rotation permits exactly 2 ahead
                    qsc, qpad = ((fill_scale, fill_pad) if fill_cfg is None
                                 else fill_cfg[qg])
                    pqg = pair_s if isinstance(pair_s, bool) else pair_s[qg]
                    step = 2 if pqg else 1
                    for g0i in range(0, len(groups), step):
                        pair = groups[g0i : g0i + step]
                        budget = 0.0
                        for gi, grp in zip(range(g0i, g0i + step), pair):
                            pts.append(emit_sgrp(h, qg, gi, grp))
                            cum = grp[-1][1] + grp[-1][2]
                            budget += ((cum * ACT_NS + 185.0) * qsc
                                       + qpad - cum * PE_NS)
                        fill(max(0.0, budget), op_ok=(qg >= op_from))
                    push_z_phase(qg, h, pts, kt2g, zq_box)

            # drain the tail (queue can grow while draining)
            while fillers:
                nxt = _pop_next(True)
                _, _, thunk = nxt if nxt else fillers.pop(0)
                thunk()

    nc.compile()
    return nc


def _pack_w(w):
    # [DM, C] -> [128, MO, C] f32: partition p holds rows {mo*128 + p}
    return np.ascontiguousarray(
        w.reshape(MO, 128, w.shape[1]).transpose(1, 0, 2)
    ).astype(np.float32)


def _split8(a):
    # f32 array -> (hi, lo) e4m3 pair with hi + lo ~= a to ~0.1%
    ah = a.astype(ml_dtypes.float8_e4m3)
    al = (a - ah.astype(np.float32)).astype(ml_dtypes.float8_e4m3)
    return np.ascontiguousarray(ah), np.ascontiguousarray(al)


def make_in_maps(x, w_qkv, b_qkv, w_out):
    # multiplicative post-exp mask: 1 where k <= q (upper incl diag), else 0
    tri = np.tri(128, 128, 0, dtype=np.float32).T.astype(ml_dtypes.bfloat16)
    ident = np.eye(128, dtype=np.float32).astype(ml_dtypes.bfloat16)
    in_maps = []
    for core in range(8):
        b = core // 4
        hg = core % 4
        c0 = hg * CLOC
        csl = slice(c0, c0 + CLOC)

        # packed consts: [128, 264] bf16-typed raw columns. Biases carry the
        # x32 weight prescale (Q,K live at 32x on device).
        cst = np.zeros((128, 264), np.uint16)
        cst[:, 0:128] = tri.view(np.uint16)
        cst[:, 128:256] = ident.view(np.uint16)
        bq = np.ascontiguousarray(
            32.0 * b_qkv[csl].astype(np.float32).reshape(2, 128).T
        )
        bk = np.ascontiguousarray(
            32.0 * b_qkv[DM + c0 : DM + c0 + CLOC].astype(np.float32)
            .reshape(2, 128).T
        )
        cst[:, 256:260] = bq.view(np.uint16).reshape(128, 4)
        cst[:, 260:264] = bk.view(np.uint16).reshape(128, 4)

        wq_p = _pack_w(32.0 * w_qkv[:, csl])
        wk_p = _pack_w(32.0 * w_qkv[:, DM + c0 : DM + c0 + CLOC])
        # [128, ctj, MO, 128]: ctj = ct*2 + j (j=0 -> Q, j=1 -> K)
        wqk = np.concatenate(
            [wq_p[:, None, :, 0:128], wk_p[:, None, :, 0:128],
             wq_p[:, None, :, 128:256], wk_p[:, None, :, 128:256]],
            axis=1,
        )
        wqkh, wqkl = _split8(wqk)
        wvh, wvl = _split8(
            _pack_w(32.0 * w_qkv[:, 2 * DM + c0 : 2 * DM + c0 + CLOC]))
        xTh, xTl = _split8(_pack_w(np.ascontiguousarray(x[b].T)))
        in_maps.append(
            {
                "xTh": xTh,
                "xTl": xTl,
                "wqkh": wqkh,
                "wqkl": wqkl,
                "wvh": wvh,
                "wvl": wvl,
                # wo: [CLOC, DM] -> [128, 2, DM]
                "wo": np.ascontiguousarray(
                    w_out[csl, :].reshape(2, 128, DM).transpose(1, 0, 2)
                ).astype(ml_dtypes.bfloat16),
                "cst": cst.view(ml_dtypes.bfloat16),
            }
        )
    return in_maps


def gather(results, b_qkv, w_out, b_out):
    # device skips the V bias; z_norm + b_v projects to a constant row:
    # y += b_v @ w_out, folded into the output bias here
    b_eff = (
        b_out.astype(np.float32)
        + b_qkv[2 * DM :].astype(np.float32) @ w_out.astype(np.float32)
    )
    out = np.empty((B, S, DM), np.float32)
    for b in range(B):
        acc = results[4 * b]["y"].astype(np.float32)
        for j in range(1, 4):
            acc = acc + results[4 * b + j]["y"]
        out[b] = acc + b_eff[None, :]
    return out


def kernel(x, w_qkv, b_qkv, w_out, b_out):
    x = np.asarray(x)
    w_qkv = np.asarray(w_qkv)
    b_qkv = np.asarray(b_qkv)
    w_out = np.asarray(w_out)
    b_out = np.asarray(b_out)

    if "nc" not in _CACHE:
        _CACHE["nc"] = build()
    nc = _CACHE["nc"]

    in_maps = make_in_maps(x, w_qkv, b_qkv, w_out)
    res = run_bass_kernel_spmd(nc, in_maps, core_ids=list(range(8)))
    return gather(res.results, b_qkv, w_out, b_out)



# revision 49
# speedup vs baseline: 1.1437x; 1.0013x over previous
"""Causal multi-head attention block (B=2, S=2048, D=1024, H=16) on 8 TRN2 cores.

Sharding: core i handles batch b = i//4 and head group hg = i%4 (4 heads =
256 model dims). Each core computes its heads' attention and a partial
output projection; the host sums the 4 partials per batch and adds b_out.

Per-core device pipeline (fp32 PSUM accumulation):
  1. QKV in compensated fp8: x and the QKV weights are hi/lo e4m3 pairs
     (hosts pre-scales weights x32 so e4m3's mantissa range is used;
     hi+lo carries ~11 mantissa bits, more precise than bf16). Each
     chain is xh@wh + xh@wl + xl@wh via DoubleRow matmuls (contraction
     256/matmul at 0.5 cycles/col = 3x bf16 throughput). Q^T,K^T land
     as [head_cols, tokens] at 32x scale (absorbed by the exp scale
     2^-13); V lands as [tokens, head_cols] at 32x, stored augmented
     with a 32.0 column so the z-matmul's row sums carry the same scale
     and the normalization cancels it exactly.
  2. Attention per head in bf16 (fp8 here fails the 2e-2 tolerance),
     flash-style in the S^T = K.Q^T orientation over the causal lower
     triangle only: S^T[k_tile, q_span] -> exp on ScalarE
     (scale=2^-13, no max subtraction; logits ~N(0,1)) -> P^T bf16 ->
     multiplicative 0/1 mask on diagonal blocks -> z[q_tile, 65]
     += P^T_chunk^T @ V_aug accumulated over k tiles in PSUM. The [q, d+1]
     z orientation makes each z matmul only 65 PE columns (vs a full
     q-span) and puts the softmax row sum in PSUM column 64 of the same
     partition as its query, so normalization is a per-partition
     tensor_scalar multiply fused into the PSUM->SBUF copy.
  3. z[q,d] tiles are transposed back to z^T[d,q] via PE transpose
     (identity matmul, 128 cols per 2-head tile) for the out-projection.
  4. Out-proj: y_partial[t, n] accumulated over the 256 local dims.

Program order is a fine-grained software pipeline: the attention loop is
a flat sequence over (q-quarter, head, k-group) with the z matmuls
lagging one group behind the S matmuls, and a filler queue (next token
group's QKV chains, previous quarters' out-proj chunks) drained between
S and z so the PE never waits on ScalarE exp. The final (ACT-bound)
quarter's last head runs a per-q-tile norm -> transpose -> out-proj
pipeline so the tail doesn't serialize, and y DMAs ride the Pool
engine's SWDGE path (+SP/ACT in the tail) to keep descriptor
generation off the exp-critical sequencers. Host pre-packs all inputs
into SBUF layouts; the V bias is folded into the output bias on the
host (b_v @ w_out).
"""

import numpy as np
import ml_dtypes

import concourse.mybir as mybir
import concourse.tile as tile
from concourse import bacc
from concourse.bass_utils import run_bass_kernel_spmd

B = 2
S = 2048
DM = 1024
HD = 64
HLOC = 4                 # heads per core
CLOC = HLOC * HD         # local model dims (256)
MO = DM // 128           # 8 k-subtiles of the model dim
NKT = S // 128           # 16 key tiles

f32 = mybir.dt.float32
bf16 = mybir.dt.bfloat16
f8 = mybir.dt.float8e4
DR = mybir.MatmulPerfMode.DoubleRow
EXP = mybir.ActivationFunctionType.Exp

ACT_NS = 0.8333333333333334
PE_NS = 0.4166666666666667

_CACHE = {}


def _groups(qg, cap=1024):
    """Pack the causal k-tile spans of query quarter qg into exp groups of
    <= cap columns. Returns list of groups; each group is a list of
    (kt, offset_in_group, width)."""
    g0 = qg * 512
    last_kt = 4 * qg + 3
    groups, cur, cum = [], [], 0
    for kt in range(last_kt + 1):
        w = g0 + 512 - max(kt * 128, g0)
        if cum + w > cap:
            groups.append(cur)
            cur, cum = [], 0
        cur.append((kt, cum, w))
        cum += w
    groups.append(cur)
    return groups


def build(pt_bufs=20, zq_bufs=4, y_bufs=4, sreg_w=1024, fill_scale=1.0,
          fill_pad=0.0, dma_splits=(2, 2, 4, 4), tri_engine="dve", op_from=2,
          zp_bufs=2, dma_transpose=True, warmup=8,
          z_delay=1, defer_v=True, copy_split=False, rr_pop=False,
          pair_s=(0, 0, 0, 1), op3_split=False, schr_groups=(0, 0, 0, 0),
          fill_cfg=((1.025, 53.0), (0.995, 56.0), (1.014, 5.0), (0.974, -100.0))):
    nc = bacc.Bacc("TRN2", target_bir_lowering=False, debug=False)

    # x and the QKV weights are fp8 e4m3 hi/lo pairs (weights pre-scaled x32
    # on the host so e4m3's mantissa is used; the x32 scales cancel in the
    # exp scale and the x32 ones-column). Q = xh@wh + xh@wl + xl@wh via
    # DoubleRow matmuls: contraction 256/matmul at 0.5 cycles/col.
    xh_d = nc.dram_tensor("xTh", [128, MO, S], f8, kind="ExternalInput")
    xl_d = nc.dram_tensor("xTl", [128, MO, S], f8, kind="ExternalInput")
    # wqk grouped per QKV chain (ctj = ct*2+j) so each chain's weights are
    # one contiguous DMA
    wqkh_d = nc.dram_tensor("wqkh", [128, 4, MO, 128], f8, kind="ExternalInput")
    wqkl_d = nc.dram_tensor("wqkl", [128, 4, MO, 128], f8, kind="ExternalInput")
    wvh_d = nc.dram_tensor("wvh", [128, MO, CLOC], f8, kind="ExternalInput")
    wvl_d = nc.dram_tensor("wvl", [128, MO, CLOC], f8, kind="ExternalInput")
    wo_d = nc.dram_tensor("wo", [128, 2, DM], bf16, kind="ExternalInput")
    # consts packed as raw bf16 columns: tri[0:128], identity[128:256],
    # bq[256:260], bk[260:264] (f32 values bit-split across bf16 pairs)
    cst_d = nc.dram_tensor("cst", [128, 264], bf16, kind="ExternalInput")
    y_d = nc.dram_tensor("y", [S, DM], bf16, kind="ExternalOutput")

    with tile.TileContext(nc) as tc:
        with (
            tc.tile_pool(name="consts", bufs=1) as consts,
            tc.tile_pool(name="acts", bufs=1) as apool,
            tc.tile_pool(name="pt", bufs=pt_bufs) as ppool,
            tc.tile_pool(name="zq", bufs=zq_bufs) as zqpool,
            tc.tile_pool(name="norm", bufs=4) as spool,
            tc.tile_pool(name="ycopy", bufs=y_bufs) as ypool,
            # 8 PSUM banks: ps_s 2x[128,1024]=4 (QK logits), ps_z
            # 2x[128,4,65]=2 (z accumulators), ps_b 2x[128,512]=2
            # (QKV / V / out-proj chains and z transposes)
            tc.tile_pool(name="ps_s", bufs=2, space="PSUM") as ps_s,
            tc.tile_pool(name="ps_z", bufs=zp_bufs, space="PSUM") as ps_z,
            tc.tile_pool(name="ps_b", bufs=2, space="PSUM") as ps_b,
        ):
            csb = consts.tile([128, 264], bf16)
            wqkh = consts.tile([128, 4, MO, 128], f8)
            wqkl = consts.tile([128, 4, MO, 128], f8)
            wvh = consts.tile([128, MO, CLOC], f8)
            wvl = consts.tile([128, MO, CLOC], f8)
            wo = consts.tile([128, 2, DM], bf16)
            xh = apool.tile([128, MO, S], f8)
            xl = apool.tile([128, MO, S], f8)

            # DMA order = consumption order (transfers serialize on the DMA
            # engines). Startup fans out over three queues (SP: weights,
            # ACT: xh, Pool/SWDGE: xl) so the first QK chain can start ~2.6us in;
            # bulk transfers stay off the ACT queue once the exp stream
            # starts (each dma_start occupies the sequencer ~660ns).
            nc.sync.dma_start(wqkh[:, 0, :, :], wqkh_d[:, 0, :, :])
            nc.scalar.dma_start(xh[:, 0:2, 0:512], xh_d[:, 0:2, 0:512])
            nc.gpsimd.dma_start(xl[:, 0:2, 0:512], xl_d[:, 0:2, 0:512])
            nc.sync.dma_start(wqkl[:, 0, :, :], wqkl_d[:, 0, :, :])
            nc.scalar.dma_start(xh[:, 2:4, 0:512], xh_d[:, 2:4, 0:512])
            nc.gpsimd.dma_start(xl[:, 2:4, 0:512], xl_d[:, 2:4, 0:512])
            nc.sync.dma_start(csb[:], cst_d[:])
            nc.scalar.dma_start(xh[:, 4:8, 0:512], xh_d[:, 4:8, 0:512])
            nc.gpsimd.dma_start(xl[:, 4:8, 0:512], xl_d[:, 4:8, 0:512])
            nc.sync.dma_start(wqkh[:, 1, :, :], wqkh_d[:, 1, :, :])
            nc.sync.dma_start(wqkl[:, 1, :, :], wqkl_d[:, 1, :, :])
            nc.sync.dma_start(wqkh[:, 2:4, :, :], wqkh_d[:, 2:4, :, :])
            nc.sync.dma_start(wqkl[:, 2:4, :, :], wqkl_d[:, 2:4, :, :])
            nc.sync.dma_start(wvh[:], wvh_d[:])
            nc.sync.dma_start(wvl[:], wvl_d[:])
            nc.sync.dma_start(xh[:, :, 512:1024], xh_d[:, :, 512:1024])
            nc.sync.dma_start(xl[:, :, 512:1024], xl_d[:, :, 512:1024])
            nc.sync.dma_start(xh[:, :, 1024:1536], xh_d[:, :, 1024:1536])
            nc.sync.dma_start(xl[:, :, 1024:1536], xl_d[:, :, 1024:1536])
            nc.sync.dma_start(xh[:, :, 1536:2048], xh_d[:, :, 1536:2048])
            nc.sync.dma_start(xl[:, :, 1536:2048], xl_d[:, :, 1536:2048])
            nc.sync.dma_start(wo[:], wo_d[:])

            tri = csb[:, 0:128]
            ident = csb[:, 128:256]
            bq_sb = csb[:, 256:260].bitcast(f32)
            bk_sb = csb[:, 260:264].bitcast(f32)

            QT = apool.tile([128, 2, S], bf16)
            KT = apool.tile([128, 2, S], bf16)
            # V augmented: [t-part, kt, h, 0:64] = v dims (x32), col 64 = 32
            # so the rowsum scale matches the v columns and the x32 cancels
            # in the normalization
            VA = apool.tile([128, NKT, HLOC, 72], bf16)
            nc.vector.memset(VA[:, :, :, 64:65], 32.0)
            zT = apool.tile([128, 2, S], bf16)

            # ---- filler queue: PE work units drained while ScalarE exps ----
            fillers = []          # list of (key, pe_ns, thunk)
            fill_debt = [0.0]
            fill_count = [0]      # fill() invocations, for z-pop delay
            z_pushed_at = {}      # (qg, h) -> fill_count at push
            rr_state = [0]

            def _z_ok(k):
                return (fill_count[0] - z_pushed_at.get((k[1], k[2]), -(1 << 30))
                        >= z_delay)

            cur_qg = [0]

            def _pop_next(op_ok=True):
                """z thunks once ScalarE has had time to produce their exp
                inputs (z_delay fill periods after push), then QKV (needed
                by the next quarter anyway), out-proj last. op thunks are
                hoarded for the late ACT-bound quarters (qg >= op_from),
                where they are the only filler class left."""
                for cls in ("z", "v", "qkv"):
                    for i, (k, ns, thunk) in enumerate(fillers):
                        if k[0] == cls and (cls != "z" or _z_ok(k)):
                            return fillers.pop(i)
                if op_ok:
                    for i, (k, ns, thunk) in enumerate(fillers):
                        if k[0] == "op":
                            return fillers.pop(i)
                    return fillers.pop(0) if fillers else None
                return None

            def fill(budget_ns, op_ok=True):
                fill_count[0] += 1
                budget = budget_ns + fill_debt[0]
                spent = 0.0
                while fillers and spent < budget:
                    nxt = _pop_next(op_ok)
                    if nxt is None:
                        # only hoarded op (or unready z) left: stop without
                        # banking debt so qg2's first fill doesn't burst
                        fill_debt[0] = 0.0
                        return
                    _, ns, thunk = nxt
                    thunk()
                    spent += ns
                fill_debt[0] = budget - spent if fillers else 0.0

            def drain(key):
                """Force-emit queued fillers matching key (dependency
                barrier: attention on quarter qg needs all of QKV(tg=qg))."""
                rest = []
                for k, ns, thunk in fillers:
                    if k == key:
                        thunk()
                    else:
                        rest.append((k, ns, thunk))
                fillers[:] = rest

            def drain_class(cls):
                rest = []
                for k, ns, thunk in fillers:
                    if k[0] == cls:
                        thunk()
                    else:
                        rest.append((k, ns, thunk))
                fillers[:] = rest

            qkv_terms = ((0, 0), (0, 1), (1, 0))  # (x lo?, w lo?) per term

            def emit_qk_chain(tg, ct, j):
                tsl = slice(tg * 512, (tg + 1) * 512)
                dst, b_sb = ((QT, bq_sb), (KT, bk_sb))[j]
                ctj = ct * 2 + j
                ps = ps_b.tile([128, 512], f32, tag="b",
                               name=f"qk_{tg}_{ct}_{j}")
                for sub in range(2):
                    t0 = tg * 512 + sub * 256
                    i = 0
                    for xlo, wlo in qkv_terms:
                        xs = (xh, xl)[xlo]
                        ws = (wqkh, wqkl)[wlo]
                        for c in range(MO // 2):
                            nc.tensor.matmul(
                                ps[:, sub * 256 : sub * 256 + 256],
                                ws[:, ctj, 2 * c : 2 * c + 2, :],
                                xs[:, 2 * c : 2 * c + 2, t0 : t0 + 256],
                                start=(i == 0),
                                stop=(i == 11),
                                perf_mode=DR,
                            )
                            i += 1
                nc.vector.tensor_scalar_add(
                    dst[:, ct, tsl], ps[:], b_sb[:, ct : ct + 1]
                )

            def emit_v_chain(tg, ti):
                tt = tg * 4 + ti
                ps = ps_b.tile([128, 512], f32, tag="b", name=f"v_{tg}_{ti}")
                i = 0
                for xlo, wlo in qkv_terms:
                    xs = (xh, xl)[xlo]
                    ws = (wvh, wvl)[wlo]
                    for c in range(MO // 2):
                        nc.tensor.matmul(
                            ps[:, 0:CLOC],
                            xs[:, 2 * c : 2 * c + 2, tt * 128 : (tt + 1) * 128],
                            ws[:, 2 * c : 2 * c + 2, :],
                            start=(i == 0),
                            stop=(i == 11),
                            perf_mode=DR,
                        )
                        i += 1
                nc.vector.tensor_copy(
                    VA[:, tt, :, 0:64],
                    ps[:, 0:CLOC].rearrange("p (h d) -> p h d", d=64),
                )

            def emit_qkv(tg):
                for ct in range(2):
                    for j in range(2):
                        emit_qk_chain(tg, ct, j)
                for ti in range(4):
                    emit_v_chain(tg, ti)

            def push_qkv_fillers(tg):
                for ct in range(2):
                    for j in range(2):
                        fillers.append(
                            (("qkv", tg), 3072 * PE_NS,
                             lambda tg=tg, ct=ct, j=j: emit_qk_chain(tg, ct, j))
                        )
                vcls = "v" if defer_v else "qkv"
                for ti in range(4):
                    fillers.append(
                        ((vcls, tg), 1536 * PE_NS,
                         lambda tg=tg, ti=ti: emit_v_chain(tg, ti))
                    )

            # ---- attention ----
            def emit_sgrp(h, qg, gi, grp):
                """S^T matmuls for one exp group + the exp + diag masks."""
                hp = (h % 2) * 64
                ct = h // 2
                g0 = qg * 512
                cum = grp[-1][1] + grp[-1][2]
                sreg = ps_s.tile([128, sreg_w], f32, tag="s",
                                 name=f"s_{h}_{qg}_{gi}")
                for kt, off, w in grp:
                    q0 = g0 + 512 - w
                    c0 = off
                    while c0 < off + w:
                        cw = min(off + w - c0, 512 - c0 % 512)
                        nc.tensor.matmul(
                            sreg[:, c0 : c0 + cw],
                            KT[hp : hp + 64, ct, kt * 128 : (kt + 1) * 128],
                            QT[hp : hp + 64, ct,
                               q0 + c0 - off : q0 + c0 - off + cw],
                        )
                        c0 += cw
                pT = ppool.tile([128, sreg_w], bf16, tag="pT",
                                name=f"pT_{h}_{qg}_{gi}")
                no_diag = all(kt * 128 < g0 for kt, _, _ in grp)
                if gi < schr_groups[qg] and no_diag:
                    # Schraudolph exp on DVE: bf16 bits of exp(x*0.125) ~=
                    # int16(x*(0.125*128/ln2) + (127*128 - 5.6)); offloads
                    # the ACT engine (the attention-band pacer) at ~2% rms
                    # error on this group's P entries. sreg is 1024x (Q,K
                    # each carry the x32 weight prescale).
                    nc.vector.tensor_scalar(
                        pT[:, :cum].bitcast(mybir.dt.int16), sreg[:, :cum],
                        scalar1=23.0831253 / 1024.0, op0=mybir.AluOpType.mult,
                        scalar2=16250.4, op1=mybir.AluOpType.add)
                else:
                    nc.scalar.activation(pT[:, :cum], sreg[:, :cum], EXP,
                                         scale=0.125 / 1024.0)
                teng = nc.gpsimd if tri_engine == "gpsimd" else nc.vector
                for kt, off, w in grp:
                    if kt * 128 >= g0:  # diagonal block leads the span
                        teng.tensor_mul(
                            pT[:, off : off + 128],
                            pT[:, off : off + 128],
                            tri[:],
                        )
                return pT

            def emit_zchain(h, qg, qt, pts, kt2g, zp):
                """z[q,65] = sum_kt pT_chunk^T @ V_aug: one sequential PSUM
                accumulation chain per q-tile (a PSUM bank supports only one
                open accumulation group at a time)."""
                g0 = qg * 512
                qa = 4 * qg + qt
                for kt in range(qa + 1):
                    gi, off, w = kt2g[kt]
                    q0 = g0 + 512 - w
                    c0 = off + (g0 + qt * 128) - q0
                    nc.tensor.matmul(
                        zp[:, qt, 0:65],
                        pts[gi][:, c0 : c0 + 128],
                        VA[:, kt, h, 0:65],
                        start=(kt == 0),
                        stop=(kt == qa),
                    )

            def emit_norm(h, qg, zp, zq):
                """1/rowsum fused into the PSUM->SBUF copy of z."""
                hp = (h % 2) * 64
                rec = spool.tile([128, 4, 1], f32, tag="rec",
                                 name=f"rec_{h}_{qg}")
                nc.vector.reciprocal(rec[:], zp[:, :, 64:65])
                with nc.allow_low_precision(reason="attn out to bf16"):
                    for qt in range(4):
                        nc.vector.tensor_scalar_mul(
                            zq[:, qt, hp : hp + 64],
                            zp[:, qt, 0:64],
                            rec[:, qt, :],
                        )

            def emit_transpose(qg, pair, zq):
                """zq [q,128d] -> zT [128d, q]. All these transposes are
                latency-tolerant (op(qg) reads zT several us later), so they
                ride the DMA xbar instead of PE+DVE; only the tail-critical
                qg3/pair1 path (in the tailpipe) uses the PE transpose."""
                if dma_transpose:
                    for qt in range(4):
                        tt = qg * 4 + qt
                        nc.sync.dma_start_transpose(
                            zT[:, pair, tt * 128 : (tt + 1) * 128],
                            zq[:, qt, :],
                        )
                    return
                quad = ps_b.tile([128, 4, 128], bf16, tag="b",
                                 name=f"tq_{qg}_{pair}")
                for qt in range(4):
                    nc.tensor.transpose(quad[:, qt, :], zq[:, qt, :], ident)
                with nc.allow_low_precision(reason="zT copy"):
                    nc.vector.tensor_copy(
                        zT[:, pair, qg * 512 : (qg + 1) * 512],
                        quad[:].rearrange("p a b -> p (a b)"),
                    )

            def emit_op_half(qg, nh, ti, yA):
                tt = qg * 4 + ti
                ps = ps_b.tile([128, 512], f32, tag="b",
                               name=f"opA_{qg}_{nh}_{ti}")
                nc.tensor.matmul(
                    ps[:],
                    zT[:, 0, tt * 128 : (tt + 1) * 128],
                    wo[:, 0, nh * 512 : (nh + 1) * 512],
                )
                nc.vector.tensor_copy(yA[:, ti, :], ps[:])

            def emit_op_chunk(qg, nh, ti, ysb, dma_split, yA=None,
                              pool=None):
                tt = qg * 4 + ti
                # tail op chunks borrow ps_s (free once the last exp group
                # is consumed) so 4 chunks pipeline instead of 2
                ps = (pool or ps_b).tile([128, 512], f32,
                                         tag="b" if pool is None else "s",
                                         name=f"op_{qg}_{nh}_{ti}")
                cos = (1,) if yA is not None else (0, 1)
                for co in cos:
                    nc.tensor.matmul(
                        ps[:],
                        zT[:, co, tt * 128 : (tt + 1) * 128],
                        wo[:, co, nh * 512 : (nh + 1) * 512],
                        start=(co == cos[0]),
                        stop=(co == 1),
                    )
                with nc.allow_low_precision(reason="y partial to bf16"):
                    if yA is not None:
                        nc.vector.tensor_add(ysb[:, ti, :], ps[:],
                                             yA[:, ti, :])
                    elif qg == 3:
                        # tail: alternate engines so the copy stream (612ns
                        # each) does not pace the final 427ns-chunk unroll
                        (nc.scalar.copy if ti % 2 == 0
                         else nc.vector.tensor_copy)(ysb[:, ti, :], ps[:])
                    else:
                        nc.vector.tensor_copy(ysb[:, ti, :], ps[:])
                nper = 4 // dma_split
                if ti % nper == nper - 1:
                    t0 = tt - nper + 1
                    # y DMA queue: Pool/SWDGE keeps descriptor gen off the
                    # ACT/SP sequencers and HWDGE during the exp-critical
                    # band; in the qg3 tail (exp done) rotate across all
                    # three so gens parallelize
                    if qg == 3:
                        deng = (nc.gpsimd, nc.sync, nc.scalar)[
                            (ti // nper + nh * dma_split) % 3]
                    else:
                        deng = (nc.gpsimd, nc.sync)[(ti // nper + nh) % 2]
                    deng.dma_start(
                        y_d[t0 * 128 : (tt + 1) * 128,
                            nh * 512 : (nh + 1) * 512].rearrange(
                            "(ti p) n -> p ti n", p=128
                        ),
                        ysb[:, ti - nper + 1 : ti + 1, :],
                    )

            yA_box = {}

            def push_opA_fillers(qg):
                # first half (co=0) of qg3's out-proj: available right after
                # pair 0's transpose, staged to SBUF f32; the co=1 half plus
                # an add happens in the tail
                for nh in range(2):
                    yA = ypool.tile([128, 4, 512], f32, tag="yA", bufs=2,
                                    name=f"yA_{qg}_{nh}")
                    yA_box[(qg, nh)] = yA
                    for ti in range(4):
                        fillers.append(
                            (("op", qg), 512 * PE_NS,
                             lambda qg=qg, nh=nh, ti=ti, yA=yA:
                                 emit_op_half(qg, nh, ti, yA))
                        )

            def push_op_fillers(qg, split=False):
                dma_split = dma_splits[qg]
                for nh in range(2):
                    ysb = ypool.tile([128, 4, 512], bf16, tag="y",
                                     name=f"ysb_{qg}_{nh}")
                    yA = yA_box.get((qg, nh)) if split else None
                    for ti in range(4):
                        fillers.append(
                            (("op", qg), (512 if split else 1024) * PE_NS,
                             lambda qg=qg, nh=nh, ti=ti, ysb=ysb,
                                    ds=dma_split, yA=yA:
                                 emit_op_chunk(qg, nh, ti, ysb, ds, yA))
                        )

            def push_z_phase(qg, h, pts, kt2g, zq_box):
                """Queue head h's z chains + normalize (+ transpose) at the
                FRONT of the filler queue; they drain during head h+1's S
                phase (one-head software pipeline)."""
                box = {}
                # final unit (qg3 h3): per-q-tile norm -> transpose -> zT
                # copy -> that tile's out-proj chunks, so the tail pipelines
                # instead of serializing behind the full head
                tailpipe = qg == 3 and h == HLOC - 1

                def chain(qt):
                    if qt == 0:
                        box["zp"] = ps_z.tile([128, 4, 65], f32, tag="z",
                                              name=f"zp_{h}_{qg}")
                        if h % 2 == 0:
                            zq_box[h // 2] = zqpool.tile(
                                [128, 4, 128], bf16, tag="zq",
                                name=f"zq_{qg}_{h // 2}")
                    emit_zchain(h, qg, qt, pts, kt2g, box["zp"])
                    if tailpipe:
                        zq = zq_box[h // 2]
                        if qt == 0:
                            box["rec"] = spool.tile([128, 4, 1], f32,
                                                    tag="rec",
                                                    name=f"rec_{h}_{qg}")
                            box["ysb"] = [
                                ypool.tile([128, 4, 512], bf16, tag="y",
                                           name=f"ysb_3_{nh}")
                                for nh in range(2)
                            ]
                        rec = box["rec"]
                        zp = box["zp"]
                        nc.vector.reciprocal(rec[:, qt, :], zp[:, qt, 64:65])
                        with nc.allow_low_precision(reason="attn out bf16"):
                            nc.vector.tensor_scalar_mul(
                                zq[:, qt, 64:128], zp[:, qt, 0:64],
                                rec[:, qt, :])
                        quad = ps_b.tile([128, 128], bf16, tag="b",
                                         name=f"tq3_{qt}")
                        nc.tensor.transpose(quad[:], zq[:, qt, :], ident)
                        tt = qg * 4 + qt
                        with nc.allow_low_precision(reason="zT copy"):
                            nc.vector.tensor_copy(
                                zT[:, 1, tt * 128 : (tt + 1) * 128], quad[:])
                        for nh in range(2):
                            fillers.append(
                                (("op", 3), 1024 * PE_NS,
                                 lambda nh=nh, ti=qt:
                                     emit_op_chunk(3, nh, ti,
                                                   box["ysb"][nh],
                                                   dma_splits[3],
                                                   pool=ps_s))
                            )

                def norm():
                    if tailpipe:
                        return
                    emit_norm(h, qg, box["zp"], zq_box[h // 2])
                    if h % 2 == 1:
                        emit_transpose(qg, h // 2, zq_box[h // 2])
                        if h == 1 and qg == 3 and op3_split:
                            push_opA_fillers(qg)
                        if h == HLOC - 1:
                            # quarter finished: queue its out-proj (reads
                            # zT(qg), complete as of this point) and the
                            # next token group's QKV
                            push_op_fillers(qg, split=(qg == 3 and op3_split))
                            if qg + 2 <= 3:
                                push_qkv_fillers(qg + 2)

                # z thunks of heads <= h-2 must fully precede this head's
                # (ps_z has 2 bufs, so h-1's z may still be queued); h's
                # thunks append after h-1's so class-FIFO order holds
                prev = ("z", qg, h - 1)
                rest = []
                for k, ns, thunk in fillers:
                    if k[0] == "z" and k != prev:
                        thunk()
                    else:
                        rest.append((k, ns, thunk))
                fillers[:] = rest
                thunks = []
                for qt in range(4):
                    ncols = (4 * qg + qt + 1) * 65
                    thunks.append(
                        (("z", qg, h), ncols * PE_NS,
                         lambda qt=qt: chain(qt))
                    )
                thunks.append((("z", qg, h), 0.0, norm))
                idx = 0
                for i, (k, _, _) in enumerate(fillers):
                    if k[0] == "z":
                        idx = i + 1
                fillers[idx:idx] = thunks
                z_pushed_at[(qg, h)] = fill_count[0]

            # PE p-state warmup: the PE reaches full clock only after 3us
            # of continuous execution; burn the initial DMA-wait window
            # with dummy matmuls on zeroed scratch so the real chains run
            # at full speed from the start
            if warmup > 0:
                wsrc = apool.tile([128, 2, 512], bf16)
                nc.vector.memset(wsrc[:], 0.0)
                for i in range(warmup):
                    wps = ps_b.tile([128, 512], f32, tag="b",
                                    name=f"warm_{i}")
                    w = 512 if i < warmup - 2 else 256
                    nc.tensor.matmul(wps[:, 0:w], wsrc[:, 0, 0:128],
                                     wsrc[:, 1, 0:w])

            # ---- program ----
            # ct0's Q,K chains (heads 0,1) emit directly so quarter 0's S/exp
            # stream starts as soon as possible; ct1 + V chains become
            # fillers drained during h0/h1's exp (barriers: v at h==1,
            # qkv(ct1) at h==2).
            for j in range(2):
                emit_qk_chain(0, 0, j)
            for jj in range(2):
                fillers.append(
                    (("qkv", 0), 3072 * PE_NS,
                     lambda jj=jj: emit_qk_chain(0, 1, jj))
                )
            for ti in range(4):
                fillers.append(
                    (("v", 0), 1536 * PE_NS,
                     lambda ti=ti: emit_v_chain(0, ti))
                )
            push_qkv_fillers(1)

            zq_box = {}
            for qg in range(4):
                groups = _groups(qg, sreg_w)
                kt2g = {}
                for gi, grp in enumerate(groups):
                    for kt, off, w in grp:
                        kt2g[kt] = (gi, off, w)
                if qg > 0:
                    # barrier: this quarter's S/z read QT/KT/VA of tg=qg
                    drain(("qkv", qg))
                for h in range(HLOC):
                    if h == 1:
                        drain(("v", qg))
                    if h == 2 and qg == 0:
                        drain(("qkv", 0))  # ct1 chains gate heads 2,3
                    pts = []
                    # emit S groups in pairs (back-to-back on PE) so ACT's
                    # exp stream has at most one bubble per pair, not per
                    # group; the 2-buffer S rotation permits exactly 2 ahead
                    qsc, qpad = ((fill_scale, fill_pad) if fill_cfg is None
                                 else fill_cfg[qg])
                    pqg = pair_s if isinstance(pair_s, bool) else pair_s[qg]
                    step = 2 if pqg else 1
                    for g0i in range(0, len(groups), step):
                        pair = groups[g0i : g0i + step]
                        budget = 0.0
                        for gi, grp in zip(range(g0i, g0i + step), pair):
                            pts.append(emit_sgrp(h, qg, gi, grp))
                            cum = grp[-1][1] + grp[-1][2]
                            budget += ((cum * ACT_NS + 185.0) * qsc
                                       + qpad - cum * PE_NS)
                        fill(max(0.0, budget), op_ok=(qg >= op_from))
                    push_z_phase(qg, h, pts, kt2g, zq_box)

            # drain the tail (queue can grow while draining)
            while fillers:
                nxt = _pop_next(True)
                _, _, thunk = nxt if nxt else fillers.pop(0)
                thunk()

    nc.compile()
    return nc


def _pack_w(w):
    # [DM, C] -> [128, MO, C] f32: partition p holds rows {mo*128 + p}
    return np.ascontiguousarray(
        w.reshape(MO, 128, w.shape[1]).transpose(1, 0, 2)
    ).astype(np.float32)


def _split8(a):
    # f32 array -> (hi, lo) e4m3 pair with hi + lo ~= a to ~0.1%
    ah = a.astype(ml_dtypes.float8_e4m3)
    al = (a - ah.astype(np.float32)).astype(ml_dtypes.float8_e4m3)
    return np.ascontiguousarray(ah), np.ascontiguousarray(al)


def make_in_maps(x, w_qkv, b_qkv, w_out):
    # multiplicative post-exp mask: 1 where k <= q (upper incl diag), else 0
    tri = np.tri(128, 128, 0, dtype=np.float32).T.astype(ml_dtypes.bfloat16)
    ident = np.eye(128, dtype=np.float32).astype(ml_dtypes.bfloat16)
    in_maps = []
    for core in range(8):
        b = core // 4
        hg = core % 4
        c0 = hg * CLOC
        csl = slice(c0, c0 + CLOC)

        # packed consts: [128, 264] bf16-typed raw columns. Biases carry the
        # x32 weight prescale (Q,K live at 32x on device).
        cst = np.zeros((128, 264), np.uint16)
        cst[:, 0:128] = tri.view(np.uint16)
        cst[:, 128:256] = ident.view(np.uint16)
        bq = np.ascontiguousarray(
            32.0 * b_qkv[csl].astype(np.float32).reshape(2, 128).T
        )
        bk = np.ascontiguousarray(
            32.0 * b_qkv[DM + c0 : DM + c0 + CLOC].astype(np.float32)
            .reshape(2, 128).T
        )
        cst[:, 256:260] = bq.view(np.uint16).reshape(128, 4)
        cst[:, 260:264] = bk.view(np.uint16).reshape(128, 4)

        wq_p = _pack_w(32.0 * w_qkv[:, csl])
        wk_p = _pack_w(32.0 * w_qkv[:, DM + c0 : DM + c0 + CLOC])
        # [128, ctj, MO, 128]: ctj = ct*2 + j (j=0 -> Q, j=1 -> K)
        wqk = np.concatenate(
            [wq_p[:, None, :, 0:128], wk_p[:, None, :, 0:128],
             wq_p[:, None, :, 128:256], wk_p[:, None, :, 128:256]],
            axis=1,
        )
        wqkh, wqkl = _split8(wqk)
        wvh, wvl = _split8(
            _pack_w(32.0 * w_qkv[:, 2 * DM + c0 : 2 * DM + c0 + CLOC]))
        xTh, xTl = _split8(_pack_w(np.ascontiguousarray(x[b].T)))
        in_maps.append(
            {
                "xTh": xTh,
                "xTl": xTl,
                "wqkh": wqkh,
                "wqkl": wqkl,
                "wvh": wvh,
                "wvl": wvl,
                # wo: [CLOC, DM] -> [128, 2, DM]
                "wo": np.ascontiguousarray(
                    w_out[csl, :].reshape(2, 128, DM).transpose(1, 0, 2)
                ).astype(ml_dtypes.bfloat16),
                "cst": cst.view(ml_dtypes.bfloat16),
            }
        )
    return in_maps


def gather(results, b_qkv, w_out, b_out):
    # device skips the V bias; z_norm + b_v projects to a constant row:
    # y += b_v @ w_out, folded into the output bias here
    b_eff = (
        b_out.astype(np.float32)
        + b_qkv[2 * DM :].astype(np.float32) @ w_out.astype(np.float32)
    )
    out = np.empty((B, S, DM), np.float32)
    for b in range(B):
        acc = results[4 * b]["y"].astype(np.float32)
        for j in range(1, 4):
            acc = acc + results[4 * b + j]["y"]
        out[b] = acc + b_eff[None, :]
    return out


def kernel(x, w_qkv, b_qkv, w_out, b_out):
    x = np.asarray(x)
    w_qkv = np.asarray(w_qkv)
    b_qkv = np.asarray(b_qkv)
    w_out = np.asarray(w_out)
    b_out = np.asarray(b_out)

    if "nc" not in _CACHE:
        _CACHE["nc"] = build()
    nc = _CACHE["nc"]

    in_maps = make_in_maps(x, w_qkv, b_qkv, w_out)
    res = run_bass_kernel_spmd(nc, in_maps, core_ids=list(range(8)))
    return gather(res.results, b_qkv, w_out, b_out)



# revision 53
# speedup vs baseline: 1.1442x; 1.0004x over previous
"""Causal multi-head attention block (B=2, S=2048, D=1024, H=16) on 8 TRN2 cores.

Sharding: core i handles batch b = i//4 and head group hg = i%4 (4 heads =
256 model dims). Each core computes its heads' attention and a partial
output projection; the host sums the 4 partials per batch and adds b_out.

Per-core device pipeline (fp32 PSUM accumulation):
  1. QKV in compensated fp8: x and the QKV weights are hi/lo e4m3 pairs
     (hosts pre-scales weights x32 so e4m3's mantissa range is used;
     hi+lo carries ~11 mantissa bits, more precise than bf16). Each
     chain is xh@wh + xh@wl + xl@wh via DoubleRow matmuls (contraction
     256/matmul at 0.5 cycles/col = 3x bf16 throughput). Q^T,K^T land
     as [head_cols, tokens] at 32x scale (absorbed by the exp scale
     2^-13); V lands as [tokens, head_cols] at 32x, stored augmented
     with a 32.0 column so the z-matmul's row sums carry the same scale
     and the normalization cancels it exactly.
  2. Attention per head in bf16 (fp8 here fails the 2e-2 tolerance),
     flash-style in the S^T = K.Q^T orientation over the causal lower
     triangle only: S^T[k_tile, q_span] -> exp on ScalarE
     (scale=2^-13, no max subtraction; logits ~N(0,1)) -> P^T bf16 ->
     multiplicative 0/1 mask on diagonal blocks -> z[q_tile, 65]
     += P^T_chunk^T @ V_aug accumulated over k tiles in PSUM. The [q, d+1]
     z orientation makes each z matmul only 65 PE columns (vs a full
     q-span) and puts the softmax row sum in PSUM column 64 of the same
     partition as its query, so normalization is a per-partition
     tensor_scalar multiply fused into the PSUM->SBUF copy.
  3. z[q,d] tiles are transposed back to z^T[d,q] via PE transpose
     (identity matmul, 128 cols per 2-head tile) for the out-projection.
  4. Out-proj: y_partial[t, n] accumulated over the 256 local dims.

Program order is a fine-grained software pipeline: the attention loop is
a flat sequence over (q-quarter, head, k-group) with the z matmuls
lagging one group behind the S matmuls, and a filler queue (next token
group's QKV chains, previous quarters' out-proj chunks) drained between
S and z so the PE never waits on ScalarE exp. The final (ACT-bound)
quarter's last head runs a per-q-tile norm -> transpose -> out-proj
pipeline so the tail doesn't serialize, and y DMAs ride the Pool
engine's SWDGE path (+SP/ACT in the tail) to keep descriptor
generation off the exp-critical sequencers. Host pre-packs all inputs
into SBUF layouts; the V bias is folded into the output bias on the
host (b_v @ w_out).
"""

import numpy as np
import ml_dtypes

import concourse.mybir as mybir
import concourse.tile as tile
from concourse import bacc
from concourse.bass_utils import run_bass_kernel_spmd

B = 2
S = 2048
DM = 1024
HD = 64
HLOC = 4                 # heads per core
CLOC = HLOC * HD         # local model dims (256)
MO = DM // 128           # 8 k-subtiles of the model dim
NKT = S // 128           # 16 key tiles

f32 = mybir.dt.float32
bf16 = mybir.dt.bfloat16
f8 = mybir.dt.float8e4
DR = mybir.MatmulPerfMode.DoubleRow
EXP = mybir.ActivationFunctionType.Exp

ACT_NS = 0.8333333333333334
PE_NS = 0.4166666666666667

_CACHE = {}


def _groups(qg, cap=1024):
    """Pack the causal k-tile spans of query quarter qg into exp groups of
    <= cap columns. Returns list of groups; each group is a list of
    (kt, offset_in_group, width)."""
    g0 = qg * 512
    last_kt = 4 * qg + 3
    groups, cur, cum = [], [], 0
    for kt in range(last_kt + 1):
        w = g0 + 512 - max(kt * 128, g0)
        if cum + w > cap:
            groups.append(cur)
            cur, cum = [], 0
        cur.append((kt, cum, w))
        cum += w
    groups.append(cur)
    return groups


def build(pt_bufs=20, zq_bufs=4, y_bufs=4, sreg_w=1024, fill_scale=1.0,
          fill_pad=0.0, dma_splits=(2, 2, 4, 4), tri_engine="dve", op_from=2,
          zp_bufs=2, dma_transpose=True, warmup=8, alt_copy=False,
          z_delay=1, defer_v=True, copy_split=False, rr_pop=False,
          pair_s=(0, 0, 0, 1), op3_split=False, schr_groups=(0, 0, 0, 0),
          fill_cfg=((1.025, 53.0), (0.995, 56.0), (1.014, 5.0), (0.974, -100.0))):
    nc = bacc.Bacc("TRN2", target_bir_lowering=False, debug=False)

    # x and the QKV weights are fp8 e4m3 hi/lo pairs (weights pre-scaled x32
    # on the host so e4m3's mantissa is used; the x32 scales cancel in the
    # exp scale and the x32 ones-column). Q = xh@wh + xh@wl + xl@wh via
    # DoubleRow matmuls: contraction 256/matmul at 0.5 cycles/col.
    xh_d = nc.dram_tensor("xTh", [128, MO, S], f8, kind="ExternalInput")
    xl_d = nc.dram_tensor("xTl", [128, MO, S], f8, kind="ExternalInput")
    # wqk grouped per QKV chain (ctj = ct*2+j) so each chain's weights are
    # one contiguous DMA
    wqkh_d = nc.dram_tensor("wqkh", [128, 4, MO, 128], f8, kind="ExternalInput")
    wqkl_d = nc.dram_tensor("wqkl", [128, 4, MO, 128], f8, kind="ExternalInput")
    wvh_d = nc.dram_tensor("wvh", [128, MO, CLOC], f8, kind="ExternalInput")
    wvl_d = nc.dram_tensor("wvl", [128, MO, CLOC], f8, kind="ExternalInput")
    wo_d = nc.dram_tensor("wo", [128, 2, DM], bf16, kind="ExternalInput")
    # consts packed as raw bf16 columns: tri[0:128], identity[128:256],
    # bq[256:260], bk[260:264] (f32 values bit-split across bf16 pairs)
    cst_d = nc.dram_tensor("cst", [128, 264], bf16, kind="ExternalInput")
    y_d = nc.dram_tensor("y", [S, DM], bf16, kind="ExternalOutput")

    with tile.TileContext(nc) as tc:
        with (
            tc.tile_pool(name="consts", bufs=1) as consts,
            tc.tile_pool(name="acts", bufs=1) as apool,
            tc.tile_pool(name="pt", bufs=pt_bufs) as ppool,
            tc.tile_pool(name="zq", bufs=zq_bufs) as zqpool,
            tc.tile_pool(name="norm", bufs=4) as spool,
            tc.tile_pool(name="ycopy", bufs=y_bufs) as ypool,
            # 8 PSUM banks: ps_s 2x[128,1024]=4 (QK logits), ps_z
            # 2x[128,4,65]=2 (z accumulators), ps_b 2x[128,512]=2
            # (QKV / V / out-proj chains and z transposes)
            tc.tile_pool(name="ps_s", bufs=2, space="PSUM") as ps_s,
            tc.tile_pool(name="ps_z", bufs=zp_bufs, space="PSUM") as ps_z,
            tc.tile_pool(name="ps_b", bufs=2, space="PSUM") as ps_b,
        ):
            csb = consts.tile([128, 264], bf16)
            wqkh = consts.tile([128, 4, MO, 128], f8)
            wqkl = consts.tile([128, 4, MO, 128], f8)
            wvh = consts.tile([128, MO, CLOC], f8)
            wvl = consts.tile([128, MO, CLOC], f8)
            wo = consts.tile([128, 2, DM], bf16)
            xh = apool.tile([128, MO, S], f8)
            xl = apool.tile([128, MO, S], f8)

            # DMA order = consumption order (transfers serialize on the DMA
            # engines). Startup fans out over three queues (SP: weights,
            # ACT: xh, Pool/SWDGE: xl) so the first QK chain can start ~2.6us in;
            # bulk transfers stay off the ACT queue once the exp stream
            # starts (each dma_start occupies the sequencer ~660ns).
            nc.sync.dma_start(wqkh[:, 0, :, :], wqkh_d[:, 0, :, :])
            nc.scalar.dma_start(xh[:, 0:2, 0:512], xh_d[:, 0:2, 0:512])
            nc.gpsimd.dma_start(xl[:, 0:2, 0:512], xl_d[:, 0:2, 0:512])
            nc.sync.dma_start(wqkl[:, 0, :, :], wqkl_d[:, 0, :, :])
            nc.scalar.dma_start(xh[:, 2:4, 0:512], xh_d[:, 2:4, 0:512])
            nc.gpsimd.dma_start(xl[:, 2:4, 0:512], xl_d[:, 2:4, 0:512])
            nc.sync.dma_start(csb[:], cst_d[:])
            nc.scalar.dma_start(xh[:, 4:8, 0:512], xh_d[:, 4:8, 0:512])
            nc.gpsimd.dma_start(xl[:, 4:8, 0:512], xl_d[:, 4:8, 0:512])
            nc.sync.dma_start(wqkh[:, 1, :, :], wqkh_d[:, 1, :, :])
            nc.sync.dma_start(wqkl[:, 1, :, :], wqkl_d[:, 1, :, :])
            nc.sync.dma_start(wqkh[:, 2:4, :, :], wqkh_d[:, 2:4, :, :])
            nc.sync.dma_start(wqkl[:, 2:4, :, :], wqkl_d[:, 2:4, :, :])
            nc.sync.dma_start(wvh[:], wvh_d[:])
            nc.sync.dma_start(wvl[:], wvl_d[:])
            nc.sync.dma_start(xh[:, :, 512:1024], xh_d[:, :, 512:1024])
            nc.sync.dma_start(xl[:, :, 512:1024], xl_d[:, :, 512:1024])
            nc.sync.dma_start(xh[:, :, 1024:1536], xh_d[:, :, 1024:1536])
            nc.sync.dma_start(xl[:, :, 1024:1536], xl_d[:, :, 1024:1536])
            nc.sync.dma_start(xh[:, :, 1536:2048], xh_d[:, :, 1536:2048])
            nc.sync.dma_start(xl[:, :, 1536:2048], xl_d[:, :, 1536:2048])
            nc.sync.dma_start(wo[:], wo_d[:])

            tri = csb[:, 0:128]
            ident = csb[:, 128:256]
            bq_sb = csb[:, 256:260].bitcast(f32)
            bk_sb = csb[:, 260:264].bitcast(f32)

            QT = apool.tile([128, 2, S], bf16)
            KT = apool.tile([128, 2, S], bf16)
            # V augmented: [t-part, kt, h, 0:64] = v dims (x32), col 64 = 32
            # so the rowsum scale matches the v columns and the x32 cancels
            # in the normalization
            VA = apool.tile([128, NKT, HLOC, 72], bf16)
            nc.vector.memset(VA[:, :, :, 64:65], 32.0)
            zT = apool.tile([128, 2, S], bf16)

            # ---- filler queue: PE work units drained while ScalarE exps ----
            fillers = []          # list of (key, pe_ns, thunk)
            fill_debt = [0.0]
            fill_count = [0]      # fill() invocations, for z-pop delay
            z_pushed_at = {}      # (qg, h) -> fill_count at push
            rr_state = [0]

            def _z_ok(k):
                return (fill_count[0] - z_pushed_at.get((k[1], k[2]), -(1 << 30))
                        >= z_delay)

            cur_qg = [0]

            def _pop_next(op_ok=True):
                """z thunks once ScalarE has had time to produce their exp
                inputs (z_delay fill periods after push), then QKV (needed
                by the next quarter anyway), out-proj last. op thunks are
                hoarded for the late ACT-bound quarters (qg >= op_from),
                where they are the only filler class left."""
                for cls in ("z", "v", "qkv"):
                    for i, (k, ns, thunk) in enumerate(fillers):
                        if k[0] == cls and (cls != "z" or _z_ok(k)):
                            return fillers.pop(i)
                if op_ok:
                    for i, (k, ns, thunk) in enumerate(fillers):
                        if k[0] == "op":
                            return fillers.pop(i)
                    return fillers.pop(0) if fillers else None
                return None

            def fill(budget_ns, op_ok=True):
                fill_count[0] += 1
                budget = budget_ns + fill_debt[0]
                spent = 0.0
                while fillers and spent < budget:
                    nxt = _pop_next(op_ok)
                    if nxt is None:
                        # only hoarded op (or unready z) left: stop without
                        # banking debt so qg2's first fill doesn't burst
                        fill_debt[0] = 0.0
                        return
                    _, ns, thunk = nxt
                    thunk()
                    spent += ns
                fill_debt[0] = budget - spent if fillers else 0.0

            def drain(key):
                """Force-emit queued fillers matching key (dependency
                barrier: attention on quarter qg needs all of QKV(tg=qg))."""
                rest = []
                for k, ns, thunk in fillers:
                    if k == key:
                        thunk()
                    else:
                        rest.append((k, ns, thunk))
                fillers[:] = rest

            def drain_class(cls):
                rest = []
                for k, ns, thunk in fillers:
                    if k[0] == cls:
                        thunk()
                    else:
                        rest.append((k, ns, thunk))
                fillers[:] = rest

            qkv_terms = ((0, 0), (0, 1), (1, 0))  # (x lo?, w lo?) per term

            def emit_qk_chain(tg, ct, j):
                tsl = slice(tg * 512, (tg + 1) * 512)
                dst, b_sb = ((QT, bq_sb), (KT, bk_sb))[j]
                ctj = ct * 2 + j
                ps = ps_b.tile([128, 512], f32, tag="b",
                               name=f"qk_{tg}_{ct}_{j}")
                for sub in range(2):
                    t0 = tg * 512 + sub * 256
                    i = 0
                    for xlo, wlo in qkv_terms:
                        xs = (xh, xl)[xlo]
                        ws = (wqkh, wqkl)[wlo]
                        for c in range(MO // 2):
                            nc.tensor.matmul(
                                ps[:, sub * 256 : sub * 256 + 256],
                                ws[:, ctj, 2 * c : 2 * c + 2, :],
                                xs[:, 2 * c : 2 * c + 2, t0 : t0 + 256],
                                start=(i == 0),
                                stop=(i == 11),
                                perf_mode=DR,
                            )
                            i += 1
                nc.vector.tensor_scalar_add(
                    dst[:, ct, tsl], ps[:], b_sb[:, ct : ct + 1]
                )

            def emit_v_chain(tg, ti):
                tt = tg * 4 + ti
                ps = ps_b.tile([128, 512], f32, tag="b", name=f"v_{tg}_{ti}")
                i = 0
                for xlo, wlo in qkv_terms:
                    xs = (xh, xl)[xlo]
                    ws = (wvh, wvl)[wlo]
                    for c in range(MO // 2):
                        nc.tensor.matmul(
                            ps[:, 0:CLOC],
                            xs[:, 2 * c : 2 * c + 2, tt * 128 : (tt + 1) * 128],
                            ws[:, 2 * c : 2 * c + 2, :],
                            start=(i == 0),
                            stop=(i == 11),
                            perf_mode=DR,
                        )
                        i += 1
                nc.vector.tensor_copy(
                    VA[:, tt, :, 0:64],
                    ps[:, 0:CLOC].rearrange("p (h d) -> p h d", d=64),
                )

            def emit_qkv(tg):
                for ct in range(2):
                    for j in range(2):
                        emit_qk_chain(tg, ct, j)
                for ti in range(4):
                    emit_v_chain(tg, ti)

            def push_qkv_fillers(tg):
                for ct in range(2):
                    for j in range(2):
                        fillers.append(
                            (("qkv", tg), 3072 * PE_NS,
                             lambda tg=tg, ct=ct, j=j: emit_qk_chain(tg, ct, j))
                        )
                vcls = "v" if defer_v else "qkv"
                for ti in range(4):
                    fillers.append(
                        ((vcls, tg), 1536 * PE_NS,
                         lambda tg=tg, ti=ti: emit_v_chain(tg, ti))
                    )

            # ---- attention ----
            def emit_sgrp(h, qg, gi, grp):
                """S^T matmuls for one exp group + the exp + diag masks."""
                hp = (h % 2) * 64
                ct = h // 2
                g0 = qg * 512
                cum = grp[-1][1] + grp[-1][2]
                sreg = ps_s.tile([128, sreg_w], f32, tag="s",
                                 name=f"s_{h}_{qg}_{gi}")
                for kt, off, w in grp:
                    q0 = g0 + 512 - w
                    c0 = off
                    while c0 < off + w:
                        cw = min(off + w - c0, 512 - c0 % 512)
                        nc.tensor.matmul(
                            sreg[:, c0 : c0 + cw],
                            KT[hp : hp + 64, ct, kt * 128 : (kt + 1) * 128],
                            QT[hp : hp + 64, ct,
                               q0 + c0 - off : q0 + c0 - off + cw],
                        )
                        c0 += cw
                pT = ppool.tile([128, sreg_w], bf16, tag="pT",
                                name=f"pT_{h}_{qg}_{gi}")
                no_diag = all(kt * 128 < g0 for kt, _, _ in grp)
                if gi < schr_groups[qg] and no_diag:
                    # Schraudolph exp on DVE: bf16 bits of exp(x*0.125) ~=
                    # int16(x*(0.125*128/ln2) + (127*128 - 5.6)); offloads
                    # the ACT engine (the attention-band pacer) at ~2% rms
                    # error on this group's P entries. sreg is 1024x (Q,K
                    # each carry the x32 weight prescale).
                    nc.vector.tensor_scalar(
                        pT[:, :cum].bitcast(mybir.dt.int16), sreg[:, :cum],
                        scalar1=23.0831253 / 1024.0, op0=mybir.AluOpType.mult,
                        scalar2=16250.4, op1=mybir.AluOpType.add)
                else:
                    nc.scalar.activation(pT[:, :cum], sreg[:, :cum], EXP,
                                         scale=0.125 / 1024.0)
                teng = nc.gpsimd if tri_engine == "gpsimd" else nc.vector
                for kt, off, w in grp:
                    if kt * 128 >= g0:  # diagonal block leads the span
                        teng.tensor_mul(
                            pT[:, off : off + 128],
                            pT[:, off : off + 128],
                            tri[:],
                        )
                return pT

            def emit_zchain(h, qg, qt, pts, kt2g, zp):
                """z[q,65] = sum_kt pT_chunk^T @ V_aug: one sequential PSUM
                accumulation chain per q-tile (a PSUM bank supports only one
                open accumulation group at a time)."""
                g0 = qg * 512
                qa = 4 * qg + qt
                for kt in range(qa + 1):
                    gi, off, w = kt2g[kt]
                    q0 = g0 + 512 - w
                    c0 = off + (g0 + qt * 128) - q0
                    nc.tensor.matmul(
                        zp[:, qt, 0:65],
                        pts[gi][:, c0 : c0 + 128],
                        VA[:, kt, h, 0:65],
                        start=(kt == 0),
                        stop=(kt == qa),
                    )

            def emit_norm(h, qg, zp, zq):
                """1/rowsum fused into the PSUM->SBUF copy of z."""
                hp = (h % 2) * 64
                rec = spool.tile([128, 4, 1], f32, tag="rec",
                                 name=f"rec_{h}_{qg}")
                nc.vector.reciprocal(rec[:], zp[:, :, 64:65])
                with nc.allow_low_precision(reason="attn out to bf16"):
                    for qt in range(4):
                        nc.vector.tensor_scalar_mul(
                            zq[:, qt, hp : hp + 64],
                            zp[:, qt, 0:64],
                            rec[:, qt, :],
                        )

            def emit_transpose(qg, pair, zq):
                """zq [q,128d] -> zT [128d, q]. All these transposes are
                latency-tolerant (op(qg) reads zT several us later), so they
                ride the DMA xbar instead of PE+DVE; only the tail-critical
                qg3/pair1 path (in the tailpipe) uses the PE transpose."""
                if dma_transpose:
                    for qt in range(4):
                        tt = qg * 4 + qt
                        nc.sync.dma_start_transpose(
                            zT[:, pair, tt * 128 : (tt + 1) * 128],
                            zq[:, qt, :],
                        )
                    return
                quad = ps_b.tile([128, 4, 128], bf16, tag="b",
                                 name=f"tq_{qg}_{pair}")
                for qt in range(4):
                    nc.tensor.transpose(quad[:, qt, :], zq[:, qt, :], ident)
                with nc.allow_low_precision(reason="zT copy"):
                    nc.vector.tensor_copy(
                        zT[:, pair, qg * 512 : (qg + 1) * 512],
                        quad[:].rearrange("p a b -> p (a b)"),
                    )

            def emit_op_half(qg, nh, ti, yA):
                tt = qg * 4 + ti
                ps = ps_b.tile([128, 512], f32, tag="b",
                               name=f"opA_{qg}_{nh}_{ti}")
                nc.tensor.matmul(
                    ps[:],
                    zT[:, 0, tt * 128 : (tt + 1) * 128],
                    wo[:, 0, nh * 512 : (nh + 1) * 512],
                )
                nc.vector.tensor_copy(yA[:, ti, :], ps[:])

            def emit_op_chunk(qg, nh, ti, ysb, dma_split, yA=None,
                              pool=None):
                tt = qg * 4 + ti
                # tail op chunks borrow ps_s (free once the last exp group
                # is consumed) so 4 chunks pipeline instead of 2
                ps = (pool or ps_b).tile([128, 512], f32,
                                         tag="b" if pool is None else "s",
                                         name=f"op_{qg}_{nh}_{ti}")
                cos = (1,) if yA is not None else (0, 1)
                for co in cos:
                    nc.tensor.matmul(
                        ps[:],
                        zT[:, co, tt * 128 : (tt + 1) * 128],
                        wo[:, co, nh * 512 : (nh + 1) * 512],
                        start=(co == cos[0]),
                        stop=(co == 1),
                    )
                with nc.allow_low_precision(reason="y partial to bf16"):
                    if yA is not None:
                        nc.vector.tensor_add(ysb[:, ti, :], ps[:],
                                             yA[:, ti, :])
                    elif qg == 3:
                        # tail: alternate engines by (ti+nh) so each q-tile's
                        # two chunk copies run on different engines in
                        # parallel instead of serializing on one
                        (nc.scalar.copy if (ti + nh) % 2 == 0
                         else nc.vector.tensor_copy)(ysb[:, ti, :], ps[:])
                    else:
                        ((nc.scalar.copy if (ti + nh) % 2 == 0
                          else nc.vector.tensor_copy) if alt_copy
                         else nc.vector.tensor_copy)(ysb[:, ti, :], ps[:])
                nper = 4 // dma_split
                if ti % nper == nper - 1:
                    t0 = tt - nper + 1
                    # y DMA queue: Pool/SWDGE keeps descriptor gen off the
                    # ACT/SP sequencers and HWDGE during the exp-critical
                    # band; in the qg3 tail (exp done) rotate across all
                    # three so gens parallelize
                    if qg == 3:
                        deng = (nc.gpsimd, nc.sync, nc.scalar)[
                            (ti // nper + nh * dma_split) % 3]
                    else:
                        deng = (nc.gpsimd, nc.sync)[(ti // nper + nh) % 2]
                    deng.dma_start(
                        y_d[t0 * 128 : (tt + 1) * 128,
                            nh * 512 : (nh + 1) * 512].rearrange(
                            "(ti p) n -> p ti n", p=128
                        ),
                        ysb[:, ti - nper + 1 : ti + 1, :],
                    )

            yA_box = {}

            def push_opA_fillers(qg):
                # first half (co=0) of qg3's out-proj: available right after
                # pair 0's transpose, staged to SBUF f32; the co=1 half plus
                # an add happens in the tail
                for nh in range(2):
                    yA = ypool.tile([128, 4, 512], f32, tag="yA", bufs=2,
                                    name=f"yA_{qg}_{nh}")
                    yA_box[(qg, nh)] = yA
                    for ti in range(4):
                        fillers.append(
                            (("op", qg), 512 * PE_NS,
                             lambda qg=qg, nh=nh, ti=ti, yA=yA:
                                 emit_op_half(qg, nh, ti, yA))
                        )

            def push_op_fillers(qg, split=False):
                dma_split = dma_splits[qg]
                for nh in range(2):
                    ysb = ypool.tile([128, 4, 512], bf16, tag="y",
                                     name=f"ysb_{qg}_{nh}")
                    yA = yA_box.get((qg, nh)) if split else None
                    for ti in range(4):
                        fillers.append(
                            (("op", qg), (512 if split else 1024) * PE_NS,
                             lambda qg=qg, nh=nh, ti=ti, ysb=ysb,
                                    ds=dma_split, yA=yA:
                                 emit_op_chunk(qg, nh, ti, ysb, ds, yA))
                        )

            def push_z_phase(qg, h, pts, kt2g, zq_box):
                """Queue head h's z chains + normalize (+ transpose) at the
                FRONT of the filler queue; they drain during head h+1's S
                phase (one-head software pipeline)."""
                box = {}
                # final unit (qg3 h3): per-q-tile norm -> transpose -> zT
                # copy -> that tile's out-proj chunks, so the tail pipelines
                # instead of serializing behind the full head
                tailpipe = qg == 3 and h == HLOC - 1

                def chain(qt):
                    if qt == 0:
                        box["zp"] = ps_z.tile([128, 4, 65], f32, tag="z",
                                              name=f"zp_{h}_{qg}")
                        if h % 2 == 0:
                            zq_box[h // 2] = zqpool.tile(
                                [128, 4, 128], bf16, tag="zq",
                                name=f"zq_{qg}_{h // 2}")
                    emit_zchain(h, qg, qt, pts, kt2g, box["zp"])
                    if tailpipe:
                        zq = zq_box[h // 2]
                        if qt == 0:
                            box["rec"] = spool.tile([128, 4, 1], f32,
                                                    tag="rec",
                                                    name=f"rec_{h}_{qg}")
                            box["ysb"] = [
                                ypool.tile([128, 4, 512], bf16, tag="y",
                                           name=f"ysb_3_{nh}")
                                for nh in range(2)
                            ]
                        rec = box["rec"]
                        zp = box["zp"]
                        nc.vector.reciprocal(rec[:, qt, :], zp[:, qt, 64:65])
                        with nc.allow_low_precision(reason="attn out bf16"):
                            nc.vector.tensor_scalar_mul(
                                zq[:, qt, 64:128], zp[:, qt, 0:64],
                                rec[:, qt, :])
                        quad = ps_b.tile([128, 128], bf16, tag="b",
                                         name=f"tq3_{qt}")
                        nc.tensor.transpose(quad[:], zq[:, qt, :], ident)
                        tt = qg * 4 + qt
                        with nc.allow_low_precision(reason="zT copy"):
                            nc.vector.tensor_copy(
                                zT[:, 1, tt * 128 : (tt + 1) * 128], quad[:])
                        for nh in range(2):
                            fillers.append(
                                (("op", 3), 1024 * PE_NS,
                                 lambda nh=nh, ti=qt:
                                     emit_op_chunk(3, nh, ti,
                                                   box["ysb"][nh],
                                                   dma_splits[3],
                                                   pool=ps_s))
                            )

                def norm():
                    if tailpipe:
                        return
                    emit_norm(h, qg, box["zp"], zq_box[h // 2])
                    if h % 2 == 1:
                        emit_transpose(qg, h // 2, zq_box[h // 2])
                        if h == 1 and qg == 3 and op3_split:
                            push_opA_fillers(qg)
                        if h == HLOC - 1:
                            # quarter finished: queue its out-proj (reads
                            # zT(qg), complete as of this point) and the
                            # next token group's QKV
                            push_op_fillers(qg, split=(qg == 3 and op3_split))
                            if qg + 2 <= 3:
                                push_qkv_fillers(qg + 2)

                # z thunks of heads <= h-2 must fully precede this head's
                # (ps_z has 2 bufs, so h-1's z may still be queued); h's
                # thunks append after h-1's so class-FIFO order holds
                prev = ("z", qg, h - 1)
                rest = []
                for k, ns, thunk in fillers:
                    if k[0] == "z" and k != prev:
                        thunk()
                    else:
                        rest.append((k, ns, thunk))
                fillers[:] = rest
                thunks = []
                for qt in range(4):
                    ncols = (4 * qg + qt + 1) * 65
                    thunks.append(
                        (("z", qg, h), ncols * PE_NS,
                         lambda qt=qt: chain(qt))
                    )
                thunks.append((("z", qg, h), 0.0, norm))
                idx = 0
                for i, (k, _, _) in enumerate(fillers):
                    if k[0] == "z":
                        idx = i + 1
                fillers[idx:idx] = thunks
                z_pushed_at[(qg, h)] = fill_count[0]

            # PE p-state warmup: the PE reaches full clock only after 3us
            # of continuous execution; burn the initial DMA-wait window
            # with dummy matmuls on zeroed scratch so the real chains run
            # at full speed from the start
            if warmup > 0:
                wsrc = apool.tile([128, 2, 512], bf16)
                nc.vector.memset(wsrc[:], 0.0)
                for i in range(warmup):
                    wps = ps_b.tile([128, 512], f32, tag="b",
                                    name=f"warm_{i}")
                    w = 512 if i < warmup - 2 else 256
                    nc.tensor.matmul(wps[:, 0:w], wsrc[:, 0, 0:128],
                                     wsrc[:, 1, 0:w])

            # ---- program ----
            # ct0's Q,K chains (heads 0,1) emit directly so quarter 0's S/exp
            # stream starts as soon as possible; ct1 + V chains become
            # fillers drained during h0/h1's exp (barriers: v at h==1,
            # qkv(ct1) at h==2).
            for j in range(2):
                emit_qk_chain(0, 0, j)
            for jj in range(2):
                fillers.append(
                    (("qkv", 0), 3072 * PE_NS,
                     lambda jj=jj: emit_qk_chain(0, 1, jj))
                )
            for ti in range(4):
                fillers.append(
                    (("v", 0), 1536 * PE_NS,
                     lambda ti=ti: emit_v_chain(0, ti))
                )
            push_qkv_fillers(1)

            zq_box = {}
            for qg in range(4):
                groups = _groups(qg, sreg_w)
                kt2g = {}
                for gi, grp in enumerate(groups):
                    for kt, off, w in grp:
                        kt2g[kt] = (gi, off, w)
                if qg > 0:
                    # barrier: this quarter's S/z read QT/KT/VA of tg=qg
                    drain(("qkv", qg))
                for h in range(HLOC):
                    if h == 1:
                        drain(("v", qg))
                    if h == 2 and qg == 0:
                        drain(("qkv", 0))  # ct1 chains gate heads 2,3
                    pts = []
                    # emit S groups in pairs (back-to-back on PE) so ACT's
                    # exp stream has at most one bubble per pair, not per
                    # group; the 2-buffer S rotation permits exactly 2 ahead
                    qsc, qpad = ((fill_scale, fill_pad) if fill_cfg is None
                                 else fill_cfg[qg])
                    pqg = pair_s if isinstance(pair_s, bool) else pair_s[qg]
                    step = 2 if pqg else 1
                    for g0i in range(0, len(groups), step):
                        pair = groups[g0i : g0i + step]
                        budget = 0.0
                        for gi, grp in zip(range(g0i, g0i + step), pair):
                            pts.append(emit_sgrp(h, qg, gi, grp))
                            cum = grp[-1][1] + grp[-1][2]
                            budget += ((cum * ACT_NS + 185.0) * qsc
                                       + qpad - cum * PE_NS)
                        fill(max(0.0, budget), op_ok=(qg >= op_from))
                    push_z_phase(qg, h, pts, kt2g, zq_box)

            # drain the tail (queue can grow while draining)
            while fillers:
                nxt = _pop_next(True)
                _, _, thunk = nxt if nxt else fillers.pop(0)
                thunk()

    nc.compile()
    return nc


def _pack_w(w):
    # [DM, C] -> [128, MO, C] f32: partition p holds rows {mo*128 + p}
    return np.ascontiguousarray(
        w.reshape(MO, 128, w.shape[1]).transpose(1, 0, 2)
    ).astype(np.float32)


def _split8(a):
    # f32 array -> (hi, lo) e4m3 pair with hi + lo ~= a to ~0.1%
    ah = a.astype(ml_dtypes.float8_e4m3)
    al = (a - ah.astype(np.float32)).astype(ml_dtypes.float8_e4m3)
    return np.ascontiguousarray(ah), np.ascontiguousarray(al)


def make_in_maps(x, w_qkv, b_qkv, w_out):
    # multiplicative post-exp mask: 1 where k <= q (upper incl diag), else 0
    tri = np.tri(128, 128, 0, dtype=np.float32).T.astype(ml_dtypes.bfloat16)
    ident = np.eye(128, dtype=np.float32).astype(ml_dtypes.bfloat16)
    in_maps = []
    for core in range(8):
        b = core // 4
        hg = core % 4
        c0 = hg * CLOC
        csl = slice(c0, c0 + CLOC)

        # packed consts: [128, 264] bf16-typed raw columns. Biases carry the
        # x32 weight prescale (Q,K live at 32x on device).
        cst = np.zeros((128, 264), np.uint16)
        cst[:, 0:128] = tri.view(np.uint16)
        cst[:, 128:256] = ident.view(np.uint16)
        bq = np.ascontiguousarray(
            32.0 * b_qkv[csl].astype(np.float32).reshape(2, 128).T
        )
        bk = np.ascontiguousarray(
            32.0 * b_qkv[DM + c0 : DM + c0 + CLOC].astype(np.float32)
            .reshape(2, 128).T
        )
        cst[:, 256:260] = bq.view(np.uint16).reshape(128, 4)
        cst[:, 260:264] = bk.view(np.uint16).reshape(128, 4)

        wq_p = _pack_w(32.0 * w_qkv[:, csl])
        wk_p = _pack_w(32.0 * w_qkv[:, DM + c0 : DM + c0 + CLOC])
        # [128, ctj, MO, 128]: ctj = ct*2 + j (j=0 -> Q, j=1 -> K)
        wqk = np.concatenate(
            [wq_p[:, None, :, 0:128], wk_p[:, None, :, 0:128],
             wq_p[:, None, :, 128:256], wk_p[:, None, :, 128:256]],
            axis=1,
        )
        wqkh, wqkl = _split8(wqk)
        wvh, wvl = _split8(
            _pack_w(32.0 * w_qkv[:, 2 * DM + c0 : 2 * DM + c0 + CLOC]))
        xTh, xTl = _split8(_pack_w(np.ascontiguousarray(x[b].T)))
        in_maps.append(
            {
                "xTh": xTh,
                "xTl": xTl,
                "wqkh": wqkh,
                "wqkl": wqkl,
                "wvh": wvh,
                "wvl": wvl,
                # wo: [CLOC, DM] -> [128, 2, DM]
                "wo": np.ascontiguousarray(
                    w_out[csl, :].reshape(2, 128, DM).transpose(1, 0, 2)
                ).astype(ml_dtypes.bfloat16),
                "cst": cst.view(ml_dtypes.bfloat16),
            }
        )
    return in_maps


def gather(results, b_qkv, w_out, b_out):
    # device skips the V bias; z_norm + b_v projects to a constant row:
    # y += b_v @ w_out, folded into the output bias here
    b_eff = (
        b_out.astype(np.float32)
        + b_qkv[2 * DM :].astype(np.float32) @ w_out.astype(np.float32)
    )
    out = np.empty((B, S, DM), np.float32)
    for b in range(B):
        acc = results[4 * b]["y"].astype(np.float32)
        for j in range(1, 4):
            acc = acc + results[4 * b + j]["y"]
        out[b] = acc + b_eff[None, :]
    return out


def kernel(x, w_qkv, b_qkv, w_out, b_out):
    x = np.asarray(x)
    w_qkv = np.asarray(w_qkv)
    b_qkv = np.asarray(b_qkv)
    w_out = np.asarray(w_out)
    b_out = np.asarray(b_out)

    if "nc" not in _CACHE:
        _CACHE["nc"] = build()
    nc = _CACHE["nc"]

    in_maps = make_in_maps(x, w_qkv, b_qkv, w_out)
    res = run_bass_kernel_spmd(nc, in_maps, core_ids=list(range(8)))
    return gather(res.results, b_qkv, w_out, b_out)



# revision 59
# speedup vs baseline: 1.1464x; 1.0020x over previous
"""Causal multi-head attention block (B=2, S=2048, D=1024, H=16) on 8 TRN2 cores.

Sharding: core i handles batch b = i//4 and head group hg = i%4 (4 heads =
256 model dims). Each core computes its heads' attention and a partial
output projection; the host sums the 4 partials per batch and adds b_out.

Per-core device pipeline (fp32 PSUM accumulation):
  1. QKV in compensated fp8: x and the QKV weights are hi/lo e4m3 pairs
     (hosts pre-scales weights x32 so e4m3's mantissa range is used;
     hi+lo carries ~11 mantissa bits, more precise than bf16). Each
     chain is xh@wh + xh@wl + xl@wh via DoubleRow matmuls (contraction
     256/matmul at 0.5 cycles/col = 3x bf16 throughput). Q^T,K^T land
     as [head_cols, tokens] at 32x scale (absorbed by the exp scale
     2^-13); V lands as [tokens, head_cols] at 32x, stored augmented
     with a 32.0 column so the z-matmul's row sums carry the same scale
     and the normalization cancels it exactly.
  2. Attention per head in bf16 (fp8 here fails the 2e-2 tolerance),
     flash-style in the S^T = K.Q^T orientation over the causal lower
     triangle only: S^T[k_tile, q_span] -> exp on ScalarE
     (scale=2^-13, no max subtraction; logits ~N(0,1)) -> P^T bf16 ->
     multiplicative 0/1 mask on diagonal blocks -> z[q_tile, 65]
     += P^T_chunk^T @ V_aug accumulated over k tiles in PSUM. The [q, d+1]
     z orientation makes each z matmul only 65 PE columns (vs a full
     q-span) and puts the softmax row sum in PSUM column 64 of the same
     partition as its query, so normalization is a per-partition
     tensor_scalar multiply fused into the PSUM->SBUF copy.
  3. z[q,d] tiles are transposed back to z^T[d,q] via PE transpose
     (identity matmul, 128 cols per 2-head tile) for the out-projection.
  4. Out-proj: y_partial[t, n] accumulated over the 256 local dims.

Program order is a fine-grained software pipeline: the attention loop is
a flat sequence over (q-quarter, head, k-group) with the z matmuls
lagging one group behind the S matmuls, and a filler queue (next token
group's QKV chains, previous quarters' out-proj chunks) drained between
S and z so the PE never waits on ScalarE exp. The final (ACT-bound)
quarter's last head runs a per-q-tile norm -> transpose -> out-proj
pipeline so the tail doesn't serialize, and y DMAs ride the Pool
engine's SWDGE path (+SP/ACT in the tail) to keep descriptor
generation off the exp-critical sequencers. Host pre-packs all inputs
into SBUF layouts; the V bias is folded into the output bias on the
host (b_v @ w_out).
"""

import numpy as np
import ml_dtypes

import concourse.mybir as mybir
import concourse.tile as tile
from concourse import bacc
from concourse.bass_utils import run_bass_kernel_spmd

B = 2
S = 2048
DM = 1024
HD = 64
HLOC = 4                 # heads per core
CLOC = HLOC * HD         # local model dims (256)
MO = DM // 128           # 8 k-subtiles of the model dim
NKT = S // 128           # 16 key tiles

f32 = mybir.dt.float32
bf16 = mybir.dt.bfloat16
f8 = mybir.dt.float8e4
DR = mybir.MatmulPerfMode.DoubleRow
EXP = mybir.ActivationFunctionType.Exp

ACT_NS = 0.8333333333333334
PE_NS = 0.4166666666666667

_CACHE = {}


def _groups(qg, cap=1024):
    """Pack the causal k-tile spans of query quarter qg into exp groups of
    <= cap columns. Returns list of groups; each group is a list of
    (kt, offset_in_group, width)."""
    g0 = qg * 512
    last_kt = 4 * qg + 3
    groups, cur, cum = [], [], 0
    for kt in range(last_kt + 1):
        w = g0 + 512 - max(kt * 128, g0)
        if cum + w > cap:
            groups.append(cur)
            cur, cum = [], 0
        cur.append((kt, cum, w))
        cum += w
    groups.append(cur)
    return groups


def build(pt_bufs=20, zq_bufs=4, y_bufs=4, sreg_w=1024, fill_scale=1.0,
          fill_pad=0.0, dma_splits=(2, 2, 4, 4), tri_engine="dve", op_from=2,
          zp_bufs=2, dma_transpose=True, warmup=8, alt_copy=False,
          z_delay=1, defer_v=True, copy_split=False, rr_pop=False,
          pair_s=(0, 0, 0, 1), op3_split=False, schr_groups=(0, 0, 0, 0),
          fill_cfg=((1.025, 53.0), (0.995, 56.0), (1.014, 5.0), (0.974, -100.0))):
    nc = bacc.Bacc("TRN2", target_bir_lowering=False, debug=False)

    # x and the QKV weights are fp8 e4m3 hi/lo pairs (weights pre-scaled x32
    # on the host so e4m3's mantissa is used; the x32 scales cancel in the
    # exp scale and the x32 ones-column). Q = xh@wh + xh@wl + xl@wh via
    # DoubleRow matmuls: contraction 256/matmul at 0.5 cycles/col.
    xh_d = nc.dram_tensor("xTh", [128, MO, S], f8, kind="ExternalInput")
    xl_d = nc.dram_tensor("xTl", [128, MO, S], f8, kind="ExternalInput")
    # wqk grouped per QKV chain (ctj = ct*2+j) so each chain's weights are
    # one contiguous DMA
    wqkh_d = nc.dram_tensor("wqkh", [128, 4, MO, 128], f8, kind="ExternalInput")
    wqkl_d = nc.dram_tensor("wqkl", [128, 4, MO, 128], f8, kind="ExternalInput")
    wvh_d = nc.dram_tensor("wvh", [128, MO, CLOC], f8, kind="ExternalInput")
    wvl_d = nc.dram_tensor("wvl", [128, MO, CLOC], f8, kind="ExternalInput")
    wo_d = nc.dram_tensor("wo", [128, 2, DM], bf16, kind="ExternalInput")
    # consts packed as raw bf16 columns: tri[0:128], identity[128:256],
    # bq[256:260], bk[260:264] (f32 values bit-split across bf16 pairs)
    cst_d = nc.dram_tensor("cst", [128, 264], bf16, kind="ExternalInput")
    y_d = nc.dram_tensor("y", [S, DM], bf16, kind="ExternalOutput")

    with tile.TileContext(nc) as tc:
        with (
            tc.tile_pool(name="consts", bufs=1) as consts,
            tc.tile_pool(name="acts", bufs=1) as apool,
            tc.tile_pool(name="pt", bufs=pt_bufs) as ppool,
            tc.tile_pool(name="zq", bufs=zq_bufs) as zqpool,
            tc.tile_pool(name="norm", bufs=4) as spool,
            tc.tile_pool(name="ycopy", bufs=y_bufs) as ypool,
            # 8 PSUM banks: ps_s 2x[128,1024]=4 (QK logits), ps_z
            # 2x[128,4,65]=2 (z accumulators), ps_b 2x[128,512]=2
            # (QKV / V / out-proj chains and z transposes)
            tc.tile_pool(name="ps_s", bufs=2, space="PSUM") as ps_s,
            tc.tile_pool(name="ps_z", bufs=zp_bufs, space="PSUM") as ps_z,
            tc.tile_pool(name="ps_b", bufs=2, space="PSUM") as ps_b,
        ):
            csb = consts.tile([128, 264], bf16)
            wqkh = consts.tile([128, 4, MO, 128], f8)
            wqkl = consts.tile([128, 4, MO, 128], f8)
            wvh = consts.tile([128, MO, CLOC], f8)
            wvl = consts.tile([128, MO, CLOC], f8)
            wo = consts.tile([128, 2, DM], bf16)
            xh = apool.tile([128, MO, S], f8)
            xl = apool.tile([128, MO, S], f8)

            # DMA order = consumption order (transfers serialize on the DMA
            # engines). Startup fans out over three queues (SP: weights,
            # ACT: xh, Pool/SWDGE: xl) so the first QK chain can start ~2.6us in;
            # bulk transfers stay off the ACT queue once the exp stream
            # starts (each dma_start occupies the sequencer ~660ns).
            nc.sync.dma_start(wqkh[:, 0, :, :], wqkh_d[:, 0, :, :])
            nc.scalar.dma_start(xh[:, 0:2, 0:512], xh_d[:, 0:2, 0:512])
            nc.gpsimd.dma_start(xl[:, 0:2, 0:512], xl_d[:, 0:2, 0:512])
            nc.sync.dma_start(wqkl[:, 0, :, :], wqkl_d[:, 0, :, :])
            nc.scalar.dma_start(xh[:, 2:4, 0:512], xh_d[:, 2:4, 0:512])
            nc.gpsimd.dma_start(xl[:, 2:4, 0:512], xl_d[:, 2:4, 0:512])
            nc.sync.dma_start(wqkh[:, 1, :, :], wqkh_d[:, 1, :, :])
            nc.scalar.dma_start(xh[:, 4:8, 0:512], xh_d[:, 4:8, 0:512])
            nc.gpsimd.dma_start(xl[:, 4:8, 0:512], xl_d[:, 4:8, 0:512])
            nc.sync.dma_start(csb[:], cst_d[:])
            nc.sync.dma_start(wqkl[:, 1, :, :], wqkl_d[:, 1, :, :])
            nc.sync.dma_start(wvh[:], wvh_d[:])
            nc.sync.dma_start(wvl[:], wvl_d[:])
            nc.sync.dma_start(wqkh[:, 2:4, :, :], wqkh_d[:, 2:4, :, :])
            nc.sync.dma_start(wqkl[:, 2:4, :, :], wqkl_d[:, 2:4, :, :])
            nc.sync.dma_start(xh[:, :, 512:1024], xh_d[:, :, 512:1024])
            nc.sync.dma_start(xl[:, :, 512:1024], xl_d[:, :, 512:1024])
            nc.sync.dma_start(xh[:, :, 1024:1536], xh_d[:, :, 1024:1536])
            nc.sync.dma_start(xl[:, :, 1024:1536], xl_d[:, :, 1024:1536])
            nc.sync.dma_start(xh[:, :, 1536:2048], xh_d[:, :, 1536:2048])
            nc.sync.dma_start(xl[:, :, 1536:2048], xl_d[:, :, 1536:2048])
            nc.sync.dma_start(wo[:], wo_d[:])

            tri = csb[:, 0:128]
            ident = csb[:, 128:256]
            bq_sb = csb[:, 256:260].bitcast(f32)
            bk_sb = csb[:, 260:264].bitcast(f32)

            QT = apool.tile([128, 2, S], bf16)
            KT = apool.tile([128, 2, S], bf16)
            # V augmented: [t-part, kt, h, 0:64] = v dims (x32), col 64 = 32
            # so the rowsum scale matches the v columns and the x32 cancels
            # in the normalization
            VA = apool.tile([128, NKT, HLOC, 72], bf16)
            nc.vector.memset(VA[:, :, :, 64:65], 32.0)
            zT = apool.tile([128, 2, S], bf16)

            # ---- filler queue: PE work units drained while ScalarE exps ----
            fillers = []          # list of (key, pe_ns, thunk)
            fill_debt = [0.0]
            fill_count = [0]      # fill() invocations, for z-pop delay
            z_pushed_at = {}      # (qg, h) -> fill_count at push
            rr_state = [0]

            def _z_ok(k):
                return (fill_count[0] - z_pushed_at.get((k[1], k[2]), -(1 << 30))
                        >= z_delay)

            cur_qg = [0]

            def _pop_next(op_ok=True):
                """z thunks once ScalarE has had time to produce their exp
                inputs (z_delay fill periods after push), then QKV (needed
                by the next quarter anyway), out-proj last. op thunks are
                hoarded for the late ACT-bound quarters (qg >= op_from),
                where they are the only filler class left."""
                for cls in ("z", "v", "qkv"):
                    for i, (k, ns, thunk) in enumerate(fillers):
                        if k[0] == cls and (cls != "z" or _z_ok(k)):
                            return fillers.pop(i)
                if op_ok:
                    for i, (k, ns, thunk) in enumerate(fillers):
                        if k[0] == "op":
                            return fillers.pop(i)
                    return fillers.pop(0) if fillers else None
                return None

            def fill(budget_ns, op_ok=True):
                fill_count[0] += 1
                budget = budget_ns + fill_debt[0]
                spent = 0.0
                while fillers and spent < budget:
                    nxt = _pop_next(op_ok)
                    if nxt is None:
                        # only hoarded op (or unready z) left: stop without
                        # banking debt so qg2's first fill doesn't burst
                        fill_debt[0] = 0.0
                        return
                    _, ns, thunk = nxt
                    thunk()
                    spent += ns
                fill_debt[0] = budget - spent if fillers else 0.0

            def drain(key):
                """Force-emit queued fillers matching key (dependency
                barrier: attention on quarter qg needs all of QKV(tg=qg))."""
                rest = []
                for k, ns, thunk in fillers:
                    if k == key:
                        thunk()
                    else:
                        rest.append((k, ns, thunk))
                fillers[:] = rest

            def drain_class(cls):
                rest = []
                for k, ns, thunk in fillers:
                    if k[0] == cls:
                        thunk()
                    else:
                        rest.append((k, ns, thunk))
                fillers[:] = rest

            qkv_terms = ((0, 0), (0, 1), (1, 0))  # (x lo?, w lo?) per term

            def emit_qk_chain(tg, ct, j):
                tsl = slice(tg * 512, (tg + 1) * 512)
                dst, b_sb = ((QT, bq_sb), (KT, bk_sb))[j]
                ctj = ct * 2 + j
                ps = ps_b.tile([128, 512], f32, tag="b",
                               name=f"qk_{tg}_{ct}_{j}")
                for sub in range(2):
                    t0 = tg * 512 + sub * 256
                    i = 0
                    for xlo, wlo in qkv_terms:
                        xs = (xh, xl)[xlo]
                        ws = (wqkh, wqkl)[wlo]
                        for c in range(MO // 2):
                            nc.tensor.matmul(
                                ps[:, sub * 256 : sub * 256 + 256],
                                ws[:, ctj, 2 * c : 2 * c + 2, :],
                                xs[:, 2 * c : 2 * c + 2, t0 : t0 + 256],
                                start=(i == 0),
                                stop=(i == 11),
                                perf_mode=DR,
                            )
                            i += 1
                nc.vector.tensor_scalar_add(
                    dst[:, ct, tsl], ps[:], b_sb[:, ct : ct + 1]
                )

            def emit_v_chain(tg, ti):
                tt = tg * 4 + ti
                ps = ps_b.tile([128, 512], f32, tag="b", name=f"v_{tg}_{ti}")
                i = 0
                for xlo, wlo in qkv_terms:
                    xs = (xh, xl)[xlo]
                    ws = (wvh, wvl)[wlo]
                    for c in range(MO // 2):
                        nc.tensor.matmul(
                            ps[:, 0:CLOC],
                            xs[:, 2 * c : 2 * c + 2, tt * 128 : (tt + 1) * 128],
                            ws[:, 2 * c : 2 * c + 2, :],
                            start=(i == 0),
                            stop=(i == 11),
                            perf_mode=DR,
                        )
                        i += 1
                nc.vector.tensor_copy(
                    VA[:, tt, :, 0:64],
                    ps[:, 0:CLOC].rearrange("p (h d) -> p h d", d=64),
                )

            def emit_qkv(tg):
                for ct in range(2):
                    for j in range(2):
                        emit_qk_chain(tg, ct, j)
                for ti in range(4):
                    emit_v_chain(tg, ti)

            def push_qkv_fillers(tg):
                for ct in range(2):
                    for j in range(2):
                        fillers.append(
                            (("qkv", tg), 3072 * PE_NS,
                             lambda tg=tg, ct=ct, j=j: emit_qk_chain(tg, ct, j))
                        )
                vcls = "v" if defer_v else "qkv"
                for ti in range(4):
                    fillers.append(
                        ((vcls, tg), 1536 * PE_NS,
                         lambda tg=tg, ti=ti: emit_v_chain(tg, ti))
                    )

            # ---- attention ----
            def emit_sgrp(h, qg, gi, grp):
                """S^T matmuls for one exp group + the exp + diag masks."""
                hp = (h % 2) * 64
                ct = h // 2
                g0 = qg * 512
                cum = grp[-1][1] + grp[-1][2]
                sreg = ps_s.tile([128, sreg_w], f32, tag="s",
                                 name=f"s_{h}_{qg}_{gi}")
                for kt, off, w in grp:
                    q0 = g0 + 512 - w
                    c0 = off
                    while c0 < off + w:
                        cw = min(off + w - c0, 512 - c0 % 512)
                        nc.tensor.matmul(
                            sreg[:, c0 : c0 + cw],
                            KT[hp : hp + 64, ct, kt * 128 : (kt + 1) * 128],
                            QT[hp : hp + 64, ct,
                               q0 + c0 - off : q0 + c0 - off + cw],
                        )
                        c0 += cw
                pT = ppool.tile([128, sreg_w], bf16, tag="pT",
                                name=f"pT_{h}_{qg}_{gi}")
                no_diag = all(kt * 128 < g0 for kt, _, _ in grp)
                if gi < schr_groups[qg] and no_diag:
                    # Schraudolph exp on DVE: bf16 bits of exp(x*0.125) ~=
                    # int16(x*(0.125*128/ln2) + (127*128 - 5.6)); offloads
                    # the ACT engine (the attention-band pacer) at ~2% rms
                    # error on this group's P entries. sreg is 1024x (Q,K
                    # each carry the x32 weight prescale).
                    nc.vector.tensor_scalar(
                        pT[:, :cum].bitcast(mybir.dt.int16), sreg[:, :cum],
                        scalar1=23.0831253 / 1024.0, op0=mybir.AluOpType.mult,
                        scalar2=16250.4, op1=mybir.AluOpType.add)
                else:
                    nc.scalar.activation(pT[:, :cum], sreg[:, :cum], EXP,
                                         scale=0.125 / 1024.0)
                teng = nc.gpsimd if tri_engine == "gpsimd" else nc.vector
                for kt, off, w in grp:
                    if kt * 128 >= g0:  # diagonal block leads the span
                        teng.tensor_mul(
                            pT[:, off : off + 128],
                            pT[:, off : off + 128],
                            tri[:],
                        )
                return pT

            def emit_zchain(h, qg, qt, pts, kt2g, zp):
                """z[q,65] = sum_kt pT_chunk^T @ V_aug: one sequential PSUM
                accumulation chain per q-tile (a PSUM bank supports only one
                open accumulation group at a time)."""
                g0 = qg * 512
                qa = 4 * qg + qt
                for kt in range(qa + 1):
                    gi, off, w = kt2g[kt]
                    q0 = g0 + 512 - w
                    c0 = off + (g0 + qt * 128) - q0
                    nc.tensor.matmul(
                        zp[:, qt, 0:65],
                        pts[gi][:, c0 : c0 + 128],
                        VA[:, kt, h, 0:65],
                        start=(kt == 0),
                        stop=(kt == qa),
                    )

            def emit_norm(h, qg, zp, zq):
                """1/rowsum fused into the PSUM->SBUF copy of z."""
                hp = (h % 2) * 64
                rec = spool.tile([128, 4, 1], f32, tag="rec",
                                 name=f"rec_{h}_{qg}")
                nc.vector.reciprocal(rec[:], zp[:, :, 64:65])
                with nc.allow_low_precision(reason="attn out to bf16"):
                    for qt in range(4):
                        nc.vector.tensor_scalar_mul(
                            zq[:, qt, hp : hp + 64],
                            zp[:, qt, 0:64],
                            rec[:, qt, :],
                        )

            def emit_transpose(qg, pair, zq):
                """zq [q,128d] -> zT [128d, q]. All these transposes are
                latency-tolerant (op(qg) reads zT several us later), so they
                ride the DMA xbar instead of PE+DVE; only the tail-critical
                qg3/pair1 path (in the tailpipe) uses the PE transpose."""
                if dma_transpose:
                    for qt in range(4):
                        tt = qg * 4 + qt
                        nc.sync.dma_start_transpose(
                            zT[:, pair, tt * 128 : (tt + 1) * 128],
                            zq[:, qt, :],
                        )
                    return
                quad = ps_b.tile([128, 4, 128], bf16, tag="b",
                                 name=f"tq_{qg}_{pair}")
                for qt in range(4):
                    nc.tensor.transpose(quad[:, qt, :], zq[:, qt, :], ident)
                with nc.allow_low_precision(reason="zT copy"):
                    nc.vector.tensor_copy(
                        zT[:, pair, qg * 512 : (qg + 1) * 512],
                        quad[:].rearrange("p a b -> p (a b)"),
                    )

            def emit_op_half(qg, nh, ti, yA):
                tt = qg * 4 + ti
                ps = ps_b.tile([128, 512], f32, tag="b",
                               name=f"opA_{qg}_{nh}_{ti}")
                nc.tensor.matmul(
                    ps[:],
                    zT[:, 0, tt * 128 : (tt + 1) * 128],
                    wo[:, 0, nh * 512 : (nh + 1) * 512],
                )
                nc.vector.tensor_copy(yA[:, ti, :], ps[:])

            def emit_op_chunk(qg, nh, ti, ysb, dma_split, yA=None,
                              pool=None):
                tt = qg * 4 + ti
                # tail op chunks borrow ps_s (free once the last exp group
                # is consumed) so 4 chunks pipeline instead of 2
                ps = (pool or ps_b).tile([128, 512], f32,
                                         tag="b" if pool is None else "s",
                                         name=f"op_{qg}_{nh}_{ti}")
                cos = (1,) if yA is not None else (0, 1)
                for co in cos:
                    nc.tensor.matmul(
                        ps[:],
                        zT[:, co, tt * 128 : (tt + 1) * 128],
                        wo[:, co, nh * 512 : (nh + 1) * 512],
                        start=(co == cos[0]),
                        stop=(co == 1),
                    )
                with nc.allow_low_precision(reason="y partial to bf16"):
                    if yA is not None:
                        nc.vector.tensor_add(ysb[:, ti, :], ps[:],
                                             yA[:, ti, :])
                    elif qg == 3:
                        # tail: alternate engines by (ti+nh) so each q-tile's
                        # two chunk copies run on different engines in
                        # parallel instead of serializing on one
                        (nc.scalar.copy if (ti + nh) % 2 == 0
                         else nc.vector.tensor_copy)(ysb[:, ti, :], ps[:])
                    else:
                        ((nc.scalar.copy if (ti + nh) % 2 == 0
                          else nc.vector.tensor_copy) if alt_copy
                         else nc.vector.tensor_copy)(ysb[:, ti, :], ps[:])
                nper = 4 // dma_split
                if ti % nper == nper - 1:
                    t0 = tt - nper + 1
                    # y DMA queue: Pool/SWDGE keeps descriptor gen off the
                    # ACT/SP sequencers and HWDGE during the exp-critical
                    # band; in the qg3 tail (exp done) rotate across all
                    # three so gens parallelize
                    if qg == 3:
                        deng = (nc.gpsimd, nc.sync, nc.scalar)[
                            (ti // nper + nh * dma_split) % 3]
                    else:
                        deng = (nc.gpsimd, nc.sync)[(ti // nper + nh) % 2]
                    deng.dma_start(
                        y_d[t0 * 128 : (tt + 1) * 128,
                            nh * 512 : (nh + 1) * 512].rearrange(
                            "(ti p) n -> p ti n", p=128
                        ),
                        ysb[:, ti - nper + 1 : ti + 1, :],
                    )

            yA_box = {}

            def push_opA_fillers(qg):
                # first half (co=0) of qg3's out-proj: available right after
                # pair 0's transpose, staged to SBUF f32; the co=1 half plus
                # an add happens in the tail
                for nh in range(2):
                    yA = ypool.tile([128, 4, 512], f32, tag="yA", bufs=2,
                                    name=f"yA_{qg}_{nh}")
                    yA_box[(qg, nh)] = yA
                    for ti in range(4):
                        fillers.append(
                            (("op", qg), 512 * PE_NS,
                             lambda qg=qg, nh=nh, ti=ti, yA=yA:
                                 emit_op_half(qg, nh, ti, yA))
                        )

            def push_op_fillers(qg, split=False):
                dma_split = dma_splits[qg]
                for nh in range(2):
                    ysb = ypool.tile([128, 4, 512], bf16, tag="y",
                                     name=f"ysb_{qg}_{nh}")
                    yA = yA_box.get((qg, nh)) if split else None
                    for ti in range(4):
                        fillers.append(
                            (("op", qg), (512 if split else 1024) * PE_NS,
                             lambda qg=qg, nh=nh, ti=ti, ysb=ysb,
                                    ds=dma_split, yA=yA:
                                 emit_op_chunk(qg, nh, ti, ysb, ds, yA))
                        )

            def push_z_phase(qg, h, pts, kt2g, zq_box):
                """Queue head h's z chains + normalize (+ transpose) at the
                FRONT of the filler queue; they drain during head h+1's S
                phase (one-head software pipeline)."""
                box = {}
                # final unit (qg3 h3): per-q-tile norm -> transpose -> zT
                # copy -> that tile's out-proj chunks, so the tail pipelines
                # instead of serializing behind the full head
                tailpipe = qg == 3 and h == HLOC - 1

                def chain(qt):
                    if qt == 0:
                        box["zp"] = ps_z.tile([128, 4, 65], f32, tag="z",
                                              name=f"zp_{h}_{qg}")
                        if h % 2 == 0:
                            zq_box[h // 2] = zqpool.tile(
                                [128, 4, 128], bf16, tag="zq",
                                name=f"zq_{qg}_{h // 2}")
                    emit_zchain(h, qg, qt, pts, kt2g, box["zp"])
                    if tailpipe:
                        zq = zq_box[h // 2]
                        if qt == 0:
                            box["rec"] = spool.tile([128, 4, 1], f32,
                                                    tag="rec",
                                                    name=f"rec_{h}_{qg}")
                            box["ysb"] = [
                                ypool.tile([128, 4, 512], bf16, tag="y",
                                           name=f"ysb_3_{nh}")
                                for nh in range(2)
                            ]
                        rec = box["rec"]
                        zp = box["zp"]
                        nc.vector.reciprocal(rec[:, qt, :], zp[:, qt, 64:65])
                        with nc.allow_low_precision(reason="attn out bf16"):
                            nc.vector.tensor_scalar_mul(
                                zq[:, qt, 64:128], zp[:, qt, 0:64],
                                rec[:, qt, :])
                        quad = ps_b.tile([128, 128], bf16, tag="b",
                                         name=f"tq3_{qt}")
                        nc.tensor.transpose(quad[:], zq[:, qt, :], ident)
                        tt = qg * 4 + qt
                        with nc.allow_low_precision(reason="zT copy"):
                            nc.vector.tensor_copy(
                                zT[:, 1, tt * 128 : (tt + 1) * 128], quad[:])
                        for nh in range(2):
                            fillers.append(
                                (("op", 3), 1024 * PE_NS,
                                 lambda nh=nh, ti=qt:
                                     emit_op_chunk(3, nh, ti,
                                                   box["ysb"][nh],
                                                   dma_splits[3],
                                                   pool=ps_s))
                            )

                def norm():
                    if tailpipe:
                        return
                    emit_norm(h, qg, box["zp"], zq_box[h // 2])
                    if h % 2 == 1:
                        emit_transpose(qg, h // 2, zq_box[h // 2])
                        if h == 1 and qg == 3 and op3_split:
                            push_opA_fillers(qg)
                        if h == HLOC - 1:
                            # quarter finished: queue its out-proj (reads
                            # zT(qg), complete as of this point) and the
                            # next token group's QKV
                            push_op_fillers(qg, split=(qg == 3 and op3_split))
                            if qg + 2 <= 3:
                                push_qkv_fillers(qg + 2)

                # z thunks of heads <= h-2 must fully precede this head's
                # (ps_z has 2 bufs, so h-1's z may still be queued); h's
                # thunks append after h-1's so class-FIFO order holds
                prev = ("z", qg, h - 1)
                rest = []
                for k, ns, thunk in fillers:
                    if k[0] == "z" and k != prev:
                        thunk()
                    else:
                        rest.append((k, ns, thunk))
                fillers[:] = rest
                thunks = []
                for qt in range(4):
                    ncols = (4 * qg + qt + 1) * 65
                    thunks.append(
                        (("z", qg, h), ncols * PE_NS,
                         lambda qt=qt: chain(qt))
                    )
                thunks.append((("z", qg, h), 0.0, norm))
                idx = 0
                for i, (k, _, _) in enumerate(fillers):
                    if k[0] == "z":
                        idx = i + 1
                fillers[idx:idx] = thunks
                z_pushed_at[(qg, h)] = fill_count[0]

            # PE p-state warmup: the PE reaches full clock only after 3us
            # of continuous execution; burn the initial DMA-wait window
            # with dummy matmuls on zeroed scratch so the real chains run
            # at full speed from the start
            if warmup > 0:
                wsrc = apool.tile([128, 2, 512], bf16)
                nc.vector.memset(wsrc[:], 0.0)
                for i in range(warmup):
                    wps = ps_b.tile([128, 512], f32, tag="b",
                                    name=f"warm_{i}")
                    w = 512 if i < warmup - 2 else 256
                    nc.tensor.matmul(wps[:, 0:w], wsrc[:, 0, 0:128],
                                     wsrc[:, 1, 0:w])

            # ---- program ----
            # ct0's Q,K chains (heads 0,1) emit directly so quarter 0's S/exp
            # stream starts as soon as possible; ct1 + V chains become
            # fillers drained during h0/h1's exp (barriers: v at h==1,
            # qkv(ct1) at h==2).
            for j in range(2):
                emit_qk_chain(0, 0, j)
            for jj in range(2):
                fillers.append(
                    (("qkv", 0), 3072 * PE_NS,
                     lambda jj=jj: emit_qk_chain(0, 1, jj))
                )
            for ti in range(4):
                fillers.append(
                    (("v", 0), 1536 * PE_NS,
                     lambda ti=ti: emit_v_chain(0, ti))
                )
            push_qkv_fillers(1)

            zq_box = {}
            for qg in range(4):
                groups = _groups(qg, sreg_w)
                kt2g = {}
                for gi, grp in enumerate(groups):
                    for kt, off, w in grp:
                        kt2g[kt] = (gi, off, w)
                if qg > 0:
                    # barrier: this quarter's S/z read QT/KT/VA of tg=qg
                    drain(("qkv", qg))
                for h in range(HLOC):
                    if h == 1:
                        drain(("v", qg))
                    if h == 2 and qg == 0:
                        drain(("qkv", 0))  # ct1 chains gate heads 2,3
                    pts = []
                    # emit S groups in pairs (back-to-back on PE) so ACT's
                    # exp stream has at most one bubble per pair, not per
                    # group; the 2-buffer S rotation permits exactly 2 ahead
                    qsc, qpad = ((fill_scale, fill_pad) if fill_cfg is None
                                 else fill_cfg[qg])
                    pqg = pair_s if isinstance(pair_s, bool) else pair_s[qg]
                    step = 2 if pqg else 1
                    for g0i in range(0, len(groups), step):
                        pair = groups[g0i : g0i + step]
                        budget = 0.0
                        for gi, grp in zip(range(g0i, g0i + step), pair):
                            pts.append(emit_sgrp(h, qg, gi, grp))
                            cum = grp[-1][1] + grp[-1][2]
                            budget += ((cum * ACT_NS + 185.0) * qsc
                                       + qpad - cum * PE_NS)
                        fill(max(0.0, budget), op_ok=(qg >= op_from))
                    push_z_phase(qg, h, pts, kt2g, zq_box)

            # drain the tail (queue can grow while draining)
            while fillers:
                nxt = _pop_next(True)
                _, _, thunk = nxt if nxt else fillers.pop(0)
                thunk()

    nc.compile()
    return nc


def _pack_w(w):
    # [DM, C] -> [128, MO, C] f32: partition p holds rows {mo*128 + p}
    return np.ascontiguousarray(
        w.reshape(MO, 128, w.shape[1]).transpose(1, 0, 2)
    ).astype(np.float32)


def _split8(a):
    # f32 array -> (hi, lo) e4m3 pair with hi + lo ~= a to ~0.1%
    ah = a.astype(ml_dtypes.float8_e4m3)
    al = (a - ah.astype(np.float32)).astype(ml_dtypes.float8_e4m3)
    return np.ascontiguousarray(ah), np.ascontiguousarray(al)


def make_in_maps(x, w_qkv, b_qkv, w_out):
    # multiplicative post-exp mask: 1 where k <= q (upper incl diag), else 0
    tri = np.tri(128, 128, 0, dtype=np.float32).T.astype(ml_dtypes.bfloat16)
    ident = np.eye(128, dtype=np.float32).astype(ml_dtypes.bfloat16)
    in_maps = []
    for core in range(8):
        b = core // 4
        hg = core % 4
        c0 = hg * CLOC
        csl = slice(c0, c0 + CLOC)

        # packed consts: [128, 264] bf16-typed raw columns. Biases carry the
        # x32 weight prescale (Q,K live at 32x on device).
        cst = np.zeros((128, 264), np.uint16)
        cst[:, 0:128] = tri.view(np.uint16)
        cst[:, 128:256] = ident.view(np.uint16)
        bq = np.ascontiguousarray(
            32.0 * b_qkv[csl].astype(np.float32).reshape(2, 128).T
        )
        bk = np.ascontiguousarray(
            32.0 * b_qkv[DM + c0 : DM + c0 + CLOC].astype(np.float32)
            .reshape(2, 128).T
        )
        cst[:, 256:260] = bq.view(np.uint16).reshape(128, 4)
        cst[:, 260:264] = bk.view(np.uint16).reshape(128, 4)

        wq_p = _pack_w(32.0 * w_qkv[:, csl])
        wk_p = _pack_w(32.0 * w_qkv[:, DM + c0 : DM + c0 + CLOC])
        # [128, ctj, MO, 128]: ctj = ct*2 + j (j=0 -> Q, j=1 -> K)
        wqk = np.concatenate(
            [wq_p[:, None, :, 0:128], wk_p[:, None, :, 0:128],
             wq_p[:, None, :, 128:256], wk_p[:, None, :, 128:256]],
            axis=1,
        )
        wqkh, wqkl = _split8(wqk)
        wvh, wvl = _split8(
            _pack_w(32.0 * w_qkv[:, 2 * DM + c0 : 2 * DM + c0 + CLOC]))
        xTh, xTl = _split8(_pack_w(np.ascontiguousarray(x[b].T)))
        in_maps.append(
            {
                "xTh": xTh,
                "xTl": xTl,
                "wqkh": wqkh,
                "wqkl": wqkl,
                "wvh": wvh,
                "wvl": wvl,
                # wo: [CLOC, DM] -> [128, 2, DM]
                "wo": np.ascontiguousarray(
                    w_out[csl, :].reshape(2, 128, DM).transpose(1, 0, 2)
                ).astype(ml_dtypes.bfloat16),
                "cst": cst.view(ml_dtypes.bfloat16),
            }
        )
    return in_maps


def gather(results, b_qkv, w_out, b_out):
    # device skips the V bias; z_norm + b_v projects to a constant row:
    # y += b_v @ w_out, folded into the output bias here
    b_eff = (
        b_out.astype(np.float32)
        + b_qkv[2 * DM :].astype(np.float32) @ w_out.astype(np.float32)
    )
    out = np.empty((B, S, DM), np.float32)
    for b in range(B):
        acc = results[4 * b]["y"].astype(np.float32)
        for j in range(1, 4):
            acc = acc + results[4 * b + j]["y"]
        out[b] = acc + b_eff[None, :]
    return out


def kernel(x, w_qkv, b_qkv, w_out, b_out):
    x = np.asarray(x)
    w_qkv = np.asarray(w_qkv)
    b_qkv = np.asarray(b_qkv)
    w_out = np.asarray(w_out)
    b_out = np.asarray(b_out)

    if "nc" not in _CACHE:
        _CACHE["nc"] = build()
    nc = _CACHE["nc"]

    in_maps = make_in_maps(x, w_qkv, b_qkv, w_out)
    res = run_bass_kernel_spmd(nc, in_maps, core_ids=list(range(8)))
    return gather(res.results, b_qkv, w_out, b_out)



# revision 60
# speedup vs baseline: 1.1490x; 1.0023x over previous
"""Causal multi-head attention block (B=2, S=2048, D=1024, H=16) on 8 TRN2 cores.

Sharding: core i handles batch b = i//4 and head group hg = i%4 (4 heads =
256 model dims). Each core computes its heads' attention and a partial
output projection; the host sums the 4 partials per batch and adds b_out.

Per-core device pipeline (fp32 PSUM accumulation):
  1. QKV in compensated fp8: x and the QKV weights are hi/lo e4m3 pairs
     (hosts pre-scales weights x32 so e4m3's mantissa range is used;
     hi+lo carries ~11 mantissa bits, more precise than bf16). Each
     chain is xh@wh + xh@wl + xl@wh via DoubleRow matmuls (contraction
     256/matmul at 0.5 cycles/col = 3x bf16 throughput). Q^T,K^T land
     as [head_cols, tokens] at 32x scale (absorbed by the exp scale
     2^-13); V lands as [tokens, head_cols] at 32x, stored augmented
     with a 32.0 column so the z-matmul's row sums carry the same scale
     and the normalization cancels it exactly.
  2. Attention per head in bf16 (fp8 here fails the 2e-2 tolerance),
     flash-style in the S^T = K.Q^T orientation over the causal lower
     triangle only: S^T[k_tile, q_span] -> exp on ScalarE
     (scale=2^-13, no max subtraction; logits ~N(0,1)) -> P^T bf16 ->
     multiplicative 0/1 mask on diagonal blocks -> z[q_tile, 65]
     += P^T_chunk^T @ V_aug accumulated over k tiles in PSUM. The [q, d+1]
     z orientation makes each z matmul only 65 PE columns (vs a full
     q-span) and puts the softmax row sum in PSUM column 64 of the same
     partition as its query, so normalization is a per-partition
     tensor_scalar multiply fused into the PSUM->SBUF copy.
  3. z[q,d] tiles are transposed back to z^T[d,q] via PE transpose
     (identity matmul, 128 cols per 2-head tile) for the out-projection.
  4. Out-proj: y_partial[t, n] accumulated over the 256 local dims.

Program order is a fine-grained software pipeline: the attention loop is
a flat sequence over (q-quarter, head, k-group) with the z matmuls
lagging one group behind the S matmuls, and a filler queue (next token
group's QKV chains, previous quarters' out-proj chunks) drained between
S and z so the PE never waits on ScalarE exp. The final (ACT-bound)
quarter's last head runs a per-q-tile norm -> transpose -> out-proj
pipeline so the tail doesn't serialize, and y DMAs ride the Pool
engine's SWDGE path (+SP/ACT in the tail) to keep descriptor
generation off the exp-critical sequencers. Host pre-packs all inputs
into SBUF layouts; the V bias is folded into the output bias on the
host (b_v @ w_out).
"""

import numpy as np
import ml_dtypes

import concourse.mybir as mybir
import concourse.tile as tile
from concourse import bacc
from concourse.bass_utils import run_bass_kernel_spmd

B = 2
S = 2048
DM = 1024
HD = 64
HLOC = 4                 # heads per core
CLOC = HLOC * HD         # local model dims (256)
MO = DM // 128           # 8 k-subtiles of the model dim
NKT = S // 128           # 16 key tiles

f32 = mybir.dt.float32
bf16 = mybir.dt.bfloat16
f8 = mybir.dt.float8e4
DR = mybir.MatmulPerfMode.DoubleRow
EXP = mybir.ActivationFunctionType.Exp

ACT_NS = 0.8333333333333334
PE_NS = 0.4166666666666667

_CACHE = {}


def _groups(qg, cap=1024):
    """Pack the causal k-tile spans of query quarter qg into exp groups of
    <= cap columns. Returns list of groups; each group is a list of
    (kt, offset_in_group, width)."""
    g0 = qg * 512
    last_kt = 4 * qg + 3
    groups, cur, cum = [], [], 0
    for kt in range(last_kt + 1):
        w = g0 + 512 - max(kt * 128, g0)
        if cum + w > cap:
            groups.append(cur)
            cur, cum = [], 0
        cur.append((kt, cum, w))
        cum += w
    groups.append(cur)
    return groups


def build(pt_bufs=22, zq_bufs=4, y_bufs=4, sreg_w=1024, fill_scale=1.0,
          fill_pad=0.0, dma_splits=(2, 2, 4, 4), tri_engine="dve", op_from=3,
          zp_bufs=2, dma_transpose=True, warmup=8, alt_copy=False,
          z_delay=1, defer_v=True, copy_split=False, rr_pop=False,
          pair_s=(0, 0, 0, 1), op3_split=False, schr_groups=(0, 0, 0, 0),
          fill_cfg=((1.025, 53.0), (0.995, 56.0), (1.014, 5.0), (0.974, -100.0))):
    nc = bacc.Bacc("TRN2", target_bir_lowering=False, debug=False)

    # x and the QKV weights are fp8 e4m3 hi/lo pairs (weights pre-scaled x32
    # on the host so e4m3's mantissa is used; the x32 scales cancel in the
    # exp scale and the x32 ones-column). Q = xh@wh + xh@wl + xl@wh via
    # DoubleRow matmuls: contraction 256/matmul at 0.5 cycles/col.
    xh_d = nc.dram_tensor("xTh", [128, MO, S], f8, kind="ExternalInput")
    xl_d = nc.dram_tensor("xTl", [128, MO, S], f8, kind="ExternalInput")
    # wqk grouped per QKV chain (ctj = ct*2+j) so each chain's weights are
    # one contiguous DMA
    wqkh_d = nc.dram_tensor("wqkh", [128, 4, MO, 128], f8, kind="ExternalInput")
    wqkl_d = nc.dram_tensor("wqkl", [128, 4, MO, 128], f8, kind="ExternalInput")
    wvh_d = nc.dram_tensor("wvh", [128, MO, CLOC], f8, kind="ExternalInput")
    wvl_d = nc.dram_tensor("wvl", [128, MO, CLOC], f8, kind="ExternalInput")
    wo_d = nc.dram_tensor("wo", [128, 2, DM], bf16, kind="ExternalInput")
    # consts packed as raw bf16 columns: tri[0:128], identity[128:256],
    # bq[256:260], bk[260:264] (f32 values bit-split across bf16 pairs)
    cst_d = nc.dram_tensor("cst", [128, 264], bf16, kind="ExternalInput")
    y_d = nc.dram_tensor("y", [S, DM], bf16, kind="ExternalOutput")

    with tile.TileContext(nc) as tc:
        with (
            tc.tile_pool(name="consts", bufs=1) as consts,
            tc.tile_pool(name="acts", bufs=1) as apool,
            tc.tile_pool(name="pt", bufs=pt_bufs) as ppool,
            tc.tile_pool(name="zq", bufs=zq_bufs) as zqpool,
            tc.tile_pool(name="norm", bufs=4) as spool,
            tc.tile_pool(name="ycopy", bufs=y_bufs) as ypool,
            # 8 PSUM banks: ps_s 2x[128,1024]=4 (QK logits), ps_z
            # 2x[128,4,65]=2 (z accumulators), ps_b 2x[128,512]=2
            # (QKV / V / out-proj chains and z transposes)
            tc.tile_pool(name="ps_s", bufs=2, space="PSUM") as ps_s,
            tc.tile_pool(name="ps_z", bufs=zp_bufs, space="PSUM") as ps_z,
            tc.tile_pool(name="ps_b", bufs=2, space="PSUM") as ps_b,
        ):
            csb = consts.tile([128, 264], bf16)
            wqkh = consts.tile([128, 4, MO, 128], f8)
            wqkl = consts.tile([128, 4, MO, 128], f8)
            wvh = consts.tile([128, MO, CLOC], f8)
            wvl = consts.tile([128, MO, CLOC], f8)
            wo = consts.tile([128, 2, DM], bf16)
            xh = apool.tile([128, MO, S], f8)
            xl = apool.tile([128, MO, S], f8)

            # DMA order = consumption order (transfers serialize on the DMA
            # engines). Startup fans out over three queues (SP: weights,
            # ACT: xh, Pool/SWDGE: xl) so the first QK chain can start ~2.6us in;
            # bulk transfers stay off the ACT queue once the exp stream
            # starts (each dma_start occupies the sequencer ~660ns).
            nc.sync.dma_start(wqkh[:, 0, :, :], wqkh_d[:, 0, :, :])
            nc.scalar.dma_start(xh[:, 0:2, 0:512], xh_d[:, 0:2, 0:512])
            nc.gpsimd.dma_start(xl[:, 0:2, 0:512], xl_d[:, 0:2, 0:512])
            nc.sync.dma_start(wqkl[:, 0, :, :], wqkl_d[:, 0, :, :])
            nc.scalar.dma_start(xh[:, 2:4, 0:512], xh_d[:, 2:4, 0:512])
            nc.gpsimd.dma_start(xl[:, 2:4, 0:512], xl_d[:, 2:4, 0:512])
            nc.sync.dma_start(wqkh[:, 1, :, :], wqkh_d[:, 1, :, :])
            nc.scalar.dma_start(xh[:, 4:8, 0:512], xh_d[:, 4:8, 0:512])
            nc.gpsimd.dma_start(xl[:, 4:8, 0:512], xl_d[:, 4:8, 0:512])
            nc.sync.dma_start(csb[:], cst_d[:])
            nc.sync.dma_start(wqkl[:, 1, :, :], wqkl_d[:, 1, :, :])
            nc.sync.dma_start(wvh[:], wvh_d[:])
            nc.sync.dma_start(wvl[:], wvl_d[:])
            nc.sync.dma_start(wqkh[:, 2:4, :, :], wqkh_d[:, 2:4, :, :])
            nc.sync.dma_start(wqkl[:, 2:4, :, :], wqkl_d[:, 2:4, :, :])
            nc.sync.dma_start(xh[:, :, 512:1024], xh_d[:, :, 512:1024])
            nc.sync.dma_start(xl[:, :, 512:1024], xl_d[:, :, 512:1024])
            nc.sync.dma_start(xh[:, :, 1024:1536], xh_d[:, :, 1024:1536])
            nc.sync.dma_start(xl[:, :, 1024:1536], xl_d[:, :, 1024:1536])
            nc.sync.dma_start(xh[:, :, 1536:2048], xh_d[:, :, 1536:2048])
            nc.sync.dma_start(xl[:, :, 1536:2048], xl_d[:, :, 1536:2048])
            nc.sync.dma_start(wo[:], wo_d[:])

            tri = csb[:, 0:128]
            ident = csb[:, 128:256]
            bq_sb = csb[:, 256:260].bitcast(f32)
            bk_sb = csb[:, 260:264].bitcast(f32)

            QT = apool.tile([128, 2, S], bf16)
            KT = apool.tile([128, 2, S], bf16)
            # V augmented: [t-part, kt, h, 0:64] = v dims (x32), col 64 = 32
            # so the rowsum scale matches the v columns and the x32 cancels
            # in the normalization
            VA = apool.tile([128, NKT, HLOC, 72], bf16)
            nc.vector.memset(VA[:, :, :, 64:65], 32.0)
            zT = apool.tile([128, 2, S], bf16)

            # ---- filler queue: PE work units drained while ScalarE exps ----
            fillers = []          # list of (key, pe_ns, thunk)
            fill_debt = [0.0]
            fill_count = [0]      # fill() invocations, for z-pop delay
            z_pushed_at = {}      # (qg, h) -> fill_count at push
            rr_state = [0]

            def _z_ok(k):
                return (fill_count[0] - z_pushed_at.get((k[1], k[2]), -(1 << 30))
                        >= z_delay)

            cur_qg = [0]

            def _pop_next(op_ok=True):
                """z thunks once ScalarE has had time to produce their exp
                inputs (z_delay fill periods after push), then QKV (needed
                by the next quarter anyway), out-proj last. op thunks are
                hoarded for the late ACT-bound quarters (qg >= op_from),
                where they are the only filler class left."""
                for cls in ("z", "v", "qkv"):
                    for i, (k, ns, thunk) in enumerate(fillers):
                        if k[0] == cls and (cls != "z" or _z_ok(k)):
                            return fillers.pop(i)
                if op_ok:
                    for i, (k, ns, thunk) in enumerate(fillers):
                        if k[0] == "op":
                            return fillers.pop(i)
                    return fillers.pop(0) if fillers else None
                return None

            def fill(budget_ns, op_ok=True):
                fill_count[0] += 1
                budget = budget_ns + fill_debt[0]
                spent = 0.0
                while fillers and spent < budget:
                    nxt = _pop_next(op_ok)
                    if nxt is None:
                        # only hoarded op (or unready z) left: stop without
                        # banking debt so qg2's first fill doesn't burst
                        fill_debt[0] = 0.0
                        return
                    _, ns, thunk = nxt
                    thunk()
                    spent += ns
                fill_debt[0] = budget - spent if fillers else 0.0

            def drain(key):
                """Force-emit queued fillers matching key (dependency
                barrier: attention on quarter qg needs all of QKV(tg=qg))."""
                rest = []
                for k, ns, thunk in fillers:
                    if k == key:
                        thunk()
                    else:
                        rest.append((k, ns, thunk))
                fillers[:] = rest

            def drain_class(cls):
                rest = []
                for k, ns, thunk in fillers:
                    if k[0] == cls:
                        thunk()
                    else:
                        rest.append((k, ns, thunk))
                fillers[:] = rest

            qkv_terms = ((0, 0), (0, 1), (1, 0))  # (x lo?, w lo?) per term

            def emit_qk_chain(tg, ct, j):
                tsl = slice(tg * 512, (tg + 1) * 512)
                dst, b_sb = ((QT, bq_sb), (KT, bk_sb))[j]
                ctj = ct * 2 + j
                ps = ps_b.tile([128, 512], f32, tag="b",
                               name=f"qk_{tg}_{ct}_{j}")
                for sub in range(2):
                    t0 = tg * 512 + sub * 256
                    i = 0
                    for xlo, wlo in qkv_terms:
                        xs = (xh, xl)[xlo]
                        ws = (wqkh, wqkl)[wlo]
                        for c in range(MO // 2):
                            nc.tensor.matmul(
                                ps[:, sub * 256 : sub * 256 + 256],
                                ws[:, ctj, 2 * c : 2 * c + 2, :],
                                xs[:, 2 * c : 2 * c + 2, t0 : t0 + 256],
                                start=(i == 0),
                                stop=(i == 11),
                                perf_mode=DR,
                            )
                            i += 1
                nc.vector.tensor_scalar_add(
                    dst[:, ct, tsl], ps[:], b_sb[:, ct : ct + 1]
                )

            def emit_v_chain(tg, ti):
                tt = tg * 4 + ti
                ps = ps_b.tile([128, 512], f32, tag="b", name=f"v_{tg}_{ti}")
                i = 0
                for xlo, wlo in qkv_terms:
                    xs = (xh, xl)[xlo]
                    ws = (wvh, wvl)[wlo]
                    for c in range(MO // 2):
                        nc.tensor.matmul(
                            ps[:, 0:CLOC],
                            xs[:, 2 * c : 2 * c + 2, tt * 128 : (tt + 1) * 128],
                            ws[:, 2 * c : 2 * c + 2, :],
                            start=(i == 0),
                            stop=(i == 11),
                            perf_mode=DR,
                        )
                        i += 1
                nc.vector.tensor_copy(
                    VA[:, tt, :, 0:64],
                    ps[:, 0:CLOC].rearrange("p (h d) -> p h d", d=64),
                )

            def emit_qkv(tg):
                for ct in range(2):
                    for j in range(2):
                        emit_qk_chain(tg, ct, j)
                for ti in range(4):
                    emit_v_chain(tg, ti)

            def push_qkv_fillers(tg):
                for ct in range(2):
                    for j in range(2):
                        fillers.append(
                            (("qkv", tg), 3072 * PE_NS,
                             lambda tg=tg, ct=ct, j=j: emit_qk_chain(tg, ct, j))
                        )
                vcls = "v" if defer_v else "qkv"
                for ti in range(4):
                    fillers.append(
                        ((vcls, tg), 1536 * PE_NS,
                         lambda tg=tg, ti=ti: emit_v_chain(tg, ti))
                    )

            # ---- attention ----
            def emit_sgrp(h, qg, gi, grp):
                """S^T matmuls for one exp group + the exp + diag masks."""
                hp = (h % 2) * 64
                ct = h // 2
                g0 = qg * 512
                cum = grp[-1][1] + grp[-1][2]
                sreg = ps_s.tile([128, sreg_w], f32, tag="s",
                                 name=f"s_{h}_{qg}_{gi}")
                for kt, off, w in grp:
                    q0 = g0 + 512 - w
                    c0 = off
                    while c0 < off + w:
                        cw = min(off + w - c0, 512 - c0 % 512)
                        nc.tensor.matmul(
                            sreg[:, c0 : c0 + cw],
                            KT[hp : hp + 64, ct, kt * 128 : (kt + 1) * 128],
                            QT[hp : hp + 64, ct,
                               q0 + c0 - off : q0 + c0 - off + cw],
                        )
                        c0 += cw
                pT = ppool.tile([128, sreg_w], bf16, tag="pT",
                                name=f"pT_{h}_{qg}_{gi}")
                no_diag = all(kt * 128 < g0 for kt, _, _ in grp)
                if gi < schr_groups[qg] and no_diag:
                    # Schraudolph exp on DVE: bf16 bits of exp(x*0.125) ~=
                    # int16(x*(0.125*128/ln2) + (127*128 - 5.6)); offloads
                    # the ACT engine (the attention-band pacer) at ~2% rms
                    # error on this group's P entries. sreg is 1024x (Q,K
                    # each carry the x32 weight prescale).
                    nc.vector.tensor_scalar(
                        pT[:, :cum].bitcast(mybir.dt.int16), sreg[:, :cum],
                        scalar1=23.0831253 / 1024.0, op0=mybir.AluOpType.mult,
                        scalar2=16250.4, op1=mybir.AluOpType.add)
                else:
                    nc.scalar.activation(pT[:, :cum], sreg[:, :cum], EXP,
                                         scale=0.125 / 1024.0)
                teng = nc.gpsimd if tri_engine == "gpsimd" else nc.vector
                for kt, off, w in grp:
                    if kt * 128 >= g0:  # diagonal block leads the span
                        teng.tensor_mul(
                            pT[:, off : off + 128],
                            pT[:, off : off + 128],
                            tri[:],
                        )
                return pT

            def emit_zchain(h, qg, qt, pts, kt2g, zp):
                """z[q,65] = sum_kt pT_chunk^T @ V_aug: one sequential PSUM
                accumulation chain per q-tile (a PSUM bank supports only one
                open accumulation group at a time)."""
                g0 = qg * 512
                qa = 4 * qg + qt
                for kt in range(qa + 1):
                    gi, off, w = kt2g[kt]
                    q0 = g0 + 512 - w
                    c0 = off + (g0 + qt * 128) - q0
                    nc.tensor.matmul(
                        zp[:, qt, 0:65],
                        pts[gi][:, c0 : c0 + 128],
                        VA[:, kt, h, 0:65],
                        start=(kt == 0),
                        stop=(kt == qa),
                    )

            def emit_norm(h, qg, zp, zq):
                """1/rowsum fused into the PSUM->SBUF copy of z."""
                hp = (h % 2) * 64
                rec = spool.tile([128, 4, 1], f32, tag="rec",
                                 name=f"rec_{h}_{qg}")
                nc.vector.reciprocal(rec[:], zp[:, :, 64:65])
                with nc.allow_low_precision(reason="attn out to bf16"):
                    for qt in range(4):
                        nc.vector.tensor_scalar_mul(
                            zq[:, qt, hp : hp + 64],
                            zp[:, qt, 0:64],
                            rec[:, qt, :],
                        )

            def emit_transpose(qg, pair, zq):
                """zq [q,128d] -> zT [128d, q]. All these transposes are
                latency-tolerant (op(qg) reads zT several us later), so they
                ride the DMA xbar instead of PE+DVE; only the tail-critical
                qg3/pair1 path (in the tailpipe) uses the PE transpose."""
                if dma_transpose:
                    for qt in range(4):
                        tt = qg * 4 + qt
                        nc.sync.dma_start_transpose(
                            zT[:, pair, tt * 128 : (tt + 1) * 128],
                            zq[:, qt, :],
                        )
                    return
                quad = ps_b.tile([128, 4, 128], bf16, tag="b",
                                 name=f"tq_{qg}_{pair}")
                for qt in range(4):
                    nc.tensor.transpose(quad[:, qt, :], zq[:, qt, :], ident)
                with nc.allow_low_precision(reason="zT copy"):
                    nc.vector.tensor_copy(
                        zT[:, pair, qg * 512 : (qg + 1) * 512],
                        quad[:].rearrange("p a b -> p (a b)"),
                    )

            def emit_op_half(qg, nh, ti, yA):
                tt = qg * 4 + ti
                ps = ps_b.tile([128, 512], f32, tag="b",
                               name=f"opA_{qg}_{nh}_{ti}")
                nc.tensor.matmul(
                    ps[:],
                    zT[:, 0, tt * 128 : (tt + 1) * 128],
                    wo[:, 0, nh * 512 : (nh + 1) * 512],
                )
                nc.vector.tensor_copy(yA[:, ti, :], ps[:])

            def emit_op_chunk(qg, nh, ti, ysb, dma_split, yA=None,
                              pool=None):
                tt = qg * 4 + ti
                # tail op chunks borrow ps_s (free once the last exp group
                # is consumed) so 4 chunks pipeline instead of 2
                ps = (pool or ps_b).tile([128, 512], f32,
                                         tag="b" if pool is None else "s",
                                         name=f"op_{qg}_{nh}_{ti}")
                cos = (1,) if yA is not None else (0, 1)
                for co in cos:
                    nc.tensor.matmul(
                        ps[:],
                        zT[:, co, tt * 128 : (tt + 1) * 128],
                        wo[:, co, nh * 512 : (nh + 1) * 512],
                        start=(co == cos[0]),
                        stop=(co == 1),
                    )
                with nc.allow_low_precision(reason="y partial to bf16"):
                    if yA is not None:
                        nc.vector.tensor_add(ysb[:, ti, :], ps[:],
                                             yA[:, ti, :])
                    elif qg == 3:
                        # tail: alternate engines by (ti+nh) so each q-tile's
                        # two chunk copies run on different engines in
                        # parallel instead of serializing on one
                        (nc.scalar.copy if (ti + nh) % 2 == 0
                         else nc.vector.tensor_copy)(ysb[:, ti, :], ps[:])
                    else:
                        ((nc.scalar.copy if (ti + nh) % 2 == 0
                          else nc.vector.tensor_copy) if alt_copy
                         else nc.vector.tensor_copy)(ysb[:, ti, :], ps[:])
                nper = 4 // dma_split
                if ti % nper == nper - 1:
                    t0 = tt - nper + 1
                    # y DMA queue: Pool/SWDGE keeps descriptor gen off the
                    # ACT/SP sequencers and HWDGE during the exp-critical
                    # band; in the qg3 tail (exp done) rotate across all
                    # three so gens parallelize
                    if qg == 3:
                        deng = (nc.gpsimd, nc.sync, nc.scalar)[
                            (ti // nper + nh * dma_split) % 3]
                    else:
                        deng = (nc.gpsimd, nc.sync)[(ti // nper + nh) % 2]
                    deng.dma_start(
                        y_d[t0 * 128 : (tt + 1) * 128,
                            nh * 512 : (nh + 1) * 512].rearrange(
                            "(ti p) n -> p ti n", p=128
                        ),
                        ysb[:, ti - nper + 1 : ti + 1, :],
                    )

            yA_box = {}

            def push_opA_fillers(qg):
                # first half (co=0) of qg3's out-proj: available right after
                # pair 0's transpose, staged to SBUF f32; the co=1 half plus
                # an add happens in the tail
                for nh in range(2):
                    yA = ypool.tile([128, 4, 512], f32, tag="yA", bufs=2,
                                    name=f"yA_{qg}_{nh}")
                    yA_box[(qg, nh)] = yA
                    for ti in range(4):
                        fillers.append(
                            (("op", qg), 512 * PE_NS,
                             lambda qg=qg, nh=nh, ti=ti, yA=yA:
                                 emit_op_half(qg, nh, ti, yA))
                        )

            def push_op_fillers(qg, split=False):
                dma_split = dma_splits[qg]
                for nh in range(2):
                    ysb = ypool.tile([128, 4, 512], bf16, tag="y",
                                     name=f"ysb_{qg}_{nh}")
                    yA = yA_box.get((qg, nh)) if split else None
                    for ti in range(4):
                        fillers.append(
                            (("op", qg), (512 if split else 1024) * PE_NS,
                             lambda qg=qg, nh=nh, ti=ti, ysb=ysb,
                                    ds=dma_split, yA=yA:
                                 emit_op_chunk(qg, nh, ti, ysb, ds, yA))
                        )

            def push_z_phase(qg, h, pts, kt2g, zq_box):
                """Queue head h's z chains + normalize (+ transpose) at the
                FRONT of the filler queue; they drain during head h+1's S
                phase (one-head software pipeline)."""
                box = {}
                # final unit (qg3 h3): per-q-tile norm -> transpose -> zT
                # copy -> that tile's out-proj chunks, so the tail pipelines
                # instead of serializing behind the full head
                tailpipe = qg == 3 and h == HLOC - 1

                def chain(qt):
                    if qt == 0:
                        box["zp"] = ps_z.tile([128, 4, 65], f32, tag="z",
                                              name=f"zp_{h}_{qg}")
                        if h % 2 == 0:
                            zq_box[h // 2] = zqpool.tile(
                                [128, 4, 128], bf16, tag="zq",
                                name=f"zq_{qg}_{h // 2}")
                    emit_zchain(h, qg, qt, pts, kt2g, box["zp"])
                    if tailpipe:
                        zq = zq_box[h // 2]
                        if qt == 0:
                            box["rec"] = spool.tile([128, 4, 1], f32,
                                                    tag="rec",
                                                    name=f"rec_{h}_{qg}")
                            box["ysb"] = [
                                ypool.tile([128, 4, 512], bf16, tag="y",
                                           name=f"ysb_3_{nh}")
                                for nh in range(2)
                            ]
                        rec = box["rec"]
                        zp = box["zp"]
                        nc.vector.reciprocal(rec[:, qt, :], zp[:, qt, 64:65])
                        with nc.allow_low_precision(reason="attn out bf16"):
                            nc.vector.tensor_scalar_mul(
                                zq[:, qt, 64:128], zp[:, qt, 0:64],
                                rec[:, qt, :])
                        quad = ps_b.tile([128, 128], bf16, tag="b",
                                         name=f"tq3_{qt}")
                        nc.tensor.transpose(quad[:], zq[:, qt, :], ident)
                        tt = qg * 4 + qt
                        with nc.allow_low_precision(reason="zT copy"):
                            nc.vector.tensor_copy(
                                zT[:, 1, tt * 128 : (tt + 1) * 128], quad[:])
                        for nh in range(2):
                            fillers.append(
                                (("op", 3), 1024 * PE_NS,
                                 lambda nh=nh, ti=qt:
                                     emit_op_chunk(3, nh, ti,
                                                   box["ysb"][nh],
                                                   dma_splits[3],
                                                   pool=ps_s))
                            )

                def norm():
                    if tailpipe:
                        return
                    emit_norm(h, qg, box["zp"], zq_box[h // 2])
                    if h % 2 == 1:
                        emit_transpose(qg, h // 2, zq_box[h // 2])
                        if h == 1 and qg == 3 and op3_split:
                            push_opA_fillers(qg)
                        if h == HLOC - 1:
                            # quarter finished: queue its out-proj (reads
                            # zT(qg), complete as of this point) and the
                            # next token group's QKV
                            push_op_fillers(qg, split=(qg == 3 and op3_split))
                            if qg + 2 <= 3:
                                push_qkv_fillers(qg + 2)

                # z thunks of heads <= h-2 must fully precede this head's
                # (ps_z has 2 bufs, so h-1's z may still be queued); h's
                # thunks append after h-1's so class-FIFO order holds
                prev = ("z", qg, h - 1)
                rest = []
                for k, ns, thunk in fillers:
                    if k[0] == "z" and k != prev:
                        thunk()
                    else:
                        rest.append((k, ns, thunk))
                fillers[:] = rest
                thunks = []
                for qt in range(4):
                    ncols = (4 * qg + qt + 1) * 65
                    thunks.append(
                        (("z", qg, h), ncols * PE_NS,
                         lambda qt=qt: chain(qt))
                    )
                thunks.append((("z", qg, h), 0.0, norm))
                idx = 0
                for i, (k, _, _) in enumerate(fillers):
                    if k[0] == "z":
                        idx = i + 1
                fillers[idx:idx] = thunks
                z_pushed_at[(qg, h)] = fill_count[0]

            # PE p-state warmup: the PE reaches full clock only after 3us
            # of continuous execution; burn the initial DMA-wait window
            # with dummy matmuls on zeroed scratch so the real chains run
            # at full speed from the start
            if warmup > 0:
                wsrc = apool.tile([128, 2, 512], bf16)
                nc.vector.memset(wsrc[:], 0.0)
                for i in range(warmup):
                    wps = ps_b.tile([128, 512], f32, tag="b",
                                    name=f"warm_{i}")
                    w = 512 if i < warmup - 2 else 256
                    nc.tensor.matmul(wps[:, 0:w], wsrc[:, 0, 0:128],
                                     wsrc[:, 1, 0:w])

            # ---- program ----
            # ct0's Q,K chains (heads 0,1) emit directly so quarter 0's S/exp
            # stream starts as soon as possible; ct1 + V chains become
            # fillers drained during h0/h1's exp (barriers: v at h==1,
            # qkv(ct1) at h==2).
            for j in range(2):
                emit_qk_chain(0, 0, j)
            for jj in range(2):
                fillers.append(
                    (("qkv", 0), 3072 * PE_NS,
                     lambda jj=jj: emit_qk_chain(0, 1, jj))
                )
            for ti in range(4):
                fillers.append(
                    (("v", 0), 1536 * PE_NS,
                     lambda ti=ti: emit_v_chain(0, ti))
                )
            push_qkv_fillers(1)

            zq_box = {}
            for qg in range(4):
                groups = _groups(qg, sreg_w)
                kt2g = {}
                for gi, grp in enumerate(groups):
                    for kt, off, w in grp:
                        kt2g[kt] = (gi, off, w)
                if qg > 0:
                    # barrier: this quarter's S/z read QT/KT/VA of tg=qg
                    drain(("qkv", qg))
                for h in range(HLOC):
                    if h == 1:
                        drain(("v", qg))
                    if h == 2 and qg == 0:
                        drain(("qkv", 0))  # ct1 chains gate heads 2,3
                    pts = []
                    # emit S groups in pairs (back-to-back on PE) so ACT's
                    # exp stream has at most one bubble per pair, not per
                    # group; the 2-buffer S rotation permits exactly 2 ahead
                    qsc, qpad = ((fill_scale, fill_pad) if fill_cfg is None
                                 else fill_cfg[qg])
                    pqg = pair_s if isinstance(pair_s, bool) else pair_s[qg]
                    step = 2 if pqg else 1
                    for g0i in range(0, len(groups), step):
                        pair = groups[g0i : g0i + step]
                        budget = 0.0
                        for gi, grp in zip(range(g0i, g0i + step), pair):
                            pts.append(emit_sgrp(h, qg, gi, grp))
                            cum = grp[-1][1] + grp[-1][2]
                            budget += ((cum * ACT_NS + 185.0) * qsc
                                       + qpad - cum * PE_NS)
                        fill(max(0.0, budget), op_ok=(qg >= op_from))
                    push_z_phase(qg, h, pts, kt2g, zq_box)

            # drain the tail (queue can grow while draining)
            while fillers:
                nxt = _pop_next(True)
                _, _, thunk = nxt if nxt else fillers.pop(0)
                thunk()

    nc.compile()
    return nc


def _pack_w(w):
    # [DM, C] -> [128, MO, C] f32: partition p holds rows {mo*128 + p}
    return np.ascontiguousarray(
        w.reshape(MO, 128, w.shape[1]).transpose(1, 0, 2)
    ).astype(np.float32)


def _split8(a):
    # f32 array -> (hi, lo) e4m3 pair with hi + lo ~= a to ~0.1%
    ah = a.astype(ml_dtypes.float8_e4m3)
    al = (a - ah.astype(np.float32)).astype(ml_dtypes.float8_e4m3)
    return np.ascontiguousarray(ah), np.ascontiguousarray(al)


def make_in_maps(x, w_qkv, b_qkv, w_out):
    # multiplicative post-exp mask: 1 where k <= q (upper incl diag), else 0
    tri = np.tri(128, 128, 0, dtype=np.float32).T.astype(ml_dtypes.bfloat16)
    ident = np.eye(128, dtype=np.float32).astype(ml_dtypes.bfloat16)
    in_maps = []
    for core in range(8):
        b = core // 4
        hg = core % 4
        c0 = hg * CLOC
        csl = slice(c0, c0 + CLOC)

        # packed consts: [128, 264] bf16-typed raw columns. Biases carry the
        # x32 weight prescale (Q,K live at 32x on device).
        cst = np.zeros((128, 264), np.uint16)
        cst[:, 0:128] = tri.view(np.uint16)
        cst[:, 128:256] = ident.view(np.uint16)
        bq = np.ascontiguousarray(
            32.0 * b_qkv[csl].astype(np.float32).reshape(2, 128).T
        )
        bk = np.ascontiguousarray(
            32.0 * b_qkv[DM + c0 : DM + c0 + CLOC].astype(np.float32)
            .reshape(2, 128).T
        )
        cst[:, 256:260] = bq.view(np.uint16).reshape(128, 4)
        cst[:, 260:264] = bk.view(np.uint16).reshape(128, 4)

        wq_p = _pack_w(32.0 * w_qkv[:, csl])
        wk_p = _pack_w(32.0 * w_qkv[:, DM + c0 : DM + c0 + CLOC])
        # [128, ctj, MO, 128]: ctj = ct*2 + j (j=0 -> Q, j=1 -> K)
        wqk = np.concatenate(
            [wq_p[:, None, :, 0:128], wk_p[:, None, :, 0:128],
             wq_p[:, None, :, 128:256], wk_p[:, None, :, 128:256]],
            axis=1,
        )
        wqkh, wqkl = _split8(wqk)
        wvh, wvl = _split8(
            _pack_w(32.0 * w_qkv[:, 2 * DM + c0 : 2 * DM + c0 + CLOC]))
        xTh, xTl = _split8(_pack_w(np.ascontiguousarray(x[b].T)))
        in_maps.append(
            {
                "xTh": xTh,
                "xTl": xTl,
                "wqkh": wqkh,
                "wqkl": wqkl,
                "wvh": wvh,
                "wvl": wvl,
                # wo: [CLOC, DM] -> [128, 2, DM]
                "wo": np.ascontiguousarray(
                    w_out[csl, :].reshape(2, 128, DM).transpose(1, 0, 2)
                ).astype(ml_dtypes.bfloat16),
                "cst": cst.view(ml_dtypes.bfloat16),
            }
        )
    return in_maps


def gather(results, b_qkv, w_out, b_out):
    # device skips the V bias; z_norm + b_v projects to a constant row:
    # y += b_v @ w_out, folded into the output bias here
    b_eff = (
        b_out.astype(np.float32)
        + b_qkv[2 * DM :].astype(np.float32) @ w_out.astype(np.float32)
    )
    out = np.empty((B, S, DM), np.float32)
    for b in range(B):
        acc = results[4 * b]["y"].astype(np.float32)
        for j in range(1, 4):
            acc = acc + results[4 * b + j]["y"]
        out[b] = acc + b_eff[None, :]
    return out


def kernel(x, w_qkv, b_qkv, w_out, b_out):
    x = np.asarray(x)
    w_qkv = np.asarray(w_qkv)
    b_qkv = np.asarray(b_qkv)
    w_out = np.asarray(w_out)
    b_out = np.asarray(b_out)

    if "nc" not in _CACHE:
        _CACHE["nc"] = build()
    nc = _CACHE["nc"]

    in_maps = make_in_maps(x, w_qkv, b_qkv, w_out)
    res = run_bass_kernel_spmd(nc, in_maps, core_ids=list(range(8)))
    return gather(res.results, b_qkv, w_out, b_out)



# revision 63
# speedup vs baseline: 1.1495x; 1.0004x over previous
"""Causal multi-head attention block (B=2, S=2048, D=1024, H=16) on 8 TRN2 cores.

Sharding: core i handles batch b = i//4 and head group hg = i%4 (4 heads =
256 model dims). Each core computes its heads' attention and a partial
output projection; the host sums the 4 partials per batch and adds b_out.

Per-core device pipeline (fp32 PSUM accumulation):
  1. QKV in compensated fp8: x and the QKV weights are hi/lo e4m3 pairs
     (hosts pre-scales weights x32 so e4m3's mantissa range is used;
     hi+lo carries ~11 mantissa bits, more precise than bf16). Each
     chain is xh@wh + xh@wl + xl@wh via DoubleRow matmuls (contraction
     256/matmul at 0.5 cycles/col = 3x bf16 throughput). Q^T,K^T land
     as [head_cols, tokens] at 32x scale (absorbed by the exp scale
     2^-13); V lands as [tokens, head_cols] at 32x, stored augmented
     with a 32.0 column so the z-matmul's row sums carry the same scale
     and the normalization cancels it exactly.
  2. Attention per head in bf16 (fp8 here fails the 2e-2 tolerance),
     flash-style in the S^T = K.Q^T orientation over the causal lower
     triangle only: S^T[k_tile, q_span] -> exp on ScalarE
     (scale=2^-13, no max subtraction; logits ~N(0,1)) -> P^T bf16 ->
     multiplicative 0/1 mask on diagonal blocks -> z[q_tile, 65]
     += P^T_chunk^T @ V_aug accumulated over k tiles in PSUM. The [q, d+1]
     z orientation makes each z matmul only 65 PE columns (vs a full
     q-span) and puts the softmax row sum in PSUM column 64 of the same
     partition as its query, so normalization is a per-partition
     tensor_scalar multiply fused into the PSUM->SBUF copy.
  3. z[q,d] tiles are transposed back to z^T[d,q] via PE transpose
     (identity matmul, 128 cols per 2-head tile) for the out-projection.
  4. Out-proj: y_partial[t, n] accumulated over the 256 local dims.

Program order is a fine-grained software pipeline: the attention loop is
a flat sequence over (q-quarter, head, k-group) with the z matmuls
lagging one group behind the S matmuls, and a filler queue (next token
group's QKV chains, previous quarters' out-proj chunks) drained between
S and z so the PE never waits on ScalarE exp. The final (ACT-bound)
quarter's last head runs a per-q-tile norm -> transpose -> out-proj
pipeline so the tail doesn't serialize, and y DMAs ride the Pool
engine's SWDGE path (+SP/ACT in the tail) to keep descriptor
generation off the exp-critical sequencers. Host pre-packs all inputs
into SBUF layouts; the V bias is folded into the output bias on the
host (b_v @ w_out).
"""

import numpy as np
import ml_dtypes

import concourse.mybir as mybir
import concourse.tile as tile
from concourse import bacc
from concourse.bass_utils import run_bass_kernel_spmd

B = 2
S = 2048
DM = 1024
HD = 64
HLOC = 4                 # heads per core
CLOC = HLOC * HD         # local model dims (256)
MO = DM // 128           # 8 k-subtiles of the model dim
NKT = S // 128           # 16 key tiles

f32 = mybir.dt.float32
bf16 = mybir.dt.bfloat16
f8 = mybir.dt.float8e4
DR = mybir.MatmulPerfMode.DoubleRow
EXP = mybir.ActivationFunctionType.Exp

ACT_NS = 0.8333333333333334
PE_NS = 0.4166666666666667

_CACHE = {}


def _groups(qg, cap=1024):
    """Pack the causal k-tile spans of query quarter qg into exp groups of
    <= cap columns. Returns list of groups; each group is a list of
    (kt, offset_in_group, width)."""
    g0 = qg * 512
    last_kt = 4 * qg + 3
    groups, cur, cum = [], [], 0
    for kt in range(last_kt + 1):
        w = g0 + 512 - max(kt * 128, g0)
        if cum + w > cap:
            groups.append(cur)
            cur, cum = [], 0
        cur.append((kt, cum, w))
        cum += w
    groups.append(cur)
    return groups


def build(pt_bufs=22, zq_bufs=4, y_bufs=4, sreg_w=1024, fill_scale=1.0,
          fill_pad=0.0, dma_splits=(2, 2, 4, 4), tri_engine="dve", op_from=3,
          zp_bufs=2, dma_transpose=True, warmup=8, alt_copy=False,
          z_delay=1, defer_v=True, copy_split=False, rr_pop=False,
          pair_s=(0, 0, 0, 1), op3_split=False, schr_groups=(0, 0, 0, 0),
          fill_cfg=((1.025, 53.0), (0.995, 56.0), (1.014, 5.0), (0.974, -100.0))):
    nc = bacc.Bacc("TRN2", target_bir_lowering=False, debug=False)

    # x and the QKV weights are fp8 e4m3 hi/lo pairs (weights pre-scaled x32
    # on the host so e4m3's mantissa is used; the x32 scales cancel in the
    # exp scale and the x32 ones-column). Q = xh@wh + xh@wl + xl@wh via
    # DoubleRow matmuls: contraction 256/matmul at 0.5 cycles/col.
    xh_d = nc.dram_tensor("xTh", [128, MO, S], f8, kind="ExternalInput")
    xl_d = nc.dram_tensor("xTl", [128, MO, S], f8, kind="ExternalInput")
    # wqk grouped per QKV chain (ctj = ct*2+j) so each chain's weights are
    # one contiguous DMA
    wqkh_d = nc.dram_tensor("wqkh", [128, 4, MO, 128], f8, kind="ExternalInput")
    wqkl_d = nc.dram_tensor("wqkl", [128, 4, MO, 128], f8, kind="ExternalInput")
    wvh_d = nc.dram_tensor("wvh", [128, MO, CLOC], f8, kind="ExternalInput")
    wvl_d = nc.dram_tensor("wvl", [128, MO, CLOC], f8, kind="ExternalInput")
    wo_d = nc.dram_tensor("wo", [128, 2, DM], bf16, kind="ExternalInput")
    # consts packed as raw bf16 columns: tri[0:128], identity[128:256],
    # bq[256:260], bk[260:264] (f32 values bit-split across bf16 pairs)
    cst_d = nc.dram_tensor("cst", [128, 264], bf16, kind="ExternalInput")
    y_d = nc.dram_tensor("y", [S, DM], bf16, kind="ExternalOutput")

    with tile.TileContext(nc) as tc:
        with (
            tc.tile_pool(name="consts", bufs=1) as consts,
            tc.tile_pool(name="acts", bufs=1) as apool,
            tc.tile_pool(name="pt", bufs=pt_bufs) as ppool,
            tc.tile_pool(name="zq", bufs=zq_bufs) as zqpool,
            tc.tile_pool(name="norm", bufs=4) as spool,
            tc.tile_pool(name="ycopy", bufs=y_bufs) as ypool,
            # 8 PSUM banks: ps_s 2x[128,1024]=4 (QK logits), ps_z
            # 2x[128,4,65]=2 (z accumulators), ps_b 2x[128,512]=2
            # (QKV / V / out-proj chains and z transposes)
            tc.tile_pool(name="ps_s", bufs=2, space="PSUM") as ps_s,
            tc.tile_pool(name="ps_z", bufs=zp_bufs, space="PSUM") as ps_z,
            tc.tile_pool(name="ps_b", bufs=2, space="PSUM") as ps_b,
        ):
            csb = consts.tile([128, 264], bf16)
            wqkh = consts.tile([128, 4, MO, 128], f8)
            wqkl = consts.tile([128, 4, MO, 128], f8)
            wvh = consts.tile([128, MO, CLOC], f8)
            wvl = consts.tile([128, MO, CLOC], f8)
            wo = consts.tile([128, 2, DM], bf16)
            xh = apool.tile([128, MO, S], f8)
            xl = apool.tile([128, MO, S], f8)

            # DMA order = consumption order (transfers serialize on the DMA
            # engines). Startup fans out over three queues (SP: weights,
            # ACT: xh, Pool/SWDGE: xl) so the first QK chain can start ~2.6us in;
            # bulk transfers stay off the ACT queue once the exp stream
            # starts (each dma_start occupies the sequencer ~660ns).
            nc.sync.dma_start(wqkh[:, 0, :, :], wqkh_d[:, 0, :, :])
            nc.scalar.dma_start(xh[:, 0:2, 0:512], xh_d[:, 0:2, 0:512])
            nc.gpsimd.dma_start(xl[:, 0:2, 0:512], xl_d[:, 0:2, 0:512])
            nc.sync.dma_start(wqkl[:, 0, :, :], wqkl_d[:, 0, :, :])
            nc.scalar.dma_start(xh[:, 2:4, 0:512], xh_d[:, 2:4, 0:512])
            nc.gpsimd.dma_start(xl[:, 2:4, 0:512], xl_d[:, 2:4, 0:512])
            nc.sync.dma_start(wqkh[:, 1, :, :], wqkh_d[:, 1, :, :])
            nc.scalar.dma_start(xh[:, 4:8, 0:512], xh_d[:, 4:8, 0:512])
            nc.gpsimd.dma_start(xl[:, 4:8, 0:512], xl_d[:, 4:8, 0:512])
            nc.sync.dma_start(csb[:], cst_d[:])
            nc.sync.dma_start(wqkl[:, 1, :, :], wqkl_d[:, 1, :, :])
            nc.sync.dma_start(wvh[:], wvh_d[:])
            nc.sync.dma_start(wvl[:], wvl_d[:])
            nc.sync.dma_start(wqkh[:, 2:4, :, :], wqkh_d[:, 2:4, :, :])
            nc.sync.dma_start(wqkl[:, 2:4, :, :], wqkl_d[:, 2:4, :, :])
            nc.sync.dma_start(xh[:, :, 512:1024], xh_d[:, :, 512:1024])
            nc.sync.dma_start(xl[:, :, 512:1024], xl_d[:, :, 512:1024])
            nc.sync.dma_start(xh[:, :, 1024:1536], xh_d[:, :, 1024:1536])
            nc.sync.dma_start(xl[:, :, 1024:1536], xl_d[:, :, 1024:1536])
            nc.sync.dma_start(xh[:, :, 1536:2048], xh_d[:, :, 1536:2048])
            nc.sync.dma_start(xl[:, :, 1536:2048], xl_d[:, :, 1536:2048])
            nc.sync.dma_start(wo[:], wo_d[:])

            tri = csb[:, 0:128]
            ident = csb[:, 128:256]
            bq_sb = csb[:, 256:260].bitcast(f32)
            bk_sb = csb[:, 260:264].bitcast(f32)

            QT = apool.tile([128, 2, S], bf16)
            KT = apool.tile([128, 2, S], bf16)
            # V augmented: [t-part, kt, h, 0:64] = v dims (x32), col 64 = 32
            # so the rowsum scale matches the v columns and the x32 cancels
            # in the normalization
            VA = apool.tile([128, NKT, HLOC, 72], bf16)
            nc.vector.memset(VA[:, :, :, 64:65], 32.0)
            zT = apool.tile([128, 2, S], bf16)

            # ---- filler queue: PE work units drained while ScalarE exps ----
            fillers = []          # list of (key, pe_ns, thunk)
            fill_debt = [0.0]
            fill_count = [0]      # fill() invocations, for z-pop delay
            z_pushed_at = {}      # (qg, h) -> fill_count at push
            rr_state = [0]

            def _z_ok(k):
                return (fill_count[0] - z_pushed_at.get((k[1], k[2]), -(1 << 30))
                        >= z_delay)

            cur_qg = [0]

            def _pop_next(op_ok=True):
                """z thunks once ScalarE has had time to produce their exp
                inputs (z_delay fill periods after push), then QKV (needed
                by the next quarter anyway), out-proj last. op thunks are
                hoarded for the late ACT-bound quarters (qg >= op_from),
                where they are the only filler class left."""
                for cls in ("z", "v", "qkv"):
                    for i, (k, ns, thunk) in enumerate(fillers):
                        if k[0] == cls and (cls != "z" or _z_ok(k)):
                            return fillers.pop(i)
                if op_ok:
                    for i, (k, ns, thunk) in enumerate(fillers):
                        if k[0] == "op":
                            return fillers.pop(i)
                    return fillers.pop(0) if fillers else None
                return None

            def fill(budget_ns, op_ok=True):
                fill_count[0] += 1
                budget = budget_ns + fill_debt[0]
                spent = 0.0
                while fillers and spent < budget:
                    nxt = _pop_next(op_ok)
                    if nxt is None:
                        # only hoarded op (or unready z) left: stop without
                        # banking debt so qg2's first fill doesn't burst
                        fill_debt[0] = 0.0
                        return
                    _, ns, thunk = nxt
                    thunk()
                    spent += ns
                fill_debt[0] = budget - spent if fillers else 0.0

            def drain(key):
                """Force-emit queued fillers matching key (dependency
                barrier: attention on quarter qg needs all of QKV(tg=qg))."""
                rest = []
                for k, ns, thunk in fillers:
                    if k == key:
                        thunk()
                    else:
                        rest.append((k, ns, thunk))
                fillers[:] = rest

            def drain_class(cls):
                rest = []
                for k, ns, thunk in fillers:
                    if k[0] == cls:
                        thunk()
                    else:
                        rest.append((k, ns, thunk))
                fillers[:] = rest

            qkv_terms = ((0, 0), (0, 1), (1, 0))  # (x lo?, w lo?) per term

            def emit_qk_chain(tg, ct, j):
                tsl = slice(tg * 512, (tg + 1) * 512)
                dst, b_sb = ((QT, bq_sb), (KT, bk_sb))[j]
                ctj = ct * 2 + j
                ps = ps_b.tile([128, 512], f32, tag="b",
                               name=f"qk_{tg}_{ct}_{j}")
                for sub in range(2):
                    t0 = tg * 512 + sub * 256
                    i = 0
                    for xlo, wlo in qkv_terms:
                        xs = (xh, xl)[xlo]
                        ws = (wqkh, wqkl)[wlo]
                        for c in range(MO // 2):
                            nc.tensor.matmul(
                                ps[:, sub * 256 : sub * 256 + 256],
                                ws[:, ctj, 2 * c : 2 * c + 2, :],
                                xs[:, 2 * c : 2 * c + 2, t0 : t0 + 256],
                                start=(i == 0),
                                stop=(i == 11),
                                perf_mode=DR,
                            )
                            i += 1
                nc.vector.tensor_scalar_add(
                    dst[:, ct, tsl], ps[:], b_sb[:, ct : ct + 1]
                )

            def emit_v_chain(tg, ti):
                tt = tg * 4 + ti
                ps = ps_b.tile([128, 512], f32, tag="b", name=f"v_{tg}_{ti}")
                i = 0
                for xlo, wlo in qkv_terms:
                    xs = (xh, xl)[xlo]
                    ws = (wvh, wvl)[wlo]
                    for c in range(MO // 2):
                        nc.tensor.matmul(
                            ps[:, 0:CLOC],
                            xs[:, 2 * c : 2 * c + 2, tt * 128 : (tt + 1) * 128],
                            ws[:, 2 * c : 2 * c + 2, :],
                            start=(i == 0),
                            stop=(i == 11),
                            perf_mode=DR,
                        )
                        i += 1
                nc.vector.tensor_copy(
                    VA[:, tt, :, 0:64],
                    ps[:, 0:CLOC].rearrange("p (h d) -> p h d", d=64),
                )

            def emit_qkv(tg):
                for ct in range(2):
                    for j in range(2):
                        emit_qk_chain(tg, ct, j)
                for ti in range(4):
                    emit_v_chain(tg, ti)

            def push_qkv_fillers(tg):
                for ct in range(2):
                    for j in range(2):
                        fillers.append(
                            (("qkv", tg), 3072 * PE_NS,
                             lambda tg=tg, ct=ct, j=j: emit_qk_chain(tg, ct, j))
                        )
                vcls = "v" if defer_v else "qkv"
                for ti in range(4):
                    fillers.append(
                        ((vcls, tg), 1536 * PE_NS,
                         lambda tg=tg, ti=ti: emit_v_chain(tg, ti))
                    )

            # ---- attention ----
            def emit_sgrp(h, qg, gi, grp):
                """S^T matmuls for one exp group + the exp + diag masks."""
                hp = (h % 2) * 64
                ct = h // 2
                g0 = qg * 512
                cum = grp[-1][1] + grp[-1][2]
                sreg = ps_s.tile([128, sreg_w], f32, tag="s",
                                 name=f"s_{h}_{qg}_{gi}")
                for kt, off, w in grp:
                    q0 = g0 + 512 - w
                    c0 = off
                    while c0 < off + w:
                        cw = min(off + w - c0, 512 - c0 % 512)
                        nc.tensor.matmul(
                            sreg[:, c0 : c0 + cw],
                            KT[hp : hp + 64, ct, kt * 128 : (kt + 1) * 128],
                            QT[hp : hp + 64, ct,
                               q0 + c0 - off : q0 + c0 - off + cw],
                        )
                        c0 += cw
                pT = ppool.tile([128, sreg_w], bf16, tag="pT",
                                name=f"pT_{h}_{qg}_{gi}")
                no_diag = all(kt * 128 < g0 for kt, _, _ in grp)
                if gi < schr_groups[qg] and no_diag:
                    # Schraudolph exp on DVE: bf16 bits of exp(x*0.125) ~=
                    # int16(x*(0.125*128/ln2) + (127*128 - 5.6)); offloads
                    # the ACT engine (the attention-band pacer) at ~2% rms
                    # error on this group's P entries. sreg is 1024x (Q,K
                    # each carry the x32 weight prescale).
                    nc.vector.tensor_scalar(
                        pT[:, :cum].bitcast(mybir.dt.int16), sreg[:, :cum],
                        scalar1=23.0831253 / 1024.0, op0=mybir.AluOpType.mult,
                        scalar2=16250.4, op1=mybir.AluOpType.add)
                else:
                    nc.scalar.activation(pT[:, :cum], sreg[:, :cum], EXP,
                                         scale=0.125 / 1024.0)
                teng = nc.gpsimd if tri_engine == "gpsimd" else nc.vector
                for kt, off, w in grp:
                    if kt * 128 >= g0:  # diagonal block leads the span
                        teng.tensor_mul(
                            pT[:, off : off + 128],
                            pT[:, off : off + 128],
                            tri[:],
                        )
                return pT

            def emit_zchain(h, qg, qt, pts, kt2g, zp):
                """z[q,65] = sum_kt pT_chunk^T @ V_aug: one sequential PSUM
                accumulation chain per q-tile (a PSUM bank supports only one
                open accumulation group at a time)."""
                g0 = qg * 512
                qa = 4 * qg + qt
                for kt in range(qa + 1):
                    gi, off, w = kt2g[kt]
                    q0 = g0 + 512 - w
                    c0 = off + (g0 + qt * 128) - q0
                    nc.tensor.matmul(
                        zp[:, qt, 0:65],
                        pts[gi][:, c0 : c0 + 128],
                        VA[:, kt, h, 0:65],
                        start=(kt == 0),
                        stop=(kt == qa),
                    )

            def emit_norm(h, qg, zp, zq):
                """1/rowsum fused into the PSUM->SBUF copy of z."""
                hp = (h % 2) * 64
                rec = spool.tile([128, 4, 1], f32, tag="rec",
                                 name=f"rec_{h}_{qg}")
                nc.vector.reciprocal(rec[:], zp[:, :, 64:65])
                with nc.allow_low_precision(reason="attn out to bf16"):
                    for qt in range(4):
                        nc.vector.tensor_scalar_mul(
                            zq[:, qt, hp : hp + 64],
                            zp[:, qt, 0:64],
                            rec[:, qt, :],
                        )

            def emit_transpose(qg, pair, zq):
                """zq [q,128d] -> zT [128d, q]. All these transposes are
                latency-tolerant (op(qg) reads zT several us later), so they
                ride the DMA xbar instead of PE+DVE; only the tail-critical
                qg3/pair1 path (in the tailpipe) uses the PE transpose."""
                if dma_transpose:
                    for qt in range(4):
                        tt = qg * 4 + qt
                        nc.sync.dma_start_transpose(
                            zT[:, pair, tt * 128 : (tt + 1) * 128],
                            zq[:, qt, :],
                        )
                    return
                quad = ps_b.tile([128, 4, 128], bf16, tag="b",
                                 name=f"tq_{qg}_{pair}")
                for qt in range(4):
                    nc.tensor.transpose(quad[:, qt, :], zq[:, qt, :], ident)
                with nc.allow_low_precision(reason="zT copy"):
                    nc.vector.tensor_copy(
                        zT[:, pair, qg * 512 : (qg + 1) * 512],
                        quad[:].rearrange("p a b -> p (a b)"),
                    )

            def emit_op_half(qg, nh, ti, yA):
                tt = qg * 4 + ti
                ps = ps_b.tile([128, 512], f32, tag="b",
                               name=f"opA_{qg}_{nh}_{ti}")
                nc.tensor.matmul(
                    ps[:],
                    zT[:, 0, tt * 128 : (tt + 1) * 128],
                    wo[:, 0, nh * 512 : (nh + 1) * 512],
                )
                nc.vector.tensor_copy(yA[:, ti, :], ps[:])

            def emit_op_chunk(qg, nh, ti, ysb, dma_split, yA=None,
                              pool=None):
                tt = qg * 4 + ti
                # tail op chunks borrow ps_s (free once the last exp group
                # is consumed) so 4 chunks pipeline instead of 2
                ps = (pool or ps_b).tile([128, 512], f32,
                                         tag="b" if pool is None else "s",
                                         name=f"op_{qg}_{nh}_{ti}")
                cos = (1,) if yA is not None else (0, 1)
                for co in cos:
                    nc.tensor.matmul(
                        ps[:],
                        zT[:, co, tt * 128 : (tt + 1) * 128],
                        wo[:, co, nh * 512 : (nh + 1) * 512],
                        start=(co == cos[0]),
                        stop=(co == 1),
                    )
                with nc.allow_low_precision(reason="y partial to bf16"):
                    if yA is not None:
                        nc.vector.tensor_add(ysb[:, ti, :], ps[:],
                                             yA[:, ti, :])
                    elif qg == 3:
                        # tail: alternate engines by (ti+nh) so each q-tile's
                        # two chunk copies run on different engines in
                        # parallel instead of serializing on one
                        (nc.scalar.copy if (ti + nh) % 2 == 0
                         else nc.vector.tensor_copy)(ysb[:, ti, :], ps[:])
                    else:
                        ((nc.scalar.copy if (ti + nh) % 2 == 0
                          else nc.vector.tensor_copy) if alt_copy
                         else nc.vector.tensor_copy)(ysb[:, ti, :], ps[:])
                nper = 4 // dma_split
                if ti % nper == nper - 1:
                    t0 = tt - nper + 1
                    # y DMA queue: Pool/SWDGE keeps descriptor gen off the
                    # ACT/SP sequencers and HWDGE during the exp-critical
                    # band; in the qg3 tail (exp done) rotate across all
                    # three so gens parallelize
                    if qg == 3:
                        deng = (nc.gpsimd, nc.sync, nc.scalar)[
                            (ti // nper + nh * dma_split) % 3]
                    else:
                        deng = (nc.gpsimd, nc.sync)[(ti // nper + nh) % 2]
                    deng.dma_start(
                        y_d[t0 * 128 : (tt + 1) * 128,
                            nh * 512 : (nh + 1) * 512].rearrange(
                            "(ti p) n -> p ti n", p=128
                        ),
                        ysb[:, ti - nper + 1 : ti + 1, :],
                    )

            yA_box = {}

            def push_opA_fillers(qg):
                # first half (co=0) of qg3's out-proj: available right after
                # pair 0's transpose, staged to SBUF f32; the co=1 half plus
                # an add happens in the tail
                for nh in range(2):
                    yA = ypool.tile([128, 4, 512], f32, tag="yA", bufs=2,
                                    name=f"yA_{qg}_{nh}")
                    yA_box[(qg, nh)] = yA
                    for ti in range(4):
                        fillers.append(
                            (("op", qg), 512 * PE_NS,
                             lambda qg=qg, nh=nh, ti=ti, yA=yA:
                                 emit_op_half(qg, nh, ti, yA))
                        )

            def push_op_fillers(qg, split=False):
                dma_split = dma_splits[qg]
                for nh in range(2):
                    ysb = ypool.tile([128, 4, 512], bf16, tag="y",
                                     name=f"ysb_{qg}_{nh}")
                    yA = yA_box.get((qg, nh)) if split else None
                    for ti in range(4):
                        fillers.append(
                            (("op", qg), (512 if split else 1024) * PE_NS,
                             lambda qg=qg, nh=nh, ti=ti, ysb=ysb,
                                    ds=dma_split, yA=yA:
                                 emit_op_chunk(qg, nh, ti, ysb, ds, yA))
                        )

            def push_z_phase(qg, h, pts, kt2g, zq_box):
                """Queue head h's z chains + normalize (+ transpose) at the
                FRONT of the filler queue; they drain during head h+1's S
                phase (one-head software pipeline)."""
                box = {}
                # final unit (qg3 h3): per-q-tile norm -> transpose -> zT
                # copy -> that tile's out-proj chunks, so the tail pipelines
                # instead of serializing behind the full head
                tailpipe = qg == 3 and h == HLOC - 1

                def chain(qt):
                    if qt == 0:
                        box["zp"] = ps_z.tile([128, 4, 65], f32, tag="z",
                                              name=f"zp_{h}_{qg}")
                        if h % 2 == 0:
                            zq_box[h // 2] = zqpool.tile(
                                [128, 4, 128], bf16, tag="zq",
                                name=f"zq_{qg}_{h // 2}")
                    emit_zchain(h, qg, qt, pts, kt2g, box["zp"])
                    if tailpipe:
                        zq = zq_box[h // 2]
                        if qt == 0:
                            box["rec"] = spool.tile([128, 4, 1], f32,
                                                    tag="rec",
                                                    name=f"rec_{h}_{qg}")
                            box["ysb"] = [
                                ypool.tile([128, 4, 512], bf16, tag="y",
                                           name=f"ysb_3_{nh}")
                                for nh in range(2)
                            ]
                        rec = box["rec"]
                        zp = box["zp"]
                        nc.vector.reciprocal(rec[:, qt, :], zp[:, qt, 64:65])
                        with nc.allow_low_precision(reason="attn out bf16"):
                            nc.vector.tensor_scalar_mul(
                                zq[:, qt, 64:128], zp[:, qt, 0:64],
                                rec[:, qt, :])
                        quad = ps_b.tile([128, 128], bf16, tag="b",
                                         name=f"tq3_{qt}")
                        nc.tensor.transpose(quad[:], zq[:, qt, :], ident)
                        tt = qg * 4 + qt
                        with nc.allow_low_precision(reason="zT copy"):
                            nc.vector.tensor_copy(
                                zT[:, 1, tt * 128 : (tt + 1) * 128], quad[:])
                        for nh in range(2):
                            fillers.append(
                                (("op", 3), 1024 * PE_NS,
                                 lambda nh=nh, ti=qt:
                                     emit_op_chunk(3, nh, ti,
                                                   box["ysb"][nh],
                                                   dma_splits[3],
                                                   pool=ps_s))
                            )

                def norm():
                    if tailpipe:
                        return
                    emit_norm(h, qg, box["zp"], zq_box[h // 2])
                    if h % 2 == 1:
                        emit_transpose(qg, h // 2, zq_box[h // 2])
                        if h == 1 and qg == 3 and op3_split:
                            push_opA_fillers(qg)
                        if h == HLOC - 1:
                            # quarter finished: queue its out-proj (reads
                            # zT(qg), complete as of this point) and the
                            # next token group's QKV
                            push_op_fillers(qg, split=(qg == 3 and op3_split))
                            if qg + 2 <= 3:
                                push_qkv_fillers(qg + 2)

                # z thunks of heads <= h-2 must fully precede this head's
                # (ps_z has 2 bufs, so h-1's z may still be queued); h's
                # thunks append after h-1's so class-FIFO order holds
                prev = ("z", qg, h - 1)
                rest = []
                for k, ns, thunk in fillers:
                    if k[0] == "z" and k != prev:
                        thunk()
                    else:
                        rest.append((k, ns, thunk))
                fillers[:] = rest
                thunks = []
                for qt in range(4):
                    ncols = (4 * qg + qt + 1) * 65
                    thunks.append(
                        (("z", qg, h), ncols * PE_NS,
                         lambda qt=qt: chain(qt))
                    )
                thunks.append((("z", qg, h), 0.0, norm))
                idx = 0
                for i, (k, _, _) in enumerate(fillers):
                    if k[0] == "z":
                        idx = i + 1
                fillers[idx:idx] = thunks
                z_pushed_at[(qg, h)] = fill_count[0]

            # PE p-state warmup: the PE reaches full clock only after 3us
            # of continuous execution; burn the initial DMA-wait window
            # with dummy matmuls on zeroed scratch so the real chains run
            # at full speed from the start
            if warmup > 0:
                # tiny memset so the ramp starts ~1.5us earlier; the moving
                # operand reads zT (first written ~15us in, so no WAR delay)
                # and the garbage results land in psum that start=True
                # chains later overwrite
                wsrc = apool.tile([128, 128], bf16)
                nc.vector.memset(wsrc[:], 0.0)
                for i in range(warmup):
                    wps = ps_b.tile([128, 512], f32, tag="b",
                                    name=f"warm_{i}")
                    w = 512 if i < warmup - 2 else 256
                    nc.tensor.matmul(wps[:, 0:w], wsrc[:],
                                     zT[:, 0, 0:w])

            # ---- program ----
            # ct0's Q,K chains (heads 0,1) emit directly so quarter 0's S/exp
            # stream starts as soon as possible; ct1 + V chains become
            # fillers drained during h0/h1's exp (barriers: v at h==1,
            # qkv(ct1) at h==2).
            for j in range(2):
                emit_qk_chain(0, 0, j)
            for jj in range(2):
                fillers.append(
                    (("qkv", 0), 3072 * PE_NS,
                     lambda jj=jj: emit_qk_chain(0, 1, jj))
                )
            for ti in range(4):
                fillers.append(
                    (("v", 0), 1536 * PE_NS,
                     lambda ti=ti: emit_v_chain(0, ti))
                )
            push_qkv_fillers(1)

            zq_box = {}
            for qg in range(4):
                groups = _groups(qg, sreg_w)
                kt2g = {}
                for gi, grp in enumerate(groups):
                    for kt, off, w in grp:
                        kt2g[kt] = (gi, off, w)
                if qg > 0:
                    # barrier: this quarter's S/z read QT/KT/VA of tg=qg
                    drain(("qkv", qg))
                for h in range(HLOC):
                    if h == 1:
                        drain(("v", qg))
                    if h == 2 and qg == 0:
                        drain(("qkv", 0))  # ct1 chains gate heads 2,3
                    pts = []
                    # emit S groups in pairs (back-to-back on PE) so ACT's
                    # exp stream has at most one bubble per pair, not per
                    # group; the 2-buffer S rotation permits exactly 2 ahead
                    qsc, qpad = ((fill_scale, fill_pad) if fill_cfg is None
                                 else fill_cfg[qg])
                    pqg = pair_s if isinstance(pair_s, bool) else pair_s[qg]
                    step = 2 if pqg else 1
                    for g0i in range(0, len(groups), step):
                        pair = groups[g0i : g0i + step]
                        budget = 0.0
                        for gi, grp in zip(range(g0i, g0i + step), pair):
                            pts.append(emit_sgrp(h, qg, gi, grp))
                            cum = grp[-1][1] + grp[-1][2]
                            budget += ((cum * ACT_NS + 185.0) * qsc
                                       + qpad - cum * PE_NS)
                        fill(max(0.0, budget), op_ok=(qg >= op_from))
                    push_z_phase(qg, h, pts, kt2g, zq_box)

            # drain the tail (queue can grow while draining)
            while fillers:
                nxt = _pop_next(True)
                _, _, thunk = nxt if nxt else fillers.pop(0)
                thunk()

    nc.compile()
    return nc


def _pack_w(w):
    # [DM, C] -> [128, MO, C] f32: partition p holds rows {mo*128 + p}
    return np.ascontiguousarray(
        w.reshape(MO, 128, w.shape[1]).transpose(1, 0, 2)
    ).astype(np.float32)


def _split8(a):
    # f32 array -> (hi, lo) e4m3 pair with hi + lo ~= a to ~0.1%
    ah = a.astype(ml_dtypes.float8_e4m3)
    al = (a - ah.astype(np.float32)).astype(ml_dtypes.float8_e4m3)
    return np.ascontiguousarray(ah), np.ascontiguousarray(al)


def make_in_maps(x, w_qkv, b_qkv, w_out):
    # multiplicative post-exp mask: 1 where k <= q (upper incl diag), else 0
    tri = np.tri(128, 128, 0, dtype=np.float32).T.astype(ml_dtypes.bfloat16)
    ident = np.eye(128, dtype=np.float32).astype(ml_dtypes.bfloat16)
    in_maps = []
    for core in range(8):
        b = core // 4
        hg = core % 4
        c0 = hg * CLOC
        csl = slice(c0, c0 + CLOC)

        # packed consts: [128, 264] bf16-typed raw columns. Biases carry the
        # x32 weight prescale (Q,K live at 32x on device).
        cst = np.zeros((128, 264), np.uint16)
        cst[:, 0:128] = tri.view(np.uint16)
        cst[:, 128:256] = ident.view(np.uint16)
        bq = np.ascontiguousarray(
            32.0 * b_qkv[csl].astype(np.float32).reshape(2, 128).T
        )
        bk = np.ascontiguousarray(
            32.0 * b_qkv[DM + c0 : DM + c0 + CLOC].astype(np.float32)
            .reshape(2, 128).T
        )
        cst[:, 256:260] = bq.view(np.uint16).reshape(128, 4)
        cst[:, 260:264] = bk.view(np.uint16).reshape(128, 4)

        wq_p = _pack_w(32.0 * w_qkv[:, csl])
        wk_p = _pack_w(32.0 * w_qkv[:, DM + c0 : DM + c0 + CLOC])
        # [128, ctj, MO, 128]: ctj = ct*2 + j (j=0 -> Q, j=1 -> K)
        wqk = np.concatenate(
            [wq_p[:, None, :, 0:128], wk_p[:, None, :, 0:128],
             wq_p[:, None, :, 128:256], wk_p[:, None, :, 128:256]],
            axis=1,
        )
        wqkh, wqkl = _split8(wqk)
        wvh, wvl = _split8(
            _pack_w(32.0 * w_qkv[:, 2 * DM + c0 : 2 * DM + c0 + CLOC]))
        xTh, xTl = _split8(_pack_w(np.ascontiguousarray(x[b].T)))
        in_maps.append(
            {
                "xTh": xTh,
                "xTl": xTl,
                "wqkh": wqkh,
                "wqkl": wqkl,
                "wvh": wvh,
                "wvl": wvl,
                # wo: [CLOC, DM] -> [128, 2, DM]
                "wo": np.ascontiguousarray(
                    w_out[csl, :].reshape(2, 128, DM).transpose(1, 0, 2)
                ).astype(ml_dtypes.bfloat16),
                "cst": cst.view(ml_dtypes.bfloat16),
            }
        )
    return in_maps


def gather(results, b_qkv, w_out, b_out):
    # device skips the V bias; z_norm + b_v projects to a constant row:
    # y += b_v @ w_out, folded into the output bias here
    b_eff = (
        b_out.astype(np.float32)
        + b_qkv[2 * DM :].astype(np.float32) @ w_out.astype(np.float32)
    )
    out = np.empty((B, S, DM), np.float32)
    for b in range(B):
        acc = results[4 * b]["y"].astype(np.float32)
        for j in range(1, 4):
            acc = acc + results[4 * b + j]["y"]
        out[b] = acc + b_eff[None, :]
    return out


def kernel(x, w_qkv, b_qkv, w_out, b_out):
    x = np.asarray(x)
    w_qkv = np.asarray(w_qkv)
    b_qkv = np.asarray(b_qkv)
    w_out = np.asarray(w_out)
    b_out = np.asarray(b_out)

    if "nc" not in _CACHE:
        _CACHE["nc"] = build()
    nc = _CACHE["nc"]

    in_maps = make_in_maps(x, w_qkv, b_qkv, w_out)
    res = run_bass_kernel_spmd(nc, in_maps, core_ids=list(range(8)))
    return gather(res.results, b_qkv, w_out, b_out)

